# revision 41
# baseline (speedup 1.0000x reference)
"""MoMKE (multimodal MoE transformer) forward on 8 trn2 NeuronCores.

Sharding: pure data-parallel over batch (B=16 -> 2 batch elements per core).
Each core runs the full model on its 2 sequences; no collectives.

On-device layout: activations are feature-major ("transposed", [feature,
token]) so weights in natural [in, out] layout serve directly as matmul
lhsT.  Tokens per core: 768 columns, ordered col = b*384 + m*128 + s for
local batch b in {0,1}, modality m in {a,t,v}, position s.

Scheduling: PE warm-up matmul stream during the initial input DMA;
expert loop software-pipelined (down-proj of expert e issued after
up-proj of e+1) with expert up-weights resident in SBUF per layer;
attention runs a cross-pair pipeline (QK matmuls of pair p+1 issued
before AV of pair p; adjacent heads at partition bases 0/64 execute
concurrently in separate PE row groups); router/gating phase is emitted
inside layer-0's attention window so its matmuls fill the PE while the
scalar engine does softmax; PSUM is organized as 4x single-bank + 2x
double-bank tiles; softmax reciprocal is Exp(-Ln(x)) on the scalar
engine (Ln/Exp share one ACT table); E[x^2] for each LayerNorm is
hoisted into the preceding phase; the residual stream is natively f32r
so stat matmuls need no dtype-copy; out-projection is n0-major so LN2
statistics start on the first token half early; elementwise work is
split across vector/gpsimd/scalar to keep the PE streaming.
"""

import numpy as np

B, S = 16, 128
ADIM, TDIM, VDIM = 512, 768, 1024
DE = 512
DEPTH = 4
NH = 8
HD = 64
E = 6
NCLS = 6
EPS = 1e-5
NCORES = 8
BL = B // NCORES          # local batch: 2
NTOK = BL * 3 * S         # 768 tokens/core
NH2 = 384                 # half of token columns (matmul N tile)

_CACHE = {}


def _f32(a):
    return np.ascontiguousarray(np.asarray(a, dtype=np.float32))


def _bf16(a):
    import ml_dtypes
    return np.ascontiguousarray(
        np.asarray(a, dtype=np.float32).astype(ml_dtypes.bfloat16))


def _fp8i(a):
    """[..., K=512, M] -> DoubleRow-interleaved fp8 [..., 128, 2, 2, M]."""
    import ml_dtypes
    a = np.asarray(a, dtype=np.float32)
    lead, (K, M) = a.shape[:-2], a.shape[-2:]
    assert K == 512
    b = a.reshape(*lead, 2, 2, 128, M)
    b = np.moveaxis(b, -2, -4)
    return np.ascontiguousarray(b.astype(ml_dtypes.float8_e4m3fn))


def _split_waits(nc, mybir):
    """This walrus build accepts at most one sync wait / one sync update per
    ISA instruction; Tile's sem assignment can attach several.  Spread the
    extras onto same-engine no-ops."""
    n = 0
    for bb in nc.main_func.blocks:
        insts = list(bb.instructions)
        out = []
        changed = False
        for ins in insts:
            si = ins.sync_info
            if si is None:
                out.append(ins)
                continue
            waits = list(si.on_wait or [])
            updates = list(si.on_update or [])
            post = []
            if len(waits) > 1 or len(updates) > 1:
                for w in waits[:-1]:
                    n += 1
                    nop = mybir.InstNoOp(name=f"xw-{n}", ins=[], outs=[])
                    nop.engine = ins.engine
                    nop.sync_info = mybir.SyncInfo(on_wait=[w], on_update=[])
                    out.append(nop)
                for u in updates[1:]:
                    n += 1
                    nop = mybir.InstNoOp(name=f"xu-{n}", ins=[], outs=[])
                    nop.engine = ins.engine
                    nop.sync_info = mybir.SyncInfo(on_wait=[], on_update=[u])
                    post.append(nop)
                ins.sync_info = mybir.SyncInfo(on_wait=waits[-1:],
                                               on_update=updates[:1])
                changed = True
            out.append(ins)
            out.extend(post)
        if changed:
            bb.instructions[:] = out
    return n


def _build():
    import concourse.bass as bass
    import concourse.mybir as mybir
    import concourse.tile as tile

    f32 = mybir.dt.float32
    fp8 = mybir.dt.float8e4
    f32r = mybir.dt.float32r
    bf16 = mybir.dt.bfloat16
    AF = mybir.ActivationFunctionType
    ALU = mybir.AluOpType
    AX = mybir.AxisListType

    nc = bass.Bass()

    d = {}

    def din(name, shape, dt):
        d[name] = nc.dram_tensor(name, shape, dt, kind="ExternalInput")

    din("ones_s", [128, 128], f32r)
    din("aT", [ADIM, BL * S], f32r)
    din("tT", [TDIM, BL * S], f32r)
    din("vT", [VDIM, BL * S], f32r)
    din("wa", [ADIM, DE], f32r)
    din("wt", [TDIM, DE], f32r)
    din("wv", [VDIM, DE], f32r)
    din("bin_r", [3, 4, 128], f32)
    din("wr1", [3, DE, DE], f32r)
    din("br1_r", [3, 4, 128], f32)
    din("wr2", [3, DE, E], f32r)
    din("br2_b", [3, 128, E], f32)
    din("wqkv", [DEPTH, DE, 3 * DE], bf16)
    din("bqkv_qk", [DEPTH, 8, 128], f32)
    din("bqkv_v", [DEPTH, 1, DE], bf16)
    din("wo", [DEPTH, DE, DE], bf16)
    din("bo_r", [DEPTH, 4, 128], f32)
    din("ws1", [DEPTH, DE, DE], bf16)
    din("bs1_r", [DEPTH, 4, 128], f32)
    din("ws2", [DEPTH, DE, DE], bf16)
    din("we1", [DEPTH, E, DE, DE], bf16)
    din("be1_r", [DEPTH, 24, 128], f32)
    din("we2", [DEPTH, E, DE, DE], bf16)
    din("bmat", [DEPTH, 7, DE], bf16)
    din("wp1", [3 * DE, 3 * DE], bf16)
    din("bp1_row", [1, 3 * DE], bf16)
    din("wh", [3 * DE, NCLS], bf16)
    din("bh_r", [NCLS, 1], f32)
    din("onesrow", [1, 128], bf16)
    din("ident", [128, 128], bf16)
    din("ones_b", [128, 128], bf16)
    din("sel", [E, E * 128], bf16)
    out_d = nc.dram_tensor("out", [NCLS, BL], f32, kind="ExternalOutput")

    with tile.TileContext(nc) as tc:
        _emit(nc, tc, d, out_d, f32, f32r, bf16, fp8, AF, ALU, AX)

    nfix = _split_waits(nc, mybir)
    return nc, nfix


def _emit(nc, tc, d, out_d, f32, f32r, bf16, fp8, AF, ALU, AX):
    from concourse.bass import ds

    import concourse.mybir as mybir
    DR = mybir.MatmulPerfMode.DoubleRow
    dma = nc.sync.dma_start

    def r32(ap):
        return ap.bitcast(f32r)

    cst = tc.alloc_tile_pool(name="cst", bufs=1)
    ph0 = tc.alloc_tile_pool(name="ph0", bufs=1, side="right")  # phase 0/1, released early
    wbig = tc.alloc_tile_pool(name="wbig", bufs=2)
    w512 = tc.alloc_tile_pool(name="w512", bufs=4)
    wexp = tc.alloc_tile_pool(name="wexp", bufs=2)
    px = tc.alloc_tile_pool(name="px", bufs=1)
    ph = tc.alloc_tile_pool(name="ph", bufs=2)
    pxb = tc.alloc_tile_pool(name="pxb", bufs=1)
    pqk = tc.alloc_tile_pool(name="pqk", bufs=1)
    pva = tc.alloc_tile_pool(name="pva", bufs=1)
    pes = tc.alloc_tile_pool(name="pes", bufs=2)
    psm = tc.alloc_tile_pool(name="psm", bufs=2)
    pln = tc.alloc_tile_pool(name="pln", bufs=4)
    peh = tc.alloc_tile_pool(name="peh", bufs=2)
    psml = tc.alloc_tile_pool(name="psml", bufs=4)
    psA = tc.alloc_tile_pool(name="psA", bufs=4, space="PSUM")
    psB = tc.alloc_tile_pool(name="psB", bufs=2, space="PSUM")

    # ---------------- PE warm-up: stream matmuls while input DMAs land ----
    ident = cst.tile([128, 128], bf16)
    dma(ident[:], d["ident"][:])
    bin_r = cst.tile([128, 3, 4], f32)
    dma(bin_r[:], d["bin_r"].rearrange("m c p -> p m c"))
    ones_b = cst.tile([128, 128], bf16)
    dma(ones_b[:], d["ones_b"][:])
    pwu = psA.tile([128, 128], f32, tag="a", name="warmup")
    for _ in range(64):
        nc.tensor.matmul(pwu[:], ident[:], ident[:], start=True, stop=True)

    eps_sb = cst.tile([128, 1], f32)
    nc.gpsimd.memset(eps_sb[:], EPS)
    G_sb = cst.tile([128, E, NTOK], bf16)        # per-expert gate rows, bcast
    gaug = cst.tile([7, NTOK], bf16)             # gates^T rows + ones row
    nc.gpsimd.memset(gaug[:], 1.0)  # rows 0..5 overwritten by gate evictions

    xT = px.tile([128, 4, NTOK], f32r)           # residual stream

    # =======================================================
    # Phase 0: input projections
    # =======================================================
    def in_proj(dname, wname, idim, m):
        kc = idim // 128
        pa = [psA.tile([128, BL * S], f32, tag="a", name=f"pa{i}") for i in range(4)]
        for k in range(kc):
            it = ph0.tile([128, BL * S], f32r, tag="it", bufs=3, name=f"it{m}{k}")
            dma(it[:], d[dname].rearrange("(c p) t -> p c t", p=128)[:, k, :])
            wi = ph0.tile([128, DE], f32r, tag="wi", bufs=3, name=f"wi{m}{k}")
            dma(wi[:], d[wname].rearrange("(c p) o -> p c o", p=128)[:, k, :])
            for mo in range(4):
                nc.tensor.matmul(pa[mo][:], wi[:, ds(mo * 128, 128)], it[:],
                                 start=(k == 0), stop=(k == kc - 1))
        for mo in range(4):
            dst = xT[:, mo, :].rearrange("p (b r) -> p b r", b=BL)[:, :, ds(m * 128, 128)]
            nc.vector.tensor_scalar(out=dst,
                                    in0=pa[mo][:].rearrange("p (b s) -> p b s", b=BL),
                                    scalar1=bin_r[:, m, mo:mo + 1], scalar2=None,
                                    op0=ALU.add)

    in_proj("aT", "wa", ADIM, 0)
    in_proj("tT", "wt", TDIM, 1)
    wq0_t = []
    for k in range(4):
        wqk = wbig.tile([128, 3 * DE], bf16, tag="wq", bufs=4, name=f"wq{k}")
        dma(wqk[:], d["wqkv"][0, ds(k * 128, 128), :])
        wq0_t.append(wqk)
    in_proj("vT", "wv", VDIM, 2)

    # ---------------- remaining constants (after the input DMAs) ----------
    ones_s = cst.tile([128, 128], f32r)          # 1/512 everywhere
    dma(ones_s[:], d["ones_s"][:])
    onesrow = cst.tile([1, 128], bf16)
    dma(onesrow[:], d["onesrow"][:])
    sel_sb = cst.tile([E, E * 128], bf16)
    dma(sel_sb[:], d["sel"][:])
    br1_r = cst.tile([128, 3, 4], f32)
    dma(br1_r[:], d["br1_r"].rearrange("m c p -> p m c"))
    wr2_sb = cst.tile([128, 3, 4, E], f32r)
    dma(wr2_sb[:], d["wr2"].rearrange("m (c p) e -> p m c e", p=128))
    br2_b = cst.tile([128, 3, E], f32)
    dma(br2_b[:], d["br2_b"].rearrange("m p e -> p m e"))
    bqkv_qk = cst.tile([128, DEPTH, 8], f32)
    dma(bqkv_qk[:], d["bqkv_qk"].rearrange("l c p -> p l c"))
    bo_r = cst.tile([128, DEPTH, 4], f32)
    dma(bo_r[:], d["bo_r"].rearrange("l c p -> p l c"))
    bs1_r = cst.tile([128, DEPTH, 4], f32)
    dma(bs1_r[:], d["bs1_r"].rearrange("l c p -> p l c"))
    be1_r = cst.tile([128, DEPTH, 24], f32)
    dma(be1_r[:], d["be1_r"].rearrange("l c p -> p l c"))
    bh_sb = cst.tile([NCLS, 1], f32)
    dma(bh_sb[:], d["bh_r"][:])

    va = pva.tile([128, 6, 8 * 128], bf16)
    nc.gpsimd.memset(
        va[:].rearrange("p t (h w) -> p t h w", h=NH)[:, :, :, 64:128], 1.0)

    # =======================================================
    # layers
    # =======================================================
    def make_xsq():
        return pxb.tile([128, 4, NTOK], bf16, tag="xsq", name="xsq")

    def emit_xsq(xsq, src, n0):
        nsl = ds(n0 * NH2, NH2)
        nc.vector.tensor_tensor(xsq[:, :, nsl], src[:, :, nsl],
                                src[:, :, nsl], op=ALU.mult)

    def layernorm_n0(src_f32, dst_slice, xsq, n0):
        if True:
            nsl = ds(n0 * NH2, NH2)
            pst = psB.tile([128, 2, 512], f32, tag="b")
            for k in range(4):
                nc.tensor.matmul(pst[:, 0, 0:NH2], ones_s[:], src_f32[:, k, nsl],
                                 start=(k == 0), stop=(k == 3))
            for k in range(4):
                nc.tensor.matmul(pst[:, 1, 0:NH2], ones_b[:], xsq[:, k, nsl],
                                 start=(k == 0), stop=(k == 3))
            mb = pln.tile([128, NH2], f32, tag="ln", bufs=3)
            nc.scalar.activation(mb[:], pst[:, 0, 0:NH2], AF.Copy)
            qq = pln.tile([128, NH2], f32, tag="ln", bufs=3)
            nc.gpsimd.tensor_tensor(qq[:], mb[:], mb[:], op=ALU.mult)
            vb = pln.tile([128, NH2], f32, tag="ln", bufs=3)
            nc.vector.scalar_tensor_tensor(out=vb[:], in0=pst[:, 1, 0:NH2],
                                           scalar=1.0, in1=qq[:],
                                           op0=ALU.mult, op1=ALU.subtract)
            sq = pln.tile([128, NH2], f32, tag="ln", bufs=3)
            nc.scalar.activation(sq[:], vb[:], AF.Ln, bias=eps_sb[:])
            rb = pln.tile([128, NH2], f32, tag="ln", bufs=3)
            nc.scalar.activation(rb[:], sq[:], AF.Exp, scale=-0.5)
            for k in range(4):
                eng = nc.vector if k < 2 else nc.gpsimd
                t = pln.tile([128, NH2], f32, tag="lt", bufs=2)
                eng.tensor_tensor(t[:], src_f32[:, k, nsl], mb[:],
                                  op=ALU.subtract)
                eng.tensor_tensor(dst_slice(k, nsl), t[:], rb[:],
                                  op=ALU.mult)

    def layernorm(src_f32, dst_slice, xsq):
        """src feature-major [128,4,NTOK] fp32 -> dst (via dst_slice(k, nsl)).
        Mean/E[x^2] via ones-matmul (broadcast across partitions); xsq
        precomputed (hoisted into the previous phase for overlap)."""
        for n0 in range(2):
            layernorm_n0(src_f32, dst_slice, xsq, n0)

    def emit_phase1():
        # Phase 1: routers + gates (emitted inside layer-0's attention
        # window so router matmuls fill the PE while ACT does softmax)
        rh = ph0.tile([128, 4, 3, BL * S], f32r, tag="hr", bufs=1)
        for m in range(3):
            wr1_sb = wr1_t[m]
            xm = xT[:].rearrange("p c (b mm s) -> p c b mm s", b=BL, mm=3)[:, :, :, m, :]
            for mo in range(4):
                pr = psA.tile([128, BL * S], f32, tag="a")
                for k in range(4):
                    nc.tensor.matmul(pr[:].rearrange("p (b s) -> p b s", b=BL),
                                     wr1_sb[:, k, ds(mo * 128, 128)], xm[:, k, :, :],
                                     start=(k == 0), stop=(k == 3))
                nc.scalar.activation(rh[:, mo, m, :], pr[:], AF.Gelu_apprx_tanh,
                                     bias=br1_r[:, m, mo:mo + 1])

        rlog = cst.tile([128, 6, E], f32)
        for m in range(3):
            for b in range(BL):
                q = b * 3 + m
                prl = psA.tile([128, E], f32, tag="a")
                for k in range(4):
                    nc.tensor.matmul(prl[:], rh[:, k, m, ds(b * 128, 128)],
                                     wr2_sb[:, m, k, :], start=(k == 0), stop=(k == 3))
                nc.vector.tensor_tensor(rlog[:, q, :], prl[:], br2_b[:, m, :], op=ALU.add)

        for q in range(6):
            r = rlog[:, q, :]
            v1 = pln.tile([128, 1], f32, tag="sc")
            nc.vector.tensor_reduce(v1[:], r, op=ALU.max, axis=AX.X)
            m1 = pln.tile([128, E], f32, tag="m6")
            nc.vector.tensor_scalar(out=m1[:], in0=r, scalar1=v1[:], scalar2=None,
                                    op0=ALU.is_equal)
            mk = pln.tile([128, E], f32, tag="m6")
            nc.vector.scalar_tensor_tensor(out=mk[:], in0=m1[:], scalar=-1e9,
                                           in1=r, op0=ALU.mult, op1=ALU.add)
            v2 = pln.tile([128, 1], f32, tag="sc")
            nc.vector.tensor_reduce(v2[:], mk[:], op=ALU.max, axis=AX.X)
            m2 = pln.tile([128, E], f32, tag="m6")
            nc.vector.tensor_scalar(out=m2[:], in0=mk[:], scalar1=v2[:], scalar2=None,
                                    op0=ALU.is_equal)
            dd = pln.tile([128, 1], f32, tag="sc")
            nc.vector.tensor_tensor(dd[:], v1[:], v2[:], op=ALU.subtract)
            g1 = pln.tile([128, 1], f32, tag="sc")
            nc.scalar.activation(g1[:], dd[:], AF.Sigmoid)
            g2 = pln.tile([128, 1], f32, tag="sc")
            nc.vector.tensor_scalar(out=g2[:], in0=g1[:], scalar1=-1.0, scalar2=1.0,
                                    op0=ALU.mult, op1=ALU.add)
            gm2 = pln.tile([128, E], f32, tag="m6")
            nc.vector.tensor_scalar(out=gm2[:], in0=m2[:], scalar1=g2[:], scalar2=None,
                                    op0=ALU.mult)
            gq = pln.tile([128, E], bf16, tag="m6b")
            nc.vector.scalar_tensor_tensor(out=gq[:], in0=m1[:], scalar=g1[:],
                                           in1=gm2[:], op0=ALU.mult, op1=ALU.add)
            pt = psA.tile([E, 128], bf16, tag="a", name="ptg")
            nc.tensor.transpose(pt[:], gq[:], ident[:])
            nc.scalar.activation(gaug[0:6, ds(q * 128, 128)], pt[:], AF.Copy)

        for e in range(E):
            for n0 in range(2):
                pg = psA.tile([128, NH2], f32, tag="a")
                nc.tensor.matmul(pg[:], sel_sb[:, ds(e * 128, 128)],
                                 gaug[0:6, ds(n0 * NH2, NH2)], start=True, stop=True)
                nc.scalar.activation(G_sb[:, e, ds(n0 * NH2, NH2)], pg[:], AF.Copy)

        ph0.release()

        wp1p = tc.alloc_tile_pool(name="wp1p", bufs=1, side="right")
        return wp1p

    for layer in range(DEPTH):
        if layer == 0:
            wq_t = wq0_t
        else:
            wq_t = []
            for k in range(4):
                wqk = wbig.tile([128, 3 * DE], bf16, tag="wq", bufs=4,
                                name=f"wq{k}")
                dma(wqk[:], d["wqkv"][layer, ds(k * 128, 128), :])
                wq_t.append(wqk)
        bqv = psml.tile([1, DE], bf16, tag="bqv", bufs=1)
        dma(bqv[:], d["bqkv_v"][layer])
        bmat_sb = psml.tile([7, DE], bf16, tag="bm", bufs=1)
        dma(bmat_sb[:], d["bmat"][layer])
        wo_sb = w512.tile([128, 4, DE], bf16, tag="w")
        dma(wo_sb[:], d["wo"][layer].rearrange("(c p) o -> p c o", p=128))
        ws1_sb = w512.tile([128, 4, DE], bf16, tag="w")
        dma(ws1_sb[:], d["ws1"][layer].rearrange("(c p) o -> p c o", p=128))
        ws2_sb = w512.tile([128, 4, DE], bf16, tag="w")
        dma(ws2_sb[:], d["ws2"][layer].rearrange("(c p) o -> p c o", p=128))
        if layer == 0:
            wr1_t = []
            for m in range(3):
                wr1_sb = ph0.tile([128, 4, DE], f32r, tag="wr", bufs=2,
                                  name="wr1_sb")
                dma(wr1_sb[:], d["wr1"][m].rearrange("(c p) o -> p c o", p=128))
                wr1_t.append(wr1_sb)

        # expert weights: resident for the whole layer, loaded once
        we_t = []
        for e in range(E):
            w1 = wexp.tile([128, 4, DE], bf16, tag="we", bufs=7, name=f"w1_{e}")
            dma(w1[:], d["we1"][layer, e].rearrange("(c p) o -> p c o", p=128))
            we_t.append(w1)
        if layer == DEPTH - 1:
            bp1_row = wp1p.tile([1, 3 * DE], bf16, tag="bp1")
            dma(bp1_row[:], d["bp1_row"][:])
            wp1_sb = []
            for g in range(4):
                wpg = wp1p.tile([128, 3, 3 * DE], bf16, tag="wp", bufs=3,
                                name=f"wp{g}")
                dma(wpg[:], d["wp1"].rearrange("(c p) o -> p c o", p=128)
                    [:, ds(g * 3, 3), :])
                wp1_sb.append(wpg)

        # ---- LN1 ----
        if layer == 0:
            xsq_next = make_xsq()
            emit_xsq(xsq_next, xT, 0)
            emit_xsq(xsq_next, xT, 1)
        hT = ph.tile([128, 4, NTOK], bf16, tag="h", bufs=2, name="hT")
        layernorm(xT, lambda k, nsl: hT[:, k, nsl], xsq_next)

        # ---- qkv: q,k feature-major ----
        qkT = pqk.tile([128, 8, NTOK], bf16)
        for mo in range(8):
            pq2 = [psA.tile([128, NH2], f32, tag="a", name=f"pq{i}") for i in range(2)]
            for k in range(4):
                for n0 in range(2):
                    nc.tensor.matmul(pq2[n0][:], wq_t[k][:, ds(mo * 128, 128)],
                                     hT[:, k, ds(n0 * NH2, NH2)],
                                     start=(k == 0), stop=(k == 3))
            for n0 in range(2):
                nc.scalar.activation(qkT[:, mo, ds(n0 * NH2, NH2)], pq2[n0][:],
                                     AF.Identity,
                                     bias=bqkv_qk[:, layer, mo:mo + 1])
        # ---- v token-major, bias via rank-1, into V_aug (pairs of tq) ----
        for tp in range(3):
            pv = psB.tile([128, 2, 512], f32, tag="b")
            for j in range(2):
                tq = tp * 2 + j
                for k in range(4):
                    nc.tensor.matmul(pv[:, j, :], hT[:, k, ds(tq * 128, 128)],
                                     wq_t[k][:, ds(2 * DE, DE)],
                                     start=(k == 0), stop=False)
                nc.tensor.matmul(pv[:, j, :], onesrow[0:1, 0:128], bqv[:],
                                 start=False, stop=True)
            for j in range(2):
                tq = tp * 2 + j
                dst = va[:, tq, :].rearrange("p (h w) -> p h w", h=NH)[:, :, 0:64]
                nc.scalar.activation(dst,
                                     pv[:, j, :].rearrange("p (h e) -> p h e", h=NH),
                                     AF.Copy)

        # ---- attention per (b, head): cross-pair software pipeline ----
        oT = ph.tile([128, 4, NTOK], bf16, tag="h", bufs=2, name="oT")
        prs = [(b, hp) for b in range(BL) for hp in range(NH // 2)]
        est = {}

        def qk_stage(pi):
            b, hp = prs[pi]
            pks, ess = [], []
            for h in (2 * hp, 2 * hp + 1):
                r0 = 64 * (h % 2)
                ck = 4 + h // 2
                qs = qkT[ds(r0, 64), h // 2, ds(b * 384, 384)]
                pk2 = psB.tile([128, 2, 512], f32, tag="b")
                for i in range(2):
                    nc.tensor.matmul(pk2[:, i, 0:NH2],
                                     qkT[ds(r0, 64), ck,
                                         ds(b * 384 + i * 128, 128)],
                                     qs, start=True, stop=True)
                pk1 = psA.tile([128, NH2], f32, tag="a")
                nc.tensor.matmul(pk1[:],
                                 qkT[ds(r0, 64), ck, ds(b * 384 + 256, 128)],
                                 qs, start=True, stop=True)
                pks.append((pk2, pk1))
            for j in range(2):
                pk2, pk1 = pks[j]
                es = pes.tile([128, 3, NH2], bf16, name=f"es{j}")
                nc.scalar.activation(es[:, 0:2, :], pk2[:, :, 0:NH2],
                                     AF.Exp, scale=0.125)
                nc.scalar.activation(es[:, 2, :], pk1[:], AF.Exp, scale=0.125)
                ess.append(es)
            est[pi] = ess

        def av_stage(pi):
            b, hp = prs[pi]
            ess = est[pi]
            spair = psm.tile([128, NH2], f32, tag="s")
            opair = psm.tile([128, NH2], f32, tag="o", bufs=1)
            for j, h in enumerate((2 * hp, 2 * hp + 1)):
                r0 = 64 * (h % 2)
                es = ess[j]
                po = psA.tile([128, NH2], f32, tag="a", name=f"po{h%2}")
                for i in range(3):
                    nc.tensor.matmul(po[:], va[:, b * 3 + i, ds(h * 128, 128)],
                                     es[:, i, :], start=(i == 0), stop=(i == 2))
                nc.vector.tensor_copy(spair[ds(r0, 64), :], po[ds(64, 64), :])
                nc.vector.tensor_copy(opair[ds(r0, 64), :], po[ds(0, 64), :])
            rcp = psm.tile([128, NH2], f32, tag="s")
            if pi % 4 == 1:
                nc.vector.reciprocal(rcp[:], spair[:])
            else:
                sln = psm.tile([128, NH2], f32, tag="s")
                nc.scalar.activation(sln[:], spair[:], AF.Ln)
                nc.scalar.activation(rcp[:], sln[:], AF.Exp, scale=-1.0)
            for h in (2 * hp, 2 * hp + 1):
                r0 = 64 * (h % 2)
                nc.gpsimd.tensor_tensor(oT[ds(r0, 64), h // 2, ds(b * 384, 384)],
                                        opair[ds(r0, 64), :], rcp[ds(r0, 64), :],
                                        op=ALU.mult)

        qk_stage(0)
        for pi in range(1, len(prs)):
            qk_stage(pi)
            av_stage(pi - 1)
        av_stage(len(prs) - 1)

        if layer == 0:
            wp1p = emit_phase1()

        # ---- attention out-projection + residual (n0-major so the n0=0
        # half of xT finalizes early and LN2 stats can start) ----
        xsq2 = make_xsq()
        for n0 in range(2):
            nsl = ds(n0 * NH2, NH2)
            for mo in range(4):
                pp = psA.tile([128, NH2], f32, tag="a", name="pp")
                for k in range(4):
                    nc.tensor.matmul(pp[:], wo_sb[:, k, ds(mo * 128, 128)],
                                     oT[:, k, nsl], start=(k == 0), stop=(k == 3))
                nc.vector.scalar_tensor_tensor(out=xT[:, mo, nsl], in0=pp[:],
                                               scalar=bo_r[:, layer, mo:mo + 1],
                                               in1=xT[:, mo, nsl],
                                               op0=ALU.add, op1=ALU.add)
            emit_xsq(xsq2, xT, n0)

        # ---- LN2 ----
        h2 = ph.tile([128, 4, NTOK], bf16, tag="h", bufs=2, name="h2")
        layernorm(xT, lambda k, nsl: h2[:, k, nsl], xsq2)

        # ---- MoE: shared expert + 6 gated experts, software-pipelined ----
        xsq_next = make_xsq()
        for n0 in range(2):
            nsl = ds(n0 * NH2, NH2)
            pd = [psA.tile([128, NH2], f32, tag="a", name=f"pd{i}") for i in range(4)]
            su = peh.tile([128, 4, NH2], bf16, tag="eh")
            for p in range(2):
                pu = psB.tile([128, 2, 512], f32, tag="b")
                for j in range(2):
                    mo = p * 2 + j
                    for k in range(4):
                        nc.tensor.matmul(pu[:, j, 0:NH2],
                                         ws1_sb[:, k, ds(mo * 128, 128)],
                                         h2[:, k, nsl], start=(k == 0), stop=(k == 3))
                    nc.scalar.activation(su[:, mo, :], pu[:, j, 0:NH2],
                                         AF.Gelu_apprx_tanh,
                                         bias=bs1_r[:, layer, mo:mo + 1])
            for mo in range(4):
                for k in range(4):
                    nc.tensor.matmul(pd[mo][:], ws2_sb[:, k, ds(mo * 128, 128)],
                                     su[:, k, :], start=(k == 0), stop=False)
                nc.tensor.matmul(pd[mo][:], bmat_sb[:, ds(mo * 128, 128)],
                                 gaug[:, nsl], start=False, stop=False)

            ehs = []

            def up_expert(e):
                w1 = we_t[e]
                eh = peh.tile([128, 4, NH2], bf16, tag="eh", name=f"eh{e%2}")
                for p in range(2):
                    pu = psB.tile([128, 2, 512], f32, tag="b")
                    for j in range(2):
                        mo = p * 2 + j
                        for k in range(4):
                            nc.tensor.matmul(pu[:, j, 0:NH2],
                                             w1[:, k, ds(mo * 128, 128)],
                                             h2[:, k, nsl],
                                             start=(k == 0), stop=(k == 3))
                        nc.scalar.activation(
                            eh[:, mo, :], pu[:, j, 0:NH2],
                            AF.Gelu_apprx_tanh,
                            bias=be1_r[:, layer, e * 4 + mo:e * 4 + mo + 1])
                    eng = nc.vector if p == 0 else nc.gpsimd
                    for j in range(2):
                        mo = p * 2 + j
                        eng.tensor_tensor(eh[:, mo, :], eh[:, mo, :],
                                          G_sb[:, e, nsl], op=ALU.mult)
                ehs.append(eh)

            def down_expert(e):
                w2 = wexp.tile([128, 4, DE], bf16, tag="we2", bufs=3, name=f"w2_{e}")
                dma(w2[:], d["we2"][layer, e].rearrange("(c p) o -> p c o", p=128))
                eh = ehs[e]
                last = (e == E - 1)
                for mo in range(4):
                    for k in range(4):
                        nc.tensor.matmul(pd[mo][:], w2[:, k, ds(mo * 128, 128)],
                                         eh[:, k, :], start=False,
                                         stop=(last and k == 3))

            up_expert(0)
            for e in range(1, E):
                up_expert(e)
                down_expert(e - 1)
            down_expert(E - 1)

            for mo in range(4):
                nc.vector.tensor_tensor(xT[:, mo, nsl], pd[mo][:],
                                        xT[:, mo, nsl], op=ALU.add)
            emit_xsq(xsq_next, xT, n0)

    # =======================================================
    # final LN + mean-pool + head
    # =======================================================
    fT = ph.tile([128, 4, NTOK], bf16, tag="h", bufs=2, name="fT")
    layernorm(xT, lambda k, nsl: fT[:, k, nsl], xsq_next)

    pooled = wp1p.tile([128, 24], f32, tag="pool")
    pooledb = wp1p.tile([128, 24], bf16, tag="poolb")
    pview = pooled[:].rearrange("p (m k b) -> p b m k", m=3, k=4, b=BL)
    for k in range(4):
        for b in range(BL):
            nc.vector.tensor_reduce(
                pview[:, b, :, k],
                fT[:, k, ds(b * 384, 384)].rearrange("p (m s) -> p m s", m=3),
                op=ALU.add, axis=AX.X)
    nc.vector.tensor_scalar(out=pooledb[:], in0=pooled[:], scalar1=1.0 / S,
                            scalar2=None, op0=ALU.mult)

    # fused = relu(pooled @ Wp1 + bp1), token-major [BL, 1536]
    pfs = [psA.tile([BL, DE], f32, tag="a", name=f"pfs{i}") for i in range(3)]
    for kj in range(12):
        for ns in range(3):
            nc.tensor.matmul(pfs[ns][:], pooledb[:, ds(kj * 2, BL)],
                             wp1_sb[kj // 3][:, kj % 3, ds(ns * DE, DE)],
                             start=(kj == 0), stop=False)
    for ns in range(3):
        nc.tensor.matmul(pfs[ns][:], onesrow[0:1, 0:BL],
                         bp1_row[:, ds(ns * DE, DE)], start=False, stop=True)
    fused_sb = wp1p.tile([BL, 3 * DE], bf16, tag="fus")
    for ns in range(3):
        nc.scalar.activation(fused_sb[:, ds(ns * DE, DE)], pfs[ns][:], AF.Relu)

    fusedT = wp1p.tile([128, 12, BL], bf16, tag="fusT")
    for kj in range(12):
        pft = psB.tile([128, BL], bf16, tag="b", name="pft")
        nc.tensor.transpose(pft[:], fused_sb[:, ds(kj * 128, 128)], ident[0:BL, 0:BL])
        nc.scalar.activation(fusedT[:, kj, :], pft[:], AF.Copy)

    wh_sb = w512.tile([128, 12, NCLS], bf16, tag="w")
    dma(wh_sb[:], d["wh"].rearrange("(c p) o -> p c o", p=128))
    pout = psA.tile([NCLS, BL], f32, tag="a")
    for kj in range(12):
        nc.tensor.matmul(pout[:], wh_sb[:, kj, :], fusedT[:, kj, :],
                         start=(kj == 0), stop=(kj == 11))
    osb = wp1p.tile([NCLS, BL], f32, tag="osb")
    nc.scalar.activation(osb[:], pout[:], AF.Identity, bias=bh_sb[:, 0:1])
    dma(out_d[:], osb[:])

    for pool in [psB, psA, psml, peh, pln, psm, pes, pva, pqk, pxb, ph, px,
                 wexp, w512, wbig, wp1p, cst]:
        pool.release()


def _host_prep(inputs):
    p = {k: np.asarray(v) for k, v in inputs.items()}

    shared = {}
    shared["wa"] = _f32(p["Wa"])
    shared["wt"] = _f32(p["Wt"])
    shared["wv"] = _f32(p["Wv"])
    shared["bin_r"] = _f32(np.stack([p["ba"].reshape(4, 128),
                                     p["bt"].reshape(4, 128),
                                     p["bv"].reshape(4, 128)]))
    shared["wr1"] = _f32(p["Wr1"])
    shared["br1_r"] = _f32(np.asarray(p["br1"]).reshape(3, 4, 128))
    shared["wr2"] = _f32(p["Wr2"])
    shared["br2_b"] = _f32(np.broadcast_to(np.asarray(p["br2"])[:, None, :],
                                           (3, 128, E)))
    shared["wqkv"] = _bf16(p["Wqkv"])
    shared["bqkv_qk"] = _f32(np.asarray(p["bqkv"])[:, :1024].reshape(DEPTH, 8, 128))
    shared["bqkv_v"] = _bf16(np.asarray(p["bqkv"])[:, 1024:].reshape(DEPTH, 1, DE))
    shared["wo"] = _bf16(p["Wo"])
    shared["bo_r"] = _f32(np.asarray(p["bo"]).reshape(DEPTH, 4, 128))
    shared["ws1"] = _bf16(p["Ws1"])
    shared["bs1_r"] = _f32(np.asarray(p["bs1"]).reshape(DEPTH, 4, 128))
    shared["ws2"] = _bf16(p["Ws2"])
    shared["we1"] = _bf16(p["We1"])
    shared["be1_r"] = _f32(np.asarray(p["be1"]).reshape(DEPTH, 24, 128))
    shared["we2"] = _bf16(p["We2"])
    bmat = np.concatenate([np.asarray(p["be2"]),
                           np.asarray(p["bs2"])[:, None, :]], axis=1)
    shared["bmat"] = _bf16(bmat)
    shared["wp1"] = _bf16(p["Wp1"])
    shared["bp1_row"] = _bf16(np.asarray(p["bp1"]).reshape(1, 3 * DE))
    shared["wh"] = _bf16(p["Wh"])
    shared["bh_r"] = _f32(np.asarray(p["bh"]).reshape(NCLS, 1))
    shared["ones_s"] = _f32(np.full((128, 128), 1.0 / DE, np.float32))
    shared["ones_b"] = _bf16(np.full((128, 128), 1.0 / DE, np.float32))
    shared["onesrow"] = _bf16(np.ones((1, 128), np.float32))
    shared["ident"] = _bf16(np.eye(128, dtype=np.float32))
    sel = np.zeros((E, E * 128), np.float32)
    for e in range(E):
        sel[e, e * 128:(e + 1) * 128] = 1.0
    shared["sel"] = _bf16(sel)

    in_maps = []
    for c in range(NCORES):
        sl = slice(BL * c, BL * (c + 1))
        m = dict(shared)
        m["aT"] = _f32(np.asarray(p["audio"])[sl].transpose(2, 0, 1)
                       .reshape(ADIM, BL * S))
        m["tT"] = _f32(np.asarray(p["text"])[sl].transpose(2, 0, 1)
                       .reshape(TDIM, BL * S))
        m["vT"] = _f32(np.asarray(p["visual"])[sl].transpose(2, 0, 1)
                       .reshape(VDIM, BL * S))
        in_maps.append(m)
    return in_maps


def kernel(**inputs):
    from concourse.bass_utils import run_bass_kernel_spmd

    if "nc" not in _CACHE:
        _CACHE["nc"] = _build()
    nc, _ = _CACHE["nc"]

    in_maps = _host_prep(inputs)
    res = run_bass_kernel_spmd(nc, in_maps, core_ids=list(range(NCORES)))
    out = np.empty((B, NCLS), np.float32)
    for c in range(NCORES):
        out[BL * c: BL * (c + 1)] = res.results[c]["out"].T
    return out


# revision 42
# speedup vs baseline: 1.1969x; 1.1969x over previous
"""MoMKE (multimodal MoE transformer) forward on 8 trn2 NeuronCores.

Sharding: pure data-parallel over batch (B=16 -> 2 batch elements per core).
Each core runs the full model on its 2 sequences; no collectives.

On-device layout: activations are feature-major ("transposed", [feature,
token]) so weights in natural [in, out] layout serve directly as matmul
lhsT.  Tokens per core: 768 columns, ordered col = b*384 + m*128 + s for
local batch b in {0,1}, modality m in {a,t,v}, position s.

Scheduling: PE warm-up matmul stream during the initial input DMA;
expert loop software-pipelined (down-proj of expert e issued after
up-proj of e+1) with expert up-weights resident in SBUF per layer;
attention runs a cross-pair pipeline (QK matmuls of pair p+1 issued
before AV of pair p; adjacent heads at partition bases 0/64 execute
concurrently in separate PE row groups); router/gating phase is emitted
inside layer-0's attention window so its matmuls fill the PE while the
scalar engine does softmax; PSUM is organized as 4x single-bank + 2x
double-bank tiles; softmax reciprocal is Exp(-Ln(x)) on the scalar
engine (Ln/Exp share one ACT table); E[x^2] for each LayerNorm is
hoisted into the preceding phase; the residual stream is natively f32r
so stat matmuls need no dtype-copy; out-projection is n0-major so LN2
statistics start on the first token half early; elementwise work is
split across vector/gpsimd/scalar to keep the PE streaming.
"""

import numpy as np

B, S = 16, 128
ADIM, TDIM, VDIM = 512, 768, 1024
DE = 512
DEPTH = 4
NH = 8
HD = 64
E = 6
NCLS = 6
EPS = 1e-5
NCORES = 8
BL = B // NCORES          # local batch: 2
NTOK = BL * 3 * S         # 768 tokens/core
NH2 = 384                 # half of token columns (matmul N tile)

_CACHE = {}


def _f32(a):
    return np.ascontiguousarray(np.asarray(a, dtype=np.float32))


def _bf16(a):
    import ml_dtypes
    return np.ascontiguousarray(
        np.asarray(a, dtype=np.float32).astype(ml_dtypes.bfloat16))


def _fp8i(a):
    """[..., K=512, M] -> DoubleRow-interleaved fp8 [..., 128, 2, 2, M]."""
    import ml_dtypes
    a = np.asarray(a, dtype=np.float32)
    lead, (K, M) = a.shape[:-2], a.shape[-2:]
    assert K == 512
    b = a.reshape(*lead, 2, 2, 128, M)
    b = np.moveaxis(b, -2, -4)
    return np.ascontiguousarray(b.astype(ml_dtypes.float8_e4m3fn))


def _split_waits(nc, mybir):
    """This walrus build accepts at most one sync wait / one sync update per
    ISA instruction; Tile's sem assignment can attach several.  Spread the
    extras onto same-engine no-ops."""
    n = 0
    for bb in nc.main_func.blocks:
        insts = list(bb.instructions)
        out = []
        changed = False
        for ins in insts:
            si = ins.sync_info
            if si is None:
                out.append(ins)
                continue
            waits = list(si.on_wait or [])
            updates = list(si.on_update or [])
            post = []
            if len(waits) > 1 or len(updates) > 1:
                for w in waits[:-1]:
                    n += 1
                    nop = mybir.InstNoOp(name=f"xw-{n}", ins=[], outs=[])
                    nop.engine = ins.engine
                    nop.sync_info = mybir.SyncInfo(on_wait=[w], on_update=[])
                    out.append(nop)
                for u in updates[1:]:
                    n += 1
                    nop = mybir.InstNoOp(name=f"xu-{n}", ins=[], outs=[])
                    nop.engine = ins.engine
                    nop.sync_info = mybir.SyncInfo(on_wait=[], on_update=[u])
                    post.append(nop)
                ins.sync_info = mybir.SyncInfo(on_wait=waits[-1:],
                                               on_update=updates[:1])
                changed = True
            out.append(ins)
            out.extend(post)
        if changed:
            bb.instructions[:] = out
    return n


def _build():
    import concourse.bass as bass
    import concourse.mybir as mybir
    import concourse.tile as tile

    f32 = mybir.dt.float32
    fp8 = mybir.dt.float8e4
    f32r = mybir.dt.float32r
    bf16 = mybir.dt.bfloat16
    AF = mybir.ActivationFunctionType
    ALU = mybir.AluOpType
    AX = mybir.AxisListType

    nc = bass.Bass()

    d = {}

    def din(name, shape, dt):
        d[name] = nc.dram_tensor(name, shape, dt, kind="ExternalInput")

    din("ones_s", [128, 128], f32r)
    din("aT", [ADIM, BL * S], f32r)
    din("tT", [TDIM, BL * S], f32r)
    din("vT", [VDIM, BL * S], f32r)
    din("wa", [ADIM, DE], f32r)
    din("wt", [TDIM, DE], f32r)
    din("wv", [VDIM, DE], f32r)
    din("bin_r", [3, 4, 128], f32)
    din("wr1", [3, DE, DE], f32r)
    din("br1_r", [3, 4, 128], f32)
    din("wr2", [3, DE, E], f32r)
    din("br2_b", [3, 128, E], f32)
    din("wqkv", [DEPTH, DE, 3 * DE], bf16)
    din("bqkv_qk", [DEPTH, 8, 128], f32)
    din("bqkv_v", [DEPTH, 1, DE], bf16)
    din("wo", [DEPTH, DE, DE], bf16)
    din("bo_r", [DEPTH, 4, 128], f32)
    din("ws1", [DEPTH, DE, DE], bf16)
    din("bs1_r", [DEPTH, 4, 128], f32)
    din("ws2", [DEPTH, DE, DE], bf16)
    din("we1", [DEPTH, E, DE, DE], bf16)
    din("be1_r", [DEPTH, 24, 128], f32)
    din("we2", [DEPTH, E, DE, DE], bf16)
    din("bmat", [DEPTH, 7, DE], bf16)
    din("wp1", [3 * DE, 3 * DE], bf16)
    din("bp1_row", [1, 3 * DE], bf16)
    din("wh", [3 * DE, NCLS], bf16)
    din("bh_r", [NCLS, 1], f32)
    din("onesrow", [1, 128], bf16)
    din("ident", [128, 128], bf16)
    din("ones_b", [128, 128], bf16)
    din("sel", [E, E * 128], bf16)
    out_d = nc.dram_tensor("out", [NCLS, BL], f32, kind="ExternalOutput")

    with tile.TileContext(nc) as tc:
        _emit(nc, tc, d, out_d, f32, f32r, bf16, fp8, AF, ALU, AX)

    nfix = _split_waits(nc, mybir)
    return nc, nfix


def _emit(nc, tc, d, out_d, f32, f32r, bf16, fp8, AF, ALU, AX):
    from concourse.bass import ds

    import concourse.mybir as mybir
    DR = mybir.MatmulPerfMode.DoubleRow
    dma = nc.sync.dma_start

    def r32(ap):
        return ap.bitcast(f32r)

    cst = tc.alloc_tile_pool(name="cst", bufs=1)
    ph0 = tc.alloc_tile_pool(name="ph0", bufs=1, side="right")  # phase 0/1, released early
    wbig = tc.alloc_tile_pool(name="wbig", bufs=2)
    w512 = tc.alloc_tile_pool(name="w512", bufs=4)
    wexp = tc.alloc_tile_pool(name="wexp", bufs=2)
    px = tc.alloc_tile_pool(name="px", bufs=1)
    ph = tc.alloc_tile_pool(name="ph", bufs=2)
    pxb = tc.alloc_tile_pool(name="pxb", bufs=1)
    pqk = tc.alloc_tile_pool(name="pqk", bufs=1)
    pva = tc.alloc_tile_pool(name="pva", bufs=1)
    pes = tc.alloc_tile_pool(name="pes", bufs=2)
    psm = tc.alloc_tile_pool(name="psm", bufs=2)
    pln = tc.alloc_tile_pool(name="pln", bufs=4)
    peh = tc.alloc_tile_pool(name="peh", bufs=2)
    psml = tc.alloc_tile_pool(name="psml", bufs=4)
    psA = tc.alloc_tile_pool(name="psA", bufs=4, space="PSUM")
    psB = tc.alloc_tile_pool(name="psB", bufs=2, space="PSUM")

    # ---------------- PE warm-up: stream matmuls while input DMAs land ----
    ident = cst.tile([128, 128], bf16)
    dma(ident[:], d["ident"][:])
    bin_r = cst.tile([128, 3, 4], f32)
    dma(bin_r[:], d["bin_r"].rearrange("m c p -> p m c"))
    ones_b = cst.tile([128, 128], bf16)
    dma(ones_b[:], d["ones_b"][:])
    pwu = psA.tile([128, 128], f32, tag="a", name="warmup")
    for _ in range(64):
        nc.tensor.matmul(pwu[:], ident[:], ident[:], start=True, stop=True)

    eps_sb = cst.tile([128, 1], f32)
    nc.gpsimd.memset(eps_sb[:], EPS)
    G_sb = cst.tile([128, E, NTOK], bf16)        # per-expert gate rows, bcast
    gaug = cst.tile([7, NTOK], bf16)             # gates^T rows + ones row
    nc.gpsimd.memset(gaug[:], 1.0)  # rows 0..5 overwritten by gate evictions

    xT = px.tile([128, 4, NTOK], f32r)           # residual stream

    # =======================================================
    # Phase 0: input projections
    # =======================================================
    def in_proj(dname, wname, idim, m):
        kc = idim // 128
        pa = [psA.tile([128, BL * S], f32, tag="a", name=f"pa{i}") for i in range(4)]
        for k in range(kc):
            it = ph0.tile([128, BL * S], f32r, tag="it", bufs=3, name=f"it{m}{k}")
            dma(it[:], d[dname].rearrange("(c p) t -> p c t", p=128)[:, k, :])
            wi = ph0.tile([128, DE], f32r, tag="wi", bufs=3, name=f"wi{m}{k}")
            dma(wi[:], d[wname].rearrange("(c p) o -> p c o", p=128)[:, k, :])
            for mo in range(4):
                nc.tensor.matmul(pa[mo][:], wi[:, ds(mo * 128, 128)], it[:],
                                 start=(k == 0), stop=(k == kc - 1))
        for mo in range(4):
            dst = xT[:, mo, :].rearrange("p (b r) -> p b r", b=BL)[:, :, ds(m * 128, 128)]
            nc.vector.tensor_scalar(out=dst,
                                    in0=pa[mo][:].rearrange("p (b s) -> p b s", b=BL),
                                    scalar1=bin_r[:, m, mo:mo + 1], scalar2=None,
                                    op0=ALU.add)

    in_proj("aT", "wa", ADIM, 0)
    in_proj("tT", "wt", TDIM, 1)
    wq0_t = []
    for k in range(4):
        wqk = wbig.tile([128, 3 * DE], bf16, tag="wq", bufs=4, name=f"wq{k}")
        dma(wqk[:], d["wqkv"][0, ds(k * 128, 128), :])
        wq0_t.append(wqk)
    in_proj("vT", "wv", VDIM, 2)

    # ---------------- remaining constants (after the input DMAs) ----------
    ones_s = cst.tile([128, 128], f32r)          # 1/512 everywhere
    dma(ones_s[:], d["ones_s"][:])
    onesrow = cst.tile([1, 128], bf16)
    dma(onesrow[:], d["onesrow"][:])
    sel_sb = cst.tile([E, E * 128], bf16)
    dma(sel_sb[:], d["sel"][:])
    br1_r = cst.tile([128, 3, 4], f32)
    dma(br1_r[:], d["br1_r"].rearrange("m c p -> p m c"))
    wr2_sb = cst.tile([128, 3, 4, E], f32r)
    dma(wr2_sb[:], d["wr2"].rearrange("m (c p) e -> p m c e", p=128))
    br2_b = cst.tile([128, 3, E], f32)
    dma(br2_b[:], d["br2_b"].rearrange("m p e -> p m e"))
    bqkv_qk = cst.tile([128, DEPTH, 8], f32)
    dma(bqkv_qk[:], d["bqkv_qk"].rearrange("l c p -> p l c"))
    bo_r = cst.tile([128, DEPTH, 4], f32)
    dma(bo_r[:], d["bo_r"].rearrange("l c p -> p l c"))
    bs1_r = cst.tile([128, DEPTH, 4], f32)
    dma(bs1_r[:], d["bs1_r"].rearrange("l c p -> p l c"))
    be1_r = cst.tile([128, DEPTH, 24], f32)
    dma(be1_r[:], d["be1_r"].rearrange("l c p -> p l c"))
    bh_sb = cst.tile([NCLS, 1], f32)
    dma(bh_sb[:], d["bh_r"][:])

    va = pva.tile([128, 6, 8 * 128], bf16)
    nc.gpsimd.memset(
        va[:].rearrange("p t (h w) -> p t h w", h=NH)[:, :, :, 64:128], 1.0)

    # =======================================================
    # layers
    # =======================================================
    def make_xsq():
        return pxb.tile([128, 4, NTOK], bf16, tag="xsq", name="xsq")

    def emit_xsq(xsq, src, n0):
        nsl = ds(n0 * NH2, NH2)
        nc.vector.tensor_tensor(xsq[:, :, nsl], src[:, :, nsl],
                                src[:, :, nsl], op=ALU.mult)

    def layernorm_n0(src_f32, dst_slice, xsq, n0):
        if True:
            nsl = ds(n0 * NH2, NH2)
            pst = psB.tile([128, 2, 512], f32, tag="b")
            for k in range(4):
                nc.tensor.matmul(pst[:, 0, 0:NH2], ones_s[:], src_f32[:, k, nsl],
                                 start=(k == 0), stop=(k == 3))
            for k in range(4):
                nc.tensor.matmul(pst[:, 1, 0:NH2], ones_b[:], xsq[:, k, nsl],
                                 start=(k == 0), stop=(k == 3))
            mb = pln.tile([128, NH2], f32, tag="ln", bufs=3)
            nc.scalar.activation(mb[:], pst[:, 0, 0:NH2], AF.Copy)
            qq = pln.tile([128, NH2], f32, tag="ln", bufs=3)
            nc.gpsimd.tensor_tensor(qq[:], mb[:], mb[:], op=ALU.mult)
            vb = pln.tile([128, NH2], f32, tag="ln", bufs=3)
            nc.vector.scalar_tensor_tensor(out=vb[:], in0=pst[:, 1, 0:NH2],
                                           scalar=1.0, in1=qq[:],
                                           op0=ALU.mult, op1=ALU.subtract)
            sq = pln.tile([128, NH2], f32, tag="ln", bufs=3)
            nc.scalar.activation(sq[:], vb[:], AF.Ln, bias=eps_sb[:])
            rb = pln.tile([128, NH2], f32, tag="ln", bufs=3)
            nc.scalar.activation(rb[:], sq[:], AF.Exp, scale=-0.5)
            for k in range(4):
                eng = nc.vector if k < 2 else nc.gpsimd
                t = pln.tile([128, NH2], f32, tag="lt", bufs=2)
                eng.tensor_tensor(t[:], src_f32[:, k, nsl], mb[:],
                                  op=ALU.subtract)
                eng.tensor_tensor(dst_slice(k, nsl), t[:], rb[:],
                                  op=ALU.mult)

    def layernorm(src_f32, dst_slice, xsq):
        """src feature-major [128,4,NTOK] fp32 -> dst (via dst_slice(k, nsl)).
        Mean/E[x^2] via ones-matmul (broadcast across partitions); xsq
        precomputed (hoisted into the previous phase for overlap)."""
        for n0 in range(2):
            layernorm_n0(src_f32, dst_slice, xsq, n0)

    def emit_phase1():
        # Phase 1: routers + gates (emitted inside layer-0's attention
        # window so router matmuls fill the PE while ACT does softmax)
        rh = ph0.tile([128, 4, 3, BL * S], f32r, tag="hr", bufs=1)
        for m in range(3):
            wr1_sb = wr1_t[m]
            xm = xT[:].rearrange("p c (b mm s) -> p c b mm s", b=BL, mm=3)[:, :, :, m, :]
            for mo in range(4):
                pr = psA.tile([128, BL * S], f32, tag="a")
                for k in range(4):
                    nc.tensor.matmul(pr[:].rearrange("p (b s) -> p b s", b=BL),
                                     wr1_sb[:, k, ds(mo * 128, 128)], xm[:, k, :, :],
                                     start=(k == 0), stop=(k == 3))
                nc.scalar.activation(rh[:, mo, m, :], pr[:], AF.Gelu_apprx_tanh,
                                     bias=br1_r[:, m, mo:mo + 1])

        rlog = cst.tile([128, 6, E], f32)
        for m in range(3):
            for b in range(BL):
                q = b * 3 + m
                prl = psA.tile([128, E], f32, tag="a")
                for k in range(4):
                    nc.tensor.matmul(prl[:], rh[:, k, m, ds(b * 128, 128)],
                                     wr2_sb[:, m, k, :], start=(k == 0), stop=(k == 3))
                nc.vector.tensor_tensor(rlog[:, q, :], prl[:], br2_b[:, m, :], op=ALU.add)

        for q in range(6):
            r = rlog[:, q, :]
            v1 = pln.tile([128, 1], f32, tag="sc")
            nc.vector.tensor_reduce(v1[:], r, op=ALU.max, axis=AX.X)
            m1 = pln.tile([128, E], f32, tag="m6")
            nc.vector.tensor_scalar(out=m1[:], in0=r, scalar1=v1[:], scalar2=None,
                                    op0=ALU.is_equal)
            mk = pln.tile([128, E], f32, tag="m6")
            nc.vector.scalar_tensor_tensor(out=mk[:], in0=m1[:], scalar=-1e9,
                                           in1=r, op0=ALU.mult, op1=ALU.add)
            v2 = pln.tile([128, 1], f32, tag="sc")
            nc.vector.tensor_reduce(v2[:], mk[:], op=ALU.max, axis=AX.X)
            m2 = pln.tile([128, E], f32, tag="m6")
            nc.vector.tensor_scalar(out=m2[:], in0=mk[:], scalar1=v2[:], scalar2=None,
                                    op0=ALU.is_equal)
            dd = pln.tile([128, 1], f32, tag="sc")
            nc.vector.tensor_tensor(dd[:], v1[:], v2[:], op=ALU.subtract)
            g1 = pln.tile([128, 1], f32, tag="sc")
            nc.scalar.activation(g1[:], dd[:], AF.Sigmoid)
            g2 = pln.tile([128, 1], f32, tag="sc")
            nc.vector.tensor_scalar(out=g2[:], in0=g1[:], scalar1=-1.0, scalar2=1.0,
                                    op0=ALU.mult, op1=ALU.add)
            gm2 = pln.tile([128, E], f32, tag="m6")
            nc.vector.tensor_scalar(out=gm2[:], in0=m2[:], scalar1=g2[:], scalar2=None,
                                    op0=ALU.mult)
            gq = pln.tile([128, E], bf16, tag="m6b")
            nc.vector.scalar_tensor_tensor(out=gq[:], in0=m1[:], scalar=g1[:],
                                           in1=gm2[:], op0=ALU.mult, op1=ALU.add)
            pt = psA.tile([E, 128], bf16, tag="a", name="ptg")
            nc.tensor.transpose(pt[:], gq[:], ident[:])
            nc.scalar.activation(gaug[0:6, ds(q * 128, 128)], pt[:], AF.Copy)

        for e in range(E):
            for n0 in range(2):
                pg = psA.tile([128, NH2], f32, tag="a")
                nc.tensor.matmul(pg[:], sel_sb[:, ds(e * 128, 128)],
                                 gaug[0:6, ds(n0 * NH2, NH2)], start=True, stop=True)
                nc.scalar.activation(G_sb[:, e, ds(n0 * NH2, NH2)], pg[:], AF.Copy)

        ph0.release()

        wp1p = tc.alloc_tile_pool(name="wp1p", bufs=1, side="right")
        return wp1p

    for layer in range(DEPTH):
        if layer == 0:
            wq_t = wq0_t
        else:
            wq_t = []
            for k in range(4):
                wqk = wbig.tile([128, 3 * DE], bf16, tag="wq", bufs=4,
                                name=f"wq{k}")
                dma(wqk[:], d["wqkv"][layer, ds(k * 128, 128), :])
                wq_t.append(wqk)
        bqv = psml.tile([1, DE], bf16, tag="bqv", bufs=1)
        dma(bqv[:], d["bqkv_v"][layer])
        bmat_sb = psml.tile([7, DE], bf16, tag="bm", bufs=1)
        dma(bmat_sb[:], d["bmat"][layer])
        wo_sb = w512.tile([128, 4, DE], bf16, tag="w")
        dma(wo_sb[:], d["wo"][layer].rearrange("(c p) o -> p c o", p=128))
        ws1_sb = w512.tile([128, 4, DE], bf16, tag="w")
        dma(ws1_sb[:], d["ws1"][layer].rearrange("(c p) o -> p c o", p=128))
        ws2_sb = w512.tile([128, 4, DE], bf16, tag="w")
        dma(ws2_sb[:], d["ws2"][layer].rearrange("(c p) o -> p c o", p=128))
        if layer == 0:
            wr1_t = []
            for m in range(3):
                wr1_sb = ph0.tile([128, 4, DE], f32r, tag="wr", bufs=2,
                                  name="wr1_sb")
                dma(wr1_sb[:], d["wr1"][m].rearrange("(c p) o -> p c o", p=128))
                wr1_t.append(wr1_sb)

        # expert weights: resident for the whole layer, loaded once
        we_t = []
        for e in range(E):
            w1 = wexp.tile([128, 4, DE], bf16, tag="we", bufs=7, name=f"w1_{e}")
            dma(w1[:], d["we1"][layer, e].rearrange("(c p) o -> p c o", p=128))
            we_t.append(w1)
        if layer == DEPTH - 1:
            bp1_row = wp1p.tile([1, 3 * DE], bf16, tag="bp1")
            dma(bp1_row[:], d["bp1_row"][:])
            wp1_sb = []
            for g in range(4):
                wpg = wp1p.tile([128, 3, 3 * DE], bf16, tag="wp", bufs=3,
                                name=f"wp{g}")
                dma(wpg[:], d["wp1"].rearrange("(c p) o -> p c o", p=128)
                    [:, ds(g * 3, 3), :])
                wp1_sb.append(wpg)

        # ---- LN1 ----
        if layer == 0:
            xsq_next = make_xsq()
            emit_xsq(xsq_next, xT, 0)
            emit_xsq(xsq_next, xT, 1)
        hT = ph.tile([128, 4, NTOK], bf16, tag="h", bufs=2, name="hT")
        layernorm(xT, lambda k, nsl: hT[:, k, nsl], xsq_next)

        # ---- qkv: q,k feature-major ----
        qkT = pqk.tile([128, 8, NTOK], bf16)
        for mo in range(8):
            pq2 = [psA.tile([128, NH2], f32, tag="a", name=f"pq{i}") for i in range(2)]
            for k in range(4):
                for n0 in range(2):
                    nc.tensor.matmul(pq2[n0][:], wq_t[k][:, ds(mo * 128, 128)],
                                     hT[:, k, ds(n0 * NH2, NH2)],
                                     start=(k == 0), stop=(k == 3))
            for n0 in range(2):
                nc.vector.tensor_scalar(out=qkT[:, mo, ds(n0 * NH2, NH2)],
                                        in0=pq2[n0][:],
                                        scalar1=bqkv_qk[:, layer, mo:mo + 1],
                                        scalar2=None, op0=ALU.add)
        # ---- v token-major, bias via rank-1, into V_aug (pairs of tq) ----
        for tp in range(3):
            pv = psB.tile([128, 2, 512], f32, tag="b")
            for j in range(2):
                tq = tp * 2 + j
                for k in range(4):
                    nc.tensor.matmul(pv[:, j, :], hT[:, k, ds(tq * 128, 128)],
                                     wq_t[k][:, ds(2 * DE, DE)],
                                     start=(k == 0), stop=False)
                nc.tensor.matmul(pv[:, j, :], onesrow[0:1, 0:128], bqv[:],
                                 start=False, stop=True)
            for j in range(2):
                tq = tp * 2 + j
                dst = va[:, tq, :].rearrange("p (h w) -> p h w", h=NH)[:, :, 0:64]
                nc.scalar.activation(dst,
                                     pv[:, j, :].rearrange("p (h e) -> p h e", h=NH),
                                     AF.Copy)

        # ---- attention per (b, head): cross-pair software pipeline ----
        oT = ph.tile([128, 4, NTOK], bf16, tag="h", bufs=2, name="oT")
        prs = [(b, hp) for b in range(BL) for hp in range(NH // 2)]
        est = {}

        def qk_stage(pi):
            b, hp = prs[pi]
            pks, ess = [], []
            for h in (2 * hp, 2 * hp + 1):
                r0 = 64 * (h % 2)
                ck = 4 + h // 2
                qs = qkT[ds(r0, 64), h // 2, ds(b * 384, 384)]
                pk2 = psB.tile([128, 2, 512], f32, tag="b")
                for i in range(2):
                    nc.tensor.matmul(pk2[:, i, 0:NH2],
                                     qkT[ds(r0, 64), ck,
                                         ds(b * 384 + i * 128, 128)],
                                     qs, start=True, stop=True)
                pk1 = psA.tile([128, NH2], f32, tag="a")
                nc.tensor.matmul(pk1[:],
                                 qkT[ds(r0, 64), ck, ds(b * 384 + 256, 128)],
                                 qs, start=True, stop=True)
                pks.append((pk2, pk1))
            for j in range(2):
                pk2, pk1 = pks[j]
                es = pes.tile([128, 3, NH2], bf16, name=f"es{j}")
                nc.scalar.activation(es[:, 0:2, :], pk2[:, :, 0:NH2],
                                     AF.Exp, scale=0.125)
                nc.scalar.activation(es[:, 2, :], pk1[:], AF.Exp, scale=0.125)
                ess.append(es)
            est[pi] = ess

        def av_stage(pi):
            b, hp = prs[pi]
            ess = est[pi]
            spair = psm.tile([128, NH2], f32, tag="s")
            opair = psm.tile([128, NH2], f32, tag="o", bufs=1)
            for j, h in enumerate((2 * hp, 2 * hp + 1)):
                r0 = 64 * (h % 2)
                es = ess[j]
                po = psA.tile([128, NH2], f32, tag="a", name=f"po{h%2}")
                for i in range(3):
                    nc.tensor.matmul(po[:], va[:, b * 3 + i, ds(h * 128, 128)],
                                     es[:, i, :], start=(i == 0), stop=(i == 2))
                nc.vector.tensor_copy(spair[ds(r0, 64), :], po[ds(64, 64), :])
                nc.vector.tensor_copy(opair[ds(r0, 64), :], po[ds(0, 64), :])
            sln = psm.tile([128, NH2], f32, tag="s")
            nc.scalar.activation(sln[:], spair[:], AF.Ln)
            rcp = psm.tile([128, NH2], f32, tag="s")
            nc.scalar.activation(rcp[:], sln[:], AF.Exp, scale=-1.0)
            for h in (2 * hp, 2 * hp + 1):
                r0 = 64 * (h % 2)
                nc.gpsimd.tensor_tensor(oT[ds(r0, 64), h // 2, ds(b * 384, 384)],
                                        opair[ds(r0, 64), :], rcp[ds(r0, 64), :],
                                        op=ALU.mult)

        qk_stage(0)
        for pi in range(1, len(prs)):
            qk_stage(pi)
            av_stage(pi - 1)
        av_stage(len(prs) - 1)

        if layer == 0:
            wp1p = emit_phase1()

        # ---- attention out-projection + residual (n0-major so the n0=0
        # half of xT finalizes early and LN2 stats can start) ----
        xsq2 = make_xsq()
        for n0 in range(2):
            nsl = ds(n0 * NH2, NH2)
            for mo in range(4):
                pp = psA.tile([128, NH2], f32, tag="a", name="pp")
                for k in range(4):
                    nc.tensor.matmul(pp[:], wo_sb[:, k, ds(mo * 128, 128)],
                                     oT[:, k, nsl], start=(k == 0), stop=(k == 3))
                nc.vector.scalar_tensor_tensor(out=xT[:, mo, nsl], in0=pp[:],
                                               scalar=bo_r[:, layer, mo:mo + 1],
                                               in1=xT[:, mo, nsl],
                                               op0=ALU.add, op1=ALU.add)
            emit_xsq(xsq2, xT, n0)

        # ---- LN2 ----
        h2 = ph.tile([128, 4, NTOK], bf16, tag="h", bufs=2, name="h2")
        layernorm(xT, lambda k, nsl: h2[:, k, nsl], xsq2)

        # ---- MoE: shared expert + 6 gated experts, software-pipelined ----
        xsq_next = make_xsq()
        for n0 in range(2):
            nsl = ds(n0 * NH2, NH2)
            pd = [psA.tile([128, NH2], f32, tag="a", name=f"pd{i}") for i in range(4)]
            su = peh.tile([128, 4, NH2], bf16, tag="eh")
            for p in range(2):
                pu = psB.tile([128, 2, 512], f32, tag="b")
                for j in range(2):
                    mo = p * 2 + j
                    for k in range(4):
                        nc.tensor.matmul(pu[:, j, 0:NH2],
                                         ws1_sb[:, k, ds(mo * 128, 128)],
                                         h2[:, k, nsl], start=(k == 0), stop=(k == 3))
                    nc.scalar.activation(su[:, mo, :], pu[:, j, 0:NH2],
                                         AF.Gelu_apprx_tanh,
                                         bias=bs1_r[:, layer, mo:mo + 1])
            for mo in range(4):
                for k in range(4):
                    nc.tensor.matmul(pd[mo][:], ws2_sb[:, k, ds(mo * 128, 128)],
                                     su[:, k, :], start=(k == 0), stop=False)
                nc.tensor.matmul(pd[mo][:], bmat_sb[:, ds(mo * 128, 128)],
                                 gaug[:, nsl], start=False, stop=False)

            ehs = []

            def up_expert(e):
                w1 = we_t[e]
                eh = peh.tile([128, 4, NH2], bf16, tag="eh", name=f"eh{e%2}")
                for p in range(2):
                    pu = psB.tile([128, 2, 512], f32, tag="b")
                    for j in range(2):
                        mo = p * 2 + j
                        for k in range(4):
                            nc.tensor.matmul(pu[:, j, 0:NH2],
                                             w1[:, k, ds(mo * 128, 128)],
                                             h2[:, k, nsl],
                                             start=(k == 0), stop=(k == 3))
                        nc.scalar.activation(
                            eh[:, mo, :], pu[:, j, 0:NH2],
                            AF.Gelu_apprx_tanh,
                            bias=be1_r[:, layer, e * 4 + mo:e * 4 + mo + 1])
                    eng = nc.vector if p == 0 else nc.gpsimd
                    for j in range(2):
                        mo = p * 2 + j
                        eng.tensor_tensor(eh[:, mo, :], eh[:, mo, :],
                                          G_sb[:, e, nsl], op=ALU.mult)
                ehs.append(eh)

            def down_expert(e):
                w2 = wexp.tile([128, 4, DE], bf16, tag="we2", bufs=3, name=f"w2_{e}")
                dma(w2[:], d["we2"][layer, e].rearrange("(c p) o -> p c o", p=128))
                eh = ehs[e]
                last = (e == E - 1)
                for mo in range(4):
                    for k in range(4):
                        nc.tensor.matmul(pd[mo][:], w2[:, k, ds(mo * 128, 128)],
                                         eh[:, k, :], start=False,
                                         stop=(last and k == 3))

            up_expert(0)
            for e in range(1, E):
                up_expert(e)
                down_expert(e - 1)
            down_expert(E - 1)

            for mo in range(4):
                nc.vector.tensor_tensor(xT[:, mo, nsl], pd[mo][:],
                                        xT[:, mo, nsl], op=ALU.add)
            emit_xsq(xsq_next, xT, n0)

    # =======================================================
    # final LN + mean-pool + head
    # =======================================================
    fT = ph.tile([128, 4, NTOK], bf16, tag="h", bufs=2, name="fT")
    layernorm(xT, lambda k, nsl: fT[:, k, nsl], xsq_next)

    pooled = wp1p.tile([128, 24], f32, tag="pool")
    pooledb = wp1p.tile([128, 24], bf16, tag="poolb")
    pview = pooled[:].rearrange("p (m k b) -> p b m k", m=3, k=4, b=BL)
    for k in range(4):
        for b in range(BL):
            nc.vector.tensor_reduce(
                pview[:, b, :, k],
                fT[:, k, ds(b * 384, 384)].rearrange("p (m s) -> p m s", m=3),
                op=ALU.add, axis=AX.X)
    nc.vector.tensor_scalar(out=pooledb[:], in0=pooled[:], scalar1=1.0 / S,
                            scalar2=None, op0=ALU.mult)

    # fused = relu(pooled @ Wp1 + bp1), token-major [BL, 1536]
    pfs = [psA.tile([BL, DE], f32, tag="a", name=f"pfs{i}") for i in range(3)]
    for kj in range(12):
        for ns in range(3):
            nc.tensor.matmul(pfs[ns][:], pooledb[:, ds(kj * 2, BL)],
                             wp1_sb[kj // 3][:, kj % 3, ds(ns * DE, DE)],
                             start=(kj == 0), stop=False)
    for ns in range(3):
        nc.tensor.matmul(pfs[ns][:], onesrow[0:1, 0:BL],
                         bp1_row[:, ds(ns * DE, DE)], start=False, stop=True)
    fused_sb = wp1p.tile([BL, 3 * DE], bf16, tag="fus")
    for ns in range(3):
        nc.scalar.activation(fused_sb[:, ds(ns * DE, DE)], pfs[ns][:], AF.Relu)

    fusedT = wp1p.tile([128, 12, BL], bf16, tag="fusT")
    for kj in range(12):
        pft = psB.tile([128, BL], bf16, tag="b", name="pft")
        nc.tensor.transpose(pft[:], fused_sb[:, ds(kj * 128, 128)], ident[0:BL, 0:BL])
        nc.scalar.activation(fusedT[:, kj, :], pft[:], AF.Copy)

    wh_sb = w512.tile([128, 12, NCLS], bf16, tag="w")
    dma(wh_sb[:], d["wh"].rearrange("(c p) o -> p c o", p=128))
    pout = psA.tile([NCLS, BL], f32, tag="a")
    for kj in range(12):
        nc.tensor.matmul(pout[:], wh_sb[:, kj, :], fusedT[:, kj, :],
                         start=(kj == 0), stop=(kj == 11))
    osb = wp1p.tile([NCLS, BL], f32, tag="osb")
    nc.scalar.activation(osb[:], pout[:], AF.Identity, bias=bh_sb[:, 0:1])
    dma(out_d[:], osb[:])

    for pool in [psB, psA, psml, peh, pln, psm, pes, pva, pqk, pxb, ph, px,
                 wexp, w512, wbig, wp1p, cst]:
        pool.release()


def _host_prep(inputs):
    p = {k: np.asarray(v) for k, v in inputs.items()}

    shared = {}
    shared["wa"] = _f32(p["Wa"])
    shared["wt"] = _f32(p["Wt"])
    shared["wv"] = _f32(p["Wv"])
    shared["bin_r"] = _f32(np.stack([p["ba"].reshape(4, 128),
                                     p["bt"].reshape(4, 128),
                                     p["bv"].reshape(4, 128)]))
    shared["wr1"] = _f32(p["Wr1"])
    shared["br1_r"] = _f32(np.asarray(p["br1"]).reshape(3, 4, 128))
    shared["wr2"] = _f32(p["Wr2"])
    shared["br2_b"] = _f32(np.broadcast_to(np.asarray(p["br2"])[:, None, :],
                                           (3, 128, E)))
    shared["wqkv"] = _bf16(p["Wqkv"])
    shared["bqkv_qk"] = _f32(np.asarray(p["bqkv"])[:, :1024].reshape(DEPTH, 8, 128))
    shared["bqkv_v"] = _bf16(np.asarray(p["bqkv"])[:, 1024:].reshape(DEPTH, 1, DE))
    shared["wo"] = _bf16(p["Wo"])
    shared["bo_r"] = _f32(np.asarray(p["bo"]).reshape(DEPTH, 4, 128))
    shared["ws1"] = _bf16(p["Ws1"])
    shared["bs1_r"] = _f32(np.asarray(p["bs1"]).reshape(DEPTH, 4, 128))
    shared["ws2"] = _bf16(p["Ws2"])
    shared["we1"] = _bf16(p["We1"])
    shared["be1_r"] = _f32(np.asarray(p["be1"]).reshape(DEPTH, 24, 128))
    shared["we2"] = _bf16(p["We2"])
    bmat = np.concatenate([np.asarray(p["be2"]),
                           np.asarray(p["bs2"])[:, None, :]], axis=1)
    shared["bmat"] = _bf16(bmat)
    shared["wp1"] = _bf16(p["Wp1"])
    shared["bp1_row"] = _bf16(np.asarray(p["bp1"]).reshape(1, 3 * DE))
    shared["wh"] = _bf16(p["Wh"])
    shared["bh_r"] = _f32(np.asarray(p["bh"]).reshape(NCLS, 1))
    shared["ones_s"] = _f32(np.full((128, 128), 1.0 / DE, np.float32))
    shared["ones_b"] = _bf16(np.full((128, 128), 1.0 / DE, np.float32))
    shared["onesrow"] = _bf16(np.ones((1, 128), np.float32))
    shared["ident"] = _bf16(np.eye(128, dtype=np.float32))
    sel = np.zeros((E, E * 128), np.float32)
    for e in range(E):
        sel[e, e * 128:(e + 1) * 128] = 1.0
    shared["sel"] = _bf16(sel)

    in_maps = []
    for c in range(NCORES):
        sl = slice(BL * c, BL * (c + 1))
        m = dict(shared)
        m["aT"] = _f32(np.asarray(p["audio"])[sl].transpose(2, 0, 1)
                       .reshape(ADIM, BL * S))
        m["tT"] = _f32(np.asarray(p["text"])[sl].transpose(2, 0, 1)
                       .reshape(TDIM, BL * S))
        m["vT"] = _f32(np.asarray(p["visual"])[sl].transpose(2, 0, 1)
                       .reshape(VDIM, BL * S))
        in_maps.append(m)
    return in_maps


def kernel(**inputs):
    from concourse.bass_utils import run_bass_kernel_spmd

    if "nc" not in _CACHE:
        _CACHE["nc"] = _build()
    nc, _ = _CACHE["nc"]

    in_maps = _host_prep(inputs)
    res = run_bass_kernel_spmd(nc, in_maps, core_ids=list(range(NCORES)))
    out = np.empty((B, NCLS), np.float32)
    for c in range(NCORES):
        out[BL * c: BL * (c + 1)] = res.results[c]["out"].T
    return out


# revision 43
# speedup vs baseline: 1.2002x; 1.0027x over previous
"""MoMKE (multimodal MoE transformer) forward on 8 trn2 NeuronCores.

Sharding: pure data-parallel over batch (B=16 -> 2 batch elements per core).
Each core runs the full model on its 2 sequences; no collectives.

On-device layout: activations are feature-major ("transposed", [feature,
token]) so weights in natural [in, out] layout serve directly as matmul
lhsT.  Tokens per core: 768 columns, ordered col = b*384 + m*128 + s for
local batch b in {0,1}, modality m in {a,t,v}, position s.

Scheduling: PE warm-up matmul stream during the initial input DMA;
expert loop software-pipelined (down-proj of expert e issued after
up-proj of e+1) with expert up-weights resident in SBUF per layer;
attention runs a cross-pair pipeline (QK matmuls of pair p+1 issued
before AV of pair p; adjacent heads at partition bases 0/64 execute
concurrently in separate PE row groups); router/gating phase is emitted
inside layer-0's attention window so its matmuls fill the PE while the
scalar engine does softmax; PSUM is organized as 4x single-bank + 2x
double-bank tiles; softmax reciprocal is Exp(-Ln(x)) on the scalar
engine (Ln/Exp share one ACT table); E[x^2] for each LayerNorm is
hoisted into the preceding phase; the residual stream is natively f32r
so stat matmuls need no dtype-copy; out-projection is n0-major so LN2
statistics start on the first token half early; elementwise work is
split across vector/gpsimd/scalar to keep the PE streaming.
"""

import numpy as np

B, S = 16, 128
ADIM, TDIM, VDIM = 512, 768, 1024
DE = 512
DEPTH = 4
NH = 8
HD = 64
E = 6
NCLS = 6
EPS = 1e-5
NCORES = 8
BL = B // NCORES          # local batch: 2
NTOK = BL * 3 * S         # 768 tokens/core
NH2 = 384                 # half of token columns (matmul N tile)

_CACHE = {}


def _f32(a):
    return np.ascontiguousarray(np.asarray(a, dtype=np.float32))


def _bf16(a):
    import ml_dtypes
    return np.ascontiguousarray(
        np.asarray(a, dtype=np.float32).astype(ml_dtypes.bfloat16))


def _fp8i(a):
    """[..., K=512, M] -> DoubleRow-interleaved fp8 [..., 128, 2, 2, M]."""
    import ml_dtypes
    a = np.asarray(a, dtype=np.float32)
    lead, (K, M) = a.shape[:-2], a.shape[-2:]
    assert K == 512
    b = a.reshape(*lead, 2, 2, 128, M)
    b = np.moveaxis(b, -2, -4)
    return np.ascontiguousarray(b.astype(ml_dtypes.float8_e4m3fn))


def _split_waits(nc, mybir):
    """This walrus build accepts at most one sync wait / one sync update per
    ISA instruction; Tile's sem assignment can attach several.  Spread the
    extras onto same-engine no-ops."""
    n = 0
    for bb in nc.main_func.blocks:
        insts = list(bb.instructions)
        out = []
        changed = False
        for ins in insts:
            si = ins.sync_info
            if si is None:
                out.append(ins)
                continue
            waits = list(si.on_wait or [])
            updates = list(si.on_update or [])
            post = []
            if len(waits) > 1 or len(updates) > 1:
                for w in waits[:-1]:
                    n += 1
                    nop = mybir.InstNoOp(name=f"xw-{n}", ins=[], outs=[])
                    nop.engine = ins.engine
                    nop.sync_info = mybir.SyncInfo(on_wait=[w], on_update=[])
                    out.append(nop)
                for u in updates[1:]:
                    n += 1
                    nop = mybir.InstNoOp(name=f"xu-{n}", ins=[], outs=[])
                    nop.engine = ins.engine
                    nop.sync_info = mybir.SyncInfo(on_wait=[], on_update=[u])
                    post.append(nop)
                ins.sync_info = mybir.SyncInfo(on_wait=waits[-1:],
                                               on_update=updates[:1])
                changed = True
            out.append(ins)
            out.extend(post)
        if changed:
            bb.instructions[:] = out
    return n


def _build():
    import concourse.bass as bass
    import concourse.mybir as mybir
    import concourse.tile as tile

    f32 = mybir.dt.float32
    fp8 = mybir.dt.float8e4
    f32r = mybir.dt.float32r
    bf16 = mybir.dt.bfloat16
    AF = mybir.ActivationFunctionType
    ALU = mybir.AluOpType
    AX = mybir.AxisListType

    nc = bass.Bass()

    d = {}

    def din(name, shape, dt):
        d[name] = nc.dram_tensor(name, shape, dt, kind="ExternalInput")

    din("ones_s", [128, 128], f32r)
    din("aT", [ADIM, BL * S], f32r)
    din("tT", [TDIM, BL * S], f32r)
    din("vT", [VDIM, BL * S], f32r)
    din("wa", [ADIM, DE], f32r)
    din("wt", [TDIM, DE], f32r)
    din("wv", [VDIM, DE], f32r)
    din("bin_r", [3, 4, 128], f32)
    din("wr1", [3, DE, DE], f32r)
    din("br1_r", [3, 4, 128], f32)
    din("wr2", [3, DE, E], f32r)
    din("br2_b", [3, 128, E], f32)
    din("wqkv", [DEPTH, DE, 3 * DE], bf16)
    din("bqkv_qk", [DEPTH, 8, 128], f32)
    din("bqkv_v", [DEPTH, 1, DE], bf16)
    din("wo", [DEPTH, DE, DE], bf16)
    din("bo_r", [DEPTH, 4, 128], f32)
    din("ws1", [DEPTH, DE, DE], bf16)
    din("bs1_r", [DEPTH, 4, 128], f32)
    din("ws2", [DEPTH, DE, DE], bf16)
    din("we1", [DEPTH, E, DE, DE], bf16)
    din("be1_r", [DEPTH, 24, 128], f32)
    din("we2", [DEPTH, E, DE, DE], bf16)
    din("bmat", [DEPTH, 7, DE], bf16)
    din("wp1", [3 * DE, 3 * DE], bf16)
    din("bp1_row", [1, 3 * DE], bf16)
    din("wh", [3 * DE, NCLS], bf16)
    din("bh_r", [NCLS, 1], f32)
    din("onesrow", [1, 128], bf16)
    din("ident", [128, 128], bf16)
    din("ones_b", [128, 128], bf16)
    din("sel", [E, E * 128], bf16)
    out_d = nc.dram_tensor("out", [NCLS, BL], f32, kind="ExternalOutput")

    with tile.TileContext(nc) as tc:
        _emit(nc, tc, d, out_d, f32, f32r, bf16, fp8, AF, ALU, AX)

    nfix = _split_waits(nc, mybir)
    return nc, nfix


def _emit(nc, tc, d, out_d, f32, f32r, bf16, fp8, AF, ALU, AX):
    from concourse.bass import ds

    import concourse.mybir as mybir
    DR = mybir.MatmulPerfMode.DoubleRow
    dma = nc.sync.dma_start

    def r32(ap):
        return ap.bitcast(f32r)

    cst = tc.alloc_tile_pool(name="cst", bufs=1)
    ph0 = tc.alloc_tile_pool(name="ph0", bufs=1, side="right")  # phase 0/1, released early
    wbig = tc.alloc_tile_pool(name="wbig", bufs=2)
    w512 = tc.alloc_tile_pool(name="w512", bufs=4)
    wexp = tc.alloc_tile_pool(name="wexp", bufs=2)
    px = tc.alloc_tile_pool(name="px", bufs=1)
    ph = tc.alloc_tile_pool(name="ph", bufs=2)
    pxb = tc.alloc_tile_pool(name="pxb", bufs=1)
    pqk = tc.alloc_tile_pool(name="pqk", bufs=1)
    pva = tc.alloc_tile_pool(name="pva", bufs=1)
    pes = tc.alloc_tile_pool(name="pes", bufs=2)
    psm = tc.alloc_tile_pool(name="psm", bufs=2)
    pln = tc.alloc_tile_pool(name="pln", bufs=4)
    peh = tc.alloc_tile_pool(name="peh", bufs=2)
    psml = tc.alloc_tile_pool(name="psml", bufs=4)
    psA = tc.alloc_tile_pool(name="psA", bufs=4, space="PSUM")
    psB = tc.alloc_tile_pool(name="psB", bufs=2, space="PSUM")

    # ---------------- PE warm-up: stream matmuls while input DMAs land ----
    ident = cst.tile([128, 128], bf16)
    dma(ident[:], d["ident"][:])
    bin_r = cst.tile([128, 3, 4], f32)
    dma(bin_r[:], d["bin_r"].rearrange("m c p -> p m c"))
    ones_b = cst.tile([128, 128], bf16)
    dma(ones_b[:], d["ones_b"][:])
    pwu = psA.tile([128, 128], f32, tag="a", name="warmup")
    for _ in range(64):
        nc.tensor.matmul(pwu[:], ident[:], ident[:], start=True, stop=True)

    eps_sb = cst.tile([128, 1], f32)
    nc.gpsimd.memset(eps_sb[:], EPS)
    G_sb = cst.tile([128, E, NTOK], bf16)        # per-expert gate rows, bcast
    gaug = cst.tile([7, NTOK], bf16)             # gates^T rows + ones row
    nc.gpsimd.memset(gaug[:], 1.0)  # rows 0..5 overwritten by gate evictions

    xT = px.tile([128, 4, NTOK], f32r)           # residual stream

    # =======================================================
    # Phase 0: input projections
    # =======================================================
    def in_proj(dname, wname, idim, m):
        kc = idim // 128
        pa = [psA.tile([128, BL * S], f32, tag="a", name=f"pa{i}") for i in range(4)]
        for k in range(kc):
            it = ph0.tile([128, BL * S], f32r, tag="it", bufs=3, name=f"it{m}{k}")
            dma(it[:], d[dname].rearrange("(c p) t -> p c t", p=128)[:, k, :])
            wi = ph0.tile([128, DE], f32r, tag="wi", bufs=3, name=f"wi{m}{k}")
            dma(wi[:], d[wname].rearrange("(c p) o -> p c o", p=128)[:, k, :])
            for mo in range(4):
                nc.tensor.matmul(pa[mo][:], wi[:, ds(mo * 128, 128)], it[:],
                                 start=(k == 0), stop=(k == kc - 1))
        for mo in range(4):
            dst = xT[:, mo, :].rearrange("p (b r) -> p b r", b=BL)[:, :, ds(m * 128, 128)]
            nc.vector.tensor_scalar(out=dst,
                                    in0=pa[mo][:].rearrange("p (b s) -> p b s", b=BL),
                                    scalar1=bin_r[:, m, mo:mo + 1], scalar2=None,
                                    op0=ALU.add)

    in_proj("aT", "wa", ADIM, 0)
    in_proj("tT", "wt", TDIM, 1)
    wq0_t = []
    for k in range(4):
        wqk = wbig.tile([128, 3 * DE], bf16, tag="wq", bufs=4, name=f"wq{k}")
        dma(wqk[:], d["wqkv"][0, ds(k * 128, 128), :])
        wq0_t.append(wqk)
    in_proj("vT", "wv", VDIM, 2)

    # ---------------- remaining constants (after the input DMAs) ----------
    ones_s = cst.tile([128, 128], f32r)          # 1/512 everywhere
    dma(ones_s[:], d["ones_s"][:])
    onesrow = cst.tile([1, 128], bf16)
    dma(onesrow[:], d["onesrow"][:])
    sel_sb = cst.tile([E, E * 128], bf16)
    dma(sel_sb[:], d["sel"][:])
    br1_r = cst.tile([128, 3, 4], f32)
    dma(br1_r[:], d["br1_r"].rearrange("m c p -> p m c"))
    wr2_sb = cst.tile([128, 3, 4, E], f32r)
    dma(wr2_sb[:], d["wr2"].rearrange("m (c p) e -> p m c e", p=128))
    br2_b = cst.tile([128, 3, E], f32)
    dma(br2_b[:], d["br2_b"].rearrange("m p e -> p m e"))
    bqkv_qk = cst.tile([128, DEPTH, 8], f32)
    dma(bqkv_qk[:], d["bqkv_qk"].rearrange("l c p -> p l c"))
    bo_r = cst.tile([128, DEPTH, 4], f32)
    dma(bo_r[:], d["bo_r"].rearrange("l c p -> p l c"))
    bs1_r = cst.tile([128, DEPTH, 4], f32)
    dma(bs1_r[:], d["bs1_r"].rearrange("l c p -> p l c"))
    be1_r = cst.tile([128, DEPTH, 24], f32)
    dma(be1_r[:], d["be1_r"].rearrange("l c p -> p l c"))
    bh_sb = cst.tile([NCLS, 1], f32)
    dma(bh_sb[:], d["bh_r"][:])

    va = pva.tile([128, 6, 8 * 128], bf16)
    nc.gpsimd.memset(
        va[:].rearrange("p t (h w) -> p t h w", h=NH)[:, :, :, 64:128], 1.0)

    # =======================================================
    # layers
    # =======================================================
    def make_xsq():
        return pxb.tile([128, 4, NTOK], bf16, tag="xsq", name="xsq")

    def emit_xsq(xsq, src, n0):
        nsl = ds(n0 * NH2, NH2)
        nc.vector.tensor_tensor(xsq[:, :, nsl], src[:, :, nsl],
                                src[:, :, nsl], op=ALU.mult)

    def layernorm_n0(src_f32, dst_slice, xsq, n0, all_gpsimd=False):
        if True:
            nsl = ds(n0 * NH2, NH2)
            pst = psB.tile([128, 2, 512], f32, tag="b")
            for k in range(4):
                nc.tensor.matmul(pst[:, 0, 0:NH2], ones_s[:], src_f32[:, k, nsl],
                                 start=(k == 0), stop=(k == 3))
            for k in range(4):
                nc.tensor.matmul(pst[:, 1, 0:NH2], ones_b[:], xsq[:, k, nsl],
                                 start=(k == 0), stop=(k == 3))
            mb = pln.tile([128, NH2], f32, tag="ln", bufs=3)
            nc.scalar.activation(mb[:], pst[:, 0, 0:NH2], AF.Copy)
            qq = pln.tile([128, NH2], f32, tag="ln", bufs=3)
            nc.gpsimd.tensor_tensor(qq[:], mb[:], mb[:], op=ALU.mult)
            vb = pln.tile([128, NH2], f32, tag="ln", bufs=3)
            nc.vector.scalar_tensor_tensor(out=vb[:], in0=pst[:, 1, 0:NH2],
                                           scalar=1.0, in1=qq[:],
                                           op0=ALU.mult, op1=ALU.subtract)
            sq = pln.tile([128, NH2], f32, tag="ln", bufs=3)
            nc.scalar.activation(sq[:], vb[:], AF.Ln, bias=eps_sb[:])
            rb = pln.tile([128, NH2], f32, tag="ln", bufs=3)
            nc.scalar.activation(rb[:], sq[:], AF.Exp, scale=-0.5)
            for k in range(4):
                eng = nc.gpsimd if (all_gpsimd or k >= 2) else nc.vector
                t = pln.tile([128, NH2], f32, tag="lt", bufs=2)
                eng.tensor_tensor(t[:], src_f32[:, k, nsl], mb[:],
                                  op=ALU.subtract)
                eng.tensor_tensor(dst_slice(k, nsl), t[:], rb[:],
                                  op=ALU.mult)

    def layernorm(src_f32, dst_slice, xsq, all_gpsimd=False):
        """src feature-major [128,4,NTOK] fp32 -> dst (via dst_slice(k, nsl)).
        Mean/E[x^2] via ones-matmul (broadcast across partitions); xsq
        precomputed (hoisted into the previous phase for overlap)."""
        for n0 in range(2):
            layernorm_n0(src_f32, dst_slice, xsq, n0, all_gpsimd)

    def emit_phase1():
        # Phase 1: routers + gates (emitted inside layer-0's attention
        # window so router matmuls fill the PE while ACT does softmax)
        rh = ph0.tile([128, 4, 3, BL * S], f32r, tag="hr", bufs=1)
        for m in range(3):
            wr1_sb = wr1_t[m]
            xm = xT[:].rearrange("p c (b mm s) -> p c b mm s", b=BL, mm=3)[:, :, :, m, :]
            for mo in range(4):
                pr = psA.tile([128, BL * S], f32, tag="a")
                for k in range(4):
                    nc.tensor.matmul(pr[:].rearrange("p (b s) -> p b s", b=BL),
                                     wr1_sb[:, k, ds(mo * 128, 128)], xm[:, k, :, :],
                                     start=(k == 0), stop=(k == 3))
                nc.scalar.activation(rh[:, mo, m, :], pr[:], AF.Gelu_apprx_tanh,
                                     bias=br1_r[:, m, mo:mo + 1])

        rlog = cst.tile([128, 6, E], f32)
        for m in range(3):
            for b in range(BL):
                q = b * 3 + m
                prl = psA.tile([128, E], f32, tag="a")
                for k in range(4):
                    nc.tensor.matmul(prl[:], rh[:, k, m, ds(b * 128, 128)],
                                     wr2_sb[:, m, k, :], start=(k == 0), stop=(k == 3))
                nc.vector.tensor_tensor(rlog[:, q, :], prl[:], br2_b[:, m, :], op=ALU.add)

        for q in range(6):
            r = rlog[:, q, :]
            v1 = pln.tile([128, 1], f32, tag="sc")
            nc.vector.tensor_reduce(v1[:], r, op=ALU.max, axis=AX.X)
            m1 = pln.tile([128, E], f32, tag="m6")
            nc.vector.tensor_scalar(out=m1[:], in0=r, scalar1=v1[:], scalar2=None,
                                    op0=ALU.is_equal)
            mk = pln.tile([128, E], f32, tag="m6")
            nc.vector.scalar_tensor_tensor(out=mk[:], in0=m1[:], scalar=-1e9,
                                           in1=r, op0=ALU.mult, op1=ALU.add)
            v2 = pln.tile([128, 1], f32, tag="sc")
            nc.vector.tensor_reduce(v2[:], mk[:], op=ALU.max, axis=AX.X)
            m2 = pln.tile([128, E], f32, tag="m6")
            nc.vector.tensor_scalar(out=m2[:], in0=mk[:], scalar1=v2[:], scalar2=None,
                                    op0=ALU.is_equal)
            dd = pln.tile([128, 1], f32, tag="sc")
            nc.vector.tensor_tensor(dd[:], v1[:], v2[:], op=ALU.subtract)
            g1 = pln.tile([128, 1], f32, tag="sc")
            nc.scalar.activation(g1[:], dd[:], AF.Sigmoid)
            g2 = pln.tile([128, 1], f32, tag="sc")
            nc.vector.tensor_scalar(out=g2[:], in0=g1[:], scalar1=-1.0, scalar2=1.0,
                                    op0=ALU.mult, op1=ALU.add)
            gm2 = pln.tile([128, E], f32, tag="m6")
            nc.vector.tensor_scalar(out=gm2[:], in0=m2[:], scalar1=g2[:], scalar2=None,
                                    op0=ALU.mult)
            gq = pln.tile([128, E], bf16, tag="m6b")
            nc.vector.scalar_tensor_tensor(out=gq[:], in0=m1[:], scalar=g1[:],
                                           in1=gm2[:], op0=ALU.mult, op1=ALU.add)
            pt = psA.tile([E, 128], bf16, tag="a", name="ptg")
            nc.tensor.transpose(pt[:], gq[:], ident[:])
            nc.scalar.activation(gaug[0:6, ds(q * 128, 128)], pt[:], AF.Copy)

        for e in range(E):
            for n0 in range(2):
                pg = psA.tile([128, NH2], f32, tag="a")
                nc.tensor.matmul(pg[:], sel_sb[:, ds(e * 128, 128)],
                                 gaug[0:6, ds(n0 * NH2, NH2)], start=True, stop=True)
                nc.scalar.activation(G_sb[:, e, ds(n0 * NH2, NH2)], pg[:], AF.Copy)

        ph0.release()

        wp1p = tc.alloc_tile_pool(name="wp1p", bufs=1, side="right")
        return wp1p

    for layer in range(DEPTH):
        if layer == 0:
            wq_t = wq0_t
        else:
            wq_t = []
            for k in range(4):
                wqk = wbig.tile([128, 3 * DE], bf16, tag="wq", bufs=4,
                                name=f"wq{k}")
                dma(wqk[:], d["wqkv"][layer, ds(k * 128, 128), :])
                wq_t.append(wqk)
        bqv = psml.tile([1, DE], bf16, tag="bqv", bufs=1)
        dma(bqv[:], d["bqkv_v"][layer])
        bmat_sb = psml.tile([7, DE], bf16, tag="bm", bufs=1)
        dma(bmat_sb[:], d["bmat"][layer])
        wo_sb = w512.tile([128, 4, DE], bf16, tag="w")
        dma(wo_sb[:], d["wo"][layer].rearrange("(c p) o -> p c o", p=128))
        ws1_sb = w512.tile([128, 4, DE], bf16, tag="w")
        dma(ws1_sb[:], d["ws1"][layer].rearrange("(c p) o -> p c o", p=128))
        ws2_sb = w512.tile([128, 4, DE], bf16, tag="w")
        dma(ws2_sb[:], d["ws2"][layer].rearrange("(c p) o -> p c o", p=128))
        if layer == 0:
            wr1_t = []
            for m in range(3):
                wr1_sb = ph0.tile([128, 4, DE], f32r, tag="wr", bufs=2,
                                  name="wr1_sb")
                dma(wr1_sb[:], d["wr1"][m].rearrange("(c p) o -> p c o", p=128))
                wr1_t.append(wr1_sb)

        # expert weights: resident for the whole layer, loaded once
        we_t = []
        for e in range(E):
            w1 = wexp.tile([128, 4, DE], bf16, tag="we", bufs=7, name=f"w1_{e}")
            dma(w1[:], d["we1"][layer, e].rearrange("(c p) o -> p c o", p=128))
            we_t.append(w1)
        if layer == DEPTH - 1:
            bp1_row = wp1p.tile([1, 3 * DE], bf16, tag="bp1")
            dma(bp1_row[:], d["bp1_row"][:])
            wp1_sb = []
            for g in range(4):
                wpg = wp1p.tile([128, 3, 3 * DE], bf16, tag="wp", bufs=3,
                                name=f"wp{g}")
                dma(wpg[:], d["wp1"].rearrange("(c p) o -> p c o", p=128)
                    [:, ds(g * 3, 3), :])
                wp1_sb.append(wpg)

        # ---- LN1 ----
        if layer == 0:
            xsq_next = make_xsq()
            emit_xsq(xsq_next, xT, 0)
            emit_xsq(xsq_next, xT, 1)
        hT = ph.tile([128, 4, NTOK], bf16, tag="h", bufs=2, name="hT")
        layernorm(xT, lambda k, nsl: hT[:, k, nsl], xsq_next)

        # ---- qkv: q,k feature-major ----
        qkT = pqk.tile([128, 8, NTOK], bf16)
        for mo in range(8):
            pq2 = [psA.tile([128, NH2], f32, tag="a", name=f"pq{i}") for i in range(2)]
            for k in range(4):
                for n0 in range(2):
                    nc.tensor.matmul(pq2[n0][:], wq_t[k][:, ds(mo * 128, 128)],
                                     hT[:, k, ds(n0 * NH2, NH2)],
                                     start=(k == 0), stop=(k == 3))
            for n0 in range(2):
                nc.vector.tensor_scalar(out=qkT[:, mo, ds(n0 * NH2, NH2)],
                                        in0=pq2[n0][:],
                                        scalar1=bqkv_qk[:, layer, mo:mo + 1],
                                        scalar2=None, op0=ALU.add)
        # ---- v token-major, bias via rank-1, into V_aug (pairs of tq) ----
        for tp in range(3):
            pv = psB.tile([128, 2, 512], f32, tag="b")
            for j in range(2):
                tq = tp * 2 + j
                for k in range(4):
                    nc.tensor.matmul(pv[:, j, :], hT[:, k, ds(tq * 128, 128)],
                                     wq_t[k][:, ds(2 * DE, DE)],
                                     start=(k == 0), stop=False)
                nc.tensor.matmul(pv[:, j, :], onesrow[0:1, 0:128], bqv[:],
                                 start=False, stop=True)
            for j in range(2):
                tq = tp * 2 + j
                dst = va[:, tq, :].rearrange("p (h w) -> p h w", h=NH)[:, :, 0:64]
                nc.scalar.activation(dst,
                                     pv[:, j, :].rearrange("p (h e) -> p h e", h=NH),
                                     AF.Copy)

        # ---- attention per (b, head): cross-pair software pipeline ----
        oT = ph.tile([128, 4, NTOK], bf16, tag="h", bufs=2, name="oT")
        prs = [(b, hp) for b in range(BL) for hp in range(NH // 2)]
        est = {}

        def qk_stage(pi):
            b, hp = prs[pi]
            pks, ess = [], []
            for h in (2 * hp, 2 * hp + 1):
                r0 = 64 * (h % 2)
                ck = 4 + h // 2
                qs = qkT[ds(r0, 64), h // 2, ds(b * 384, 384)]
                pk2 = psB.tile([128, 2, 512], f32, tag="b")
                for i in range(2):
                    nc.tensor.matmul(pk2[:, i, 0:NH2],
                                     qkT[ds(r0, 64), ck,
                                         ds(b * 384 + i * 128, 128)],
                                     qs, start=True, stop=True)
                pk1 = psA.tile([128, NH2], f32, tag="a")
                nc.tensor.matmul(pk1[:],
                                 qkT[ds(r0, 64), ck, ds(b * 384 + 256, 128)],
                                 qs, start=True, stop=True)
                pks.append((pk2, pk1))
            for j in range(2):
                pk2, pk1 = pks[j]
                es = pes.tile([128, 3, NH2], bf16, name=f"es{j}")
                nc.scalar.activation(es[:, 0:2, :], pk2[:, :, 0:NH2],
                                     AF.Exp, scale=0.125)
                nc.scalar.activation(es[:, 2, :], pk1[:], AF.Exp, scale=0.125)
                ess.append(es)
            est[pi] = ess

        def av_stage(pi):
            b, hp = prs[pi]
            ess = est[pi]
            spair = psm.tile([128, NH2], f32, tag="s")
            opair = psm.tile([128, NH2], f32, tag="o", bufs=1)
            for j, h in enumerate((2 * hp, 2 * hp + 1)):
                r0 = 64 * (h % 2)
                es = ess[j]
                po = psA.tile([128, NH2], f32, tag="a", name=f"po{h%2}")
                for i in range(3):
                    nc.tensor.matmul(po[:], va[:, b * 3 + i, ds(h * 128, 128)],
                                     es[:, i, :], start=(i == 0), stop=(i == 2))
                nc.vector.tensor_copy(spair[ds(r0, 64), :], po[ds(64, 64), :])
                nc.vector.tensor_copy(opair[ds(r0, 64), :], po[ds(0, 64), :])
            sln = psm.tile([128, NH2], f32, tag="s")
            nc.scalar.activation(sln[:], spair[:], AF.Ln)
            rcp = psm.tile([128, NH2], f32, tag="s")
            nc.scalar.activation(rcp[:], sln[:], AF.Exp, scale=-1.0)
            for h in (2 * hp, 2 * hp + 1):
                r0 = 64 * (h % 2)
                nc.gpsimd.tensor_tensor(oT[ds(r0, 64), h // 2, ds(b * 384, 384)],
                                        opair[ds(r0, 64), :], rcp[ds(r0, 64), :],
                                        op=ALU.mult)

        qk_stage(0)
        for pi in range(1, len(prs)):
            qk_stage(pi)
            av_stage(pi - 1)
        av_stage(len(prs) - 1)

        if layer == 0:
            wp1p = emit_phase1()

        # ---- attention out-projection + residual (n0-major so the n0=0
        # half of xT finalizes early and LN2 stats can start) ----
        xsq2 = make_xsq()
        for n0 in range(2):
            nsl = ds(n0 * NH2, NH2)
            for mo in range(4):
                pp = psA.tile([128, NH2], f32, tag="a", name="pp")
                for k in range(4):
                    nc.tensor.matmul(pp[:], wo_sb[:, k, ds(mo * 128, 128)],
                                     oT[:, k, nsl], start=(k == 0), stop=(k == 3))
                nc.vector.scalar_tensor_tensor(out=xT[:, mo, nsl], in0=pp[:],
                                               scalar=bo_r[:, layer, mo:mo + 1],
                                               in1=xT[:, mo, nsl],
                                               op0=ALU.add, op1=ALU.add)
            emit_xsq(xsq2, xT, n0)

        # ---- LN2 ----
        h2 = ph.tile([128, 4, NTOK], bf16, tag="h", bufs=2, name="h2")
        layernorm(xT, lambda k, nsl: h2[:, k, nsl], xsq2)

        # ---- MoE: shared expert + 6 gated experts, software-pipelined ----
        xsq_next = make_xsq()
        for n0 in range(2):
            nsl = ds(n0 * NH2, NH2)
            pd = [psA.tile([128, NH2], f32, tag="a", name=f"pd{i}") for i in range(4)]
            for mo in range(4):
                nc.tensor.matmul(pd[mo][:], bmat_sb[:, ds(mo * 128, 128)],
                                 gaug[:, nsl], start=True, stop=False)
            su = peh.tile([128, 4, NH2], bf16, tag="eh")
            for p in range(2):
                pu = psB.tile([128, 2, 512], f32, tag="b")
                for j in range(2):
                    mo = p * 2 + j
                    for k in range(4):
                        nc.tensor.matmul(pu[:, j, 0:NH2],
                                         ws1_sb[:, k, ds(mo * 128, 128)],
                                         h2[:, k, nsl], start=(k == 0), stop=(k == 3))
                    nc.scalar.activation(su[:, mo, :], pu[:, j, 0:NH2],
                                         AF.Gelu_apprx_tanh,
                                         bias=bs1_r[:, layer, mo:mo + 1])
            for mo in range(4):
                for k in range(4):
                    nc.tensor.matmul(pd[mo][:], ws2_sb[:, k, ds(mo * 128, 128)],
                                     su[:, k, :], start=False, stop=False)

            ehs = []

            def up_expert(e):
                w1 = we_t[e]
                eh = peh.tile([128, 4, NH2], bf16, tag="eh", name=f"eh{e%2}")
                for p in range(2):
                    pu = psB.tile([128, 2, 512], f32, tag="b")
                    for j in range(2):
                        mo = p * 2 + j
                        for k in range(4):
                            nc.tensor.matmul(pu[:, j, 0:NH2],
                                             w1[:, k, ds(mo * 128, 128)],
                                             h2[:, k, nsl],
                                             start=(k == 0), stop=(k == 3))
                        nc.scalar.activation(
                            eh[:, mo, :], pu[:, j, 0:NH2],
                            AF.Gelu_apprx_tanh,
                            bias=be1_r[:, layer, e * 4 + mo:e * 4 + mo + 1])
                    eng = (nc.gpsimd if (p == 1 or
                           (layer == DEPTH - 1 and n0 == 1)) else nc.vector)
                    for j in range(2):
                        mo = p * 2 + j
                        eng.tensor_tensor(eh[:, mo, :], eh[:, mo, :],
                                          G_sb[:, e, nsl], op=ALU.mult)
                ehs.append(eh)

            def down_expert(e):
                w2 = wexp.tile([128, 4, DE], bf16, tag="we2", bufs=3, name=f"w2_{e}")
                dma(w2[:], d["we2"][layer, e].rearrange("(c p) o -> p c o", p=128))
                eh = ehs[e]
                last = (e == E - 1)
                for mo in range(4):
                    for k in range(4):
                        nc.tensor.matmul(pd[mo][:], w2[:, k, ds(mo * 128, 128)],
                                         eh[:, k, :], start=False,
                                         stop=(last and k == 3))

            up_expert(0)
            for e in range(1, E):
                up_expert(e)
                down_expert(e - 1)
            down_expert(E - 1)

            for mo in range(4):
                nc.vector.tensor_tensor(xT[:, mo, nsl], pd[mo][:],
                                        xT[:, mo, nsl], op=ALU.add)
            emit_xsq(xsq_next, xT, n0)

    # =======================================================
    # final LN + mean-pool + head
    # =======================================================
    fT = ph.tile([128, 4, NTOK], bf16, tag="h", bufs=2, name="fT")
    layernorm(xT, lambda k, nsl: fT[:, k, nsl], xsq_next, all_gpsimd=True)

    pooled = wp1p.tile([128, 24], f32, tag="pool")
    pooledb = wp1p.tile([128, 24], bf16, tag="poolb")
    pview = pooled[:].rearrange("p (m k b) -> p b m k", m=3, k=4, b=BL)
    for k in range(4):
        for b in range(BL):
            nc.vector.tensor_reduce(
                pview[:, b, :, k],
                fT[:, k, ds(b * 384, 384)].rearrange("p (m s) -> p m s", m=3),
                op=ALU.add, axis=AX.X)
    nc.vector.tensor_scalar(out=pooledb[:], in0=pooled[:], scalar1=1.0 / S,
                            scalar2=None, op0=ALU.mult)

    # fused = relu(pooled @ Wp1 + bp1), token-major [BL, 1536]
    pfs = [psA.tile([BL, DE], f32, tag="a", name=f"pfs{i}") for i in range(3)]
    for kj in range(12):
        for ns in range(3):
            nc.tensor.matmul(pfs[ns][:], pooledb[:, ds(kj * 2, BL)],
                             wp1_sb[kj // 3][:, kj % 3, ds(ns * DE, DE)],
                             start=(kj == 0), stop=False)
    for ns in range(3):
        nc.tensor.matmul(pfs[ns][:], onesrow[0:1, 0:BL],
                         bp1_row[:, ds(ns * DE, DE)], start=False, stop=True)
    fused_sb = wp1p.tile([BL, 3 * DE], bf16, tag="fus")
    for ns in range(3):
        nc.scalar.activation(fused_sb[:, ds(ns * DE, DE)], pfs[ns][:], AF.Relu)

    fusedT = wp1p.tile([128, 12, BL], bf16, tag="fusT")
    for kj in range(12):
        pft = psB.tile([128, BL], bf16, tag="b", name="pft")
        nc.tensor.transpose(pft[:], fused_sb[:, ds(kj * 128, 128)], ident[0:BL, 0:BL])
        nc.scalar.activation(fusedT[:, kj, :], pft[:], AF.Copy)

    wh_sb = w512.tile([128, 12, NCLS], bf16, tag="w")
    dma(wh_sb[:], d["wh"].rearrange("(c p) o -> p c o", p=128))
    pout = psA.tile([NCLS, BL], f32, tag="a")
    for kj in range(12):
        nc.tensor.matmul(pout[:], wh_sb[:, kj, :], fusedT[:, kj, :],
                         start=(kj == 0), stop=(kj == 11))
    osb = wp1p.tile([NCLS, BL], f32, tag="osb")
    nc.scalar.activation(osb[:], pout[:], AF.Identity, bias=bh_sb[:, 0:1])
    dma(out_d[:], osb[:])

    for pool in [psB, psA, psml, peh, pln, psm, pes, pva, pqk, pxb, ph, px,
                 wexp, w512, wbig, wp1p, cst]:
        pool.release()


def _host_prep(inputs):
    p = {k: np.asarray(v) for k, v in inputs.items()}

    shared = {}
    shared["wa"] = _f32(p["Wa"])
    shared["wt"] = _f32(p["Wt"])
    shared["wv"] = _f32(p["Wv"])
    shared["bin_r"] = _f32(np.stack([p["ba"].reshape(4, 128),
                                     p["bt"].reshape(4, 128),
                                     p["bv"].reshape(4, 128)]))
    shared["wr1"] = _f32(p["Wr1"])
    shared["br1_r"] = _f32(np.asarray(p["br1"]).reshape(3, 4, 128))
    shared["wr2"] = _f32(p["Wr2"])
    shared["br2_b"] = _f32(np.broadcast_to(np.asarray(p["br2"])[:, None, :],
                                           (3, 128, E)))
    shared["wqkv"] = _bf16(p["Wqkv"])
    shared["bqkv_qk"] = _f32(np.asarray(p["bqkv"])[:, :1024].reshape(DEPTH, 8, 128))
    shared["bqkv_v"] = _bf16(np.asarray(p["bqkv"])[:, 1024:].reshape(DEPTH, 1, DE))
    shared["wo"] = _bf16(p["Wo"])
    shared["bo_r"] = _f32(np.asarray(p["bo"]).reshape(DEPTH, 4, 128))
    shared["ws1"] = _bf16(p["Ws1"])
    shared["bs1_r"] = _f32(np.asarray(p["bs1"]).reshape(DEPTH, 4, 128))
    shared["ws2"] = _bf16(p["Ws2"])
    shared["we1"] = _bf16(p["We1"])
    shared["be1_r"] = _f32(np.asarray(p["be1"]).reshape(DEPTH, 24, 128))
    shared["we2"] = _bf16(p["We2"])
    bmat = np.concatenate([np.asarray(p["be2"]),
                           np.asarray(p["bs2"])[:, None, :]], axis=1)
    shared["bmat"] = _bf16(bmat)
    shared["wp1"] = _bf16(p["Wp1"])
    shared["bp1_row"] = _bf16(np.asarray(p["bp1"]).reshape(1, 3 * DE))
    shared["wh"] = _bf16(p["Wh"])
    shared["bh_r"] = _f32(np.asarray(p["bh"]).reshape(NCLS, 1))
    shared["ones_s"] = _f32(np.full((128, 128), 1.0 / DE, np.float32))
    shared["ones_b"] = _bf16(np.full((128, 128), 1.0 / DE, np.float32))
    shared["onesrow"] = _bf16(np.ones((1, 128), np.float32))
    shared["ident"] = _bf16(np.eye(128, dtype=np.float32))
    sel = np.zeros((E, E * 128), np.float32)
    for e in range(E):
        sel[e, e * 128:(e + 1) * 128] = 1.0
    shared["sel"] = _bf16(sel)

    in_maps = []
    for c in range(NCORES):
        sl = slice(BL * c, BL * (c + 1))
        m = dict(shared)
        m["aT"] = _f32(np.asarray(p["audio"])[sl].transpose(2, 0, 1)
                       .reshape(ADIM, BL * S))
        m["tT"] = _f32(np.asarray(p["text"])[sl].transpose(2, 0, 1)
                       .reshape(TDIM, BL * S))
        m["vT"] = _f32(np.asarray(p["visual"])[sl].transpose(2, 0, 1)
                       .reshape(VDIM, BL * S))
        in_maps.append(m)
    return in_maps


def kernel(**inputs):
    from concourse.bass_utils import run_bass_kernel_spmd

    if "nc" not in _CACHE:
        _CACHE["nc"] = _build()
    nc, _ = _CACHE["nc"]

    in_maps = _host_prep(inputs)
    res = run_bass_kernel_spmd(nc, in_maps, core_ids=list(range(NCORES)))
    out = np.empty((B, NCLS), np.float32)
    for c in range(NCORES):
        out[BL * c: BL * (c + 1)] = res.results[c]["out"].T
    return out


# revision 44
# speedup vs baseline: 1.2056x; 1.0045x over previous
"""MoMKE (multimodal MoE transformer) forward on 8 trn2 NeuronCores.

Sharding: pure data-parallel over batch (B=16 -> 2 batch elements per core).
Each core runs the full model on its 2 sequences; no collectives.

On-device layout: activations are feature-major ("transposed", [feature,
token]) so weights in natural [in, out] layout serve directly as matmul
lhsT.  Tokens per core: 768 columns, ordered col = b*384 + m*128 + s for
local batch b in {0,1}, modality m in {a,t,v}, position s.

Scheduling: PE warm-up matmul stream during the initial input DMA;
expert loop software-pipelined (down-proj of expert e issued after
up-proj of e+1) with expert up-weights resident in SBUF per layer;
attention runs a cross-pair pipeline (QK matmuls of pair p+1 issued
before AV of pair p; adjacent heads at partition bases 0/64 execute
concurrently in separate PE row groups); router/gating phase is emitted
inside layer-0's attention window so its matmuls fill the PE while the
scalar engine does softmax; PSUM is organized as 4x single-bank + 2x
double-bank tiles; softmax reciprocal is Exp(-Ln(x)) on the scalar
engine (Ln/Exp share one ACT table); E[x^2] for each LayerNorm is
hoisted into the preceding phase; the residual stream is natively f32r
so stat matmuls need no dtype-copy; out-projection is n0-major so LN2
statistics start on the first token half early; elementwise work is
split across vector/gpsimd/scalar to keep the PE streaming.
"""

import numpy as np

B, S = 16, 128
ADIM, TDIM, VDIM = 512, 768, 1024
DE = 512
DEPTH = 4
NH = 8
HD = 64
E = 6
NCLS = 6
EPS = 1e-5
NCORES = 8
BL = B // NCORES          # local batch: 2
NTOK = BL * 3 * S         # 768 tokens/core
NH2 = 384                 # half of token columns (matmul N tile)

_CACHE = {}


def _f32(a):
    return np.ascontiguousarray(np.asarray(a, dtype=np.float32))


def _bf16(a):
    import ml_dtypes
    return np.ascontiguousarray(
        np.asarray(a, dtype=np.float32).astype(ml_dtypes.bfloat16))


def _fp8i(a):
    """[..., K=512, M] -> DoubleRow-interleaved fp8 [..., 128, 2, 2, M]."""
    import ml_dtypes
    a = np.asarray(a, dtype=np.float32)
    lead, (K, M) = a.shape[:-2], a.shape[-2:]
    assert K == 512
    b = a.reshape(*lead, 2, 2, 128, M)
    b = np.moveaxis(b, -2, -4)
    return np.ascontiguousarray(b.astype(ml_dtypes.float8_e4m3fn))


def _split_waits(nc, mybir):
    """This walrus build accepts at most one sync wait / one sync update per
    ISA instruction; Tile's sem assignment can attach several.  Spread the
    extras onto same-engine no-ops."""
    n = 0
    for bb in nc.main_func.blocks:
        insts = list(bb.instructions)
        out = []
        changed = False
        for ins in insts:
            si = ins.sync_info
            if si is None:
                out.append(ins)
                continue
            waits = list(si.on_wait or [])
            updates = list(si.on_update or [])
            post = []
            if len(waits) > 1 or len(updates) > 1:
                for w in waits[:-1]:
                    n += 1
                    nop = mybir.InstNoOp(name=f"xw-{n}", ins=[], outs=[])
                    nop.engine = ins.engine
                    nop.sync_info = mybir.SyncInfo(on_wait=[w], on_update=[])
                    out.append(nop)
                for u in updates[1:]:
                    n += 1
                    nop = mybir.InstNoOp(name=f"xu-{n}", ins=[], outs=[])
                    nop.engine = ins.engine
                    nop.sync_info = mybir.SyncInfo(on_wait=[], on_update=[u])
                    post.append(nop)
                ins.sync_info = mybir.SyncInfo(on_wait=waits[-1:],
                                               on_update=updates[:1])
                changed = True
            out.append(ins)
            out.extend(post)
        if changed:
            bb.instructions[:] = out
    return n


def _build():
    import concourse.bass as bass
    import concourse.mybir as mybir
    import concourse.tile as tile

    f32 = mybir.dt.float32
    fp8 = mybir.dt.float8e4
    f32r = mybir.dt.float32r
    bf16 = mybir.dt.bfloat16
    AF = mybir.ActivationFunctionType
    ALU = mybir.AluOpType
    AX = mybir.AxisListType

    nc = bass.Bass()

    d = {}

    def din(name, shape, dt):
        d[name] = nc.dram_tensor(name, shape, dt, kind="ExternalInput")

    din("ones_s", [128, 128], f32r)
    din("aT", [ADIM, BL * S], f32r)
    din("tT", [TDIM, BL * S], f32r)
    din("vT", [VDIM, BL * S], f32r)
    din("wa", [ADIM, DE], f32r)
    din("wt", [TDIM, DE], f32r)
    din("wv", [VDIM, DE], f32r)
    din("bin_r", [3, 4, 128], f32)
    din("wr1", [3, DE, DE], f32r)
    din("br1_r", [3, 4, 128], f32)
    din("wr2", [3, DE, E], f32r)
    din("br2_b", [3, 128, E], f32)
    din("wqkv", [DEPTH, DE, 3 * DE], bf16)
    din("bqkv_qk", [DEPTH, 8, 128], f32)
    din("bqkv_v", [DEPTH, 1, DE], bf16)
    din("wo", [DEPTH, DE, DE], bf16)
    din("bo_r", [DEPTH, 4, 128], f32)
    din("ws1", [DEPTH, DE, DE], bf16)
    din("bs1_r", [DEPTH, 4, 128], f32)
    din("ws2", [DEPTH, DE, DE], bf16)
    din("we1", [DEPTH, E, DE, DE], bf16)
    din("be1_r", [DEPTH, 24, 128], f32)
    din("we2", [DEPTH, E, DE, DE], bf16)
    din("bmat", [DEPTH, 7, DE], bf16)
    din("wp1", [3 * DE, 3 * DE], bf16)
    din("bp1_row", [1, 3 * DE], bf16)
    din("wh", [3 * DE, NCLS], bf16)
    din("bh_r", [NCLS, 1], f32)
    din("onesrow", [1, 128], bf16)
    din("ident", [128, 128], bf16)
    din("ones_b", [128, 128], bf16)
    din("sel", [E, E * 128], bf16)
    out_d = nc.dram_tensor("out", [NCLS, BL], f32, kind="ExternalOutput")

    with tile.TileContext(nc) as tc:
        _emit(nc, tc, d, out_d, f32, f32r, bf16, fp8, AF, ALU, AX)

    nfix = _split_waits(nc, mybir)
    return nc, nfix


def _emit(nc, tc, d, out_d, f32, f32r, bf16, fp8, AF, ALU, AX):
    from concourse.bass import ds

    import concourse.mybir as mybir
    DR = mybir.MatmulPerfMode.DoubleRow
    dma = nc.sync.dma_start

    def r32(ap):
        return ap.bitcast(f32r)

    cst = tc.alloc_tile_pool(name="cst", bufs=1)
    ph0 = tc.alloc_tile_pool(name="ph0", bufs=1, side="right")  # phase 0/1, released early
    wbig = tc.alloc_tile_pool(name="wbig", bufs=2)
    w512 = tc.alloc_tile_pool(name="w512", bufs=4)
    wexp = tc.alloc_tile_pool(name="wexp", bufs=2)
    px = tc.alloc_tile_pool(name="px", bufs=1)
    ph = tc.alloc_tile_pool(name="ph", bufs=2)
    pxb = tc.alloc_tile_pool(name="pxb", bufs=1)
    pqk = tc.alloc_tile_pool(name="pqk", bufs=1)
    pva = tc.alloc_tile_pool(name="pva", bufs=1)
    pes = tc.alloc_tile_pool(name="pes", bufs=2)
    psm = tc.alloc_tile_pool(name="psm", bufs=2)
    pln = tc.alloc_tile_pool(name="pln", bufs=4)
    peh = tc.alloc_tile_pool(name="peh", bufs=2)
    psml = tc.alloc_tile_pool(name="psml", bufs=4)
    psA = tc.alloc_tile_pool(name="psA", bufs=4, space="PSUM")
    psB = tc.alloc_tile_pool(name="psB", bufs=2, space="PSUM")

    # ---------------- PE warm-up: stream matmuls while input DMAs land ----
    ident = cst.tile([128, 128], bf16)
    dma(ident[:], d["ident"][:])
    bin_r = cst.tile([128, 3, 4], f32)
    dma(bin_r[:], d["bin_r"].rearrange("m c p -> p m c"))
    ones_b = cst.tile([128, 128], bf16)
    dma(ones_b[:], d["ones_b"][:])
    pwu = psA.tile([128, 128], f32, tag="a", name="warmup")
    for _ in range(64):
        nc.tensor.matmul(pwu[:], ident[:], ident[:], start=True, stop=True)

    eps_sb = cst.tile([128, 1], f32)
    nc.gpsimd.memset(eps_sb[:], EPS)
    G_sb = cst.tile([128, E, NTOK], bf16)        # per-expert gate rows, bcast
    gaug = cst.tile([7, NTOK], bf16)             # gates^T rows + ones row
    nc.gpsimd.memset(gaug[:], 1.0)  # rows 0..5 overwritten by gate evictions

    xT = px.tile([128, 4, NTOK], f32r)           # residual stream

    # =======================================================
    # Phase 0: input projections
    # =======================================================
    def in_proj(dname, wname, idim, m):
        kc = idim // 128
        pa = [psA.tile([128, BL * S], f32, tag="a", name=f"pa{i}") for i in range(4)]
        for k in range(kc):
            it = ph0.tile([128, BL * S], f32r, tag="it", bufs=3, name=f"it{m}{k}")
            dma(it[:], d[dname].rearrange("(c p) t -> p c t", p=128)[:, k, :])
            wi = ph0.tile([128, DE], f32r, tag="wi", bufs=3, name=f"wi{m}{k}")
            dma(wi[:], d[wname].rearrange("(c p) o -> p c o", p=128)[:, k, :])
            for mo in range(4):
                nc.tensor.matmul(pa[mo][:], wi[:, ds(mo * 128, 128)], it[:],
                                 start=(k == 0), stop=(k == kc - 1))
        for mo in range(4):
            dst = xT[:, mo, :].rearrange("p (b r) -> p b r", b=BL)[:, :, ds(m * 128, 128)]
            nc.vector.tensor_scalar(out=dst,
                                    in0=pa[mo][:].rearrange("p (b s) -> p b s", b=BL),
                                    scalar1=bin_r[:, m, mo:mo + 1], scalar2=None,
                                    op0=ALU.add)

    in_proj("aT", "wa", ADIM, 0)
    in_proj("tT", "wt", TDIM, 1)
    wq0_t = []
    for k in range(4):
        wqk = wbig.tile([128, 3 * DE], bf16, tag="wq", bufs=4, name=f"wq{k}")
        dma(wqk[:], d["wqkv"][0, ds(k * 128, 128), :])
        wq0_t.append(wqk)
    in_proj("vT", "wv", VDIM, 2)

    # ---------------- remaining constants (after the input DMAs) ----------
    ones_s = cst.tile([128, 128], f32r)          # 1/512 everywhere
    dma(ones_s[:], d["ones_s"][:])
    onesrow = cst.tile([1, 128], bf16)
    dma(onesrow[:], d["onesrow"][:])
    sel_sb = cst.tile([E, E * 128], bf16)
    dma(sel_sb[:], d["sel"][:])
    br1_r = cst.tile([128, 3, 4], f32)
    dma(br1_r[:], d["br1_r"].rearrange("m c p -> p m c"))
    wr2_sb = cst.tile([128, 3, 4, E], f32r)
    dma(wr2_sb[:], d["wr2"].rearrange("m (c p) e -> p m c e", p=128))
    br2_b = cst.tile([128, 3, E], f32)
    dma(br2_b[:], d["br2_b"].rearrange("m p e -> p m e"))
    bqkv_qk = cst.tile([128, DEPTH, 8], f32)
    dma(bqkv_qk[:], d["bqkv_qk"].rearrange("l c p -> p l c"))
    bo_r = cst.tile([128, DEPTH, 4], f32)
    dma(bo_r[:], d["bo_r"].rearrange("l c p -> p l c"))
    bs1_r = cst.tile([128, DEPTH, 4], f32)
    dma(bs1_r[:], d["bs1_r"].rearrange("l c p -> p l c"))
    be1_r = cst.tile([128, DEPTH, 24], f32)
    dma(be1_r[:], d["be1_r"].rearrange("l c p -> p l c"))
    bh_sb = cst.tile([NCLS, 1], f32)
    dma(bh_sb[:], d["bh_r"][:])

    va = pva.tile([128, 6, 8 * 128], bf16)
    nc.gpsimd.memset(
        va[:].rearrange("p t (h w) -> p t h w", h=NH)[:, :, :, 64:128], 1.0)

    # =======================================================
    # layers
    # =======================================================
    def make_xsq():
        return pxb.tile([128, 4, NTOK], bf16, tag="xsq", name="xsq")

    def emit_xsq(xsq, src, n0):
        nsl = ds(n0 * NH2, NH2)
        nc.vector.tensor_tensor(xsq[:, :, nsl], src[:, :, nsl],
                                src[:, :, nsl], op=ALU.mult)

    def layernorm_n0(src_f32, dst_slice, xsq, n0, all_gpsimd=False):
        if True:
            nsl = ds(n0 * NH2, NH2)
            pst = psB.tile([128, 2, 512], f32, tag="b")
            for k in range(4):
                nc.tensor.matmul(pst[:, 0, 0:NH2], ones_s[:], src_f32[:, k, nsl],
                                 start=(k == 0), stop=(k == 3))
            for k in range(4):
                nc.tensor.matmul(pst[:, 1, 0:NH2], ones_b[:], xsq[:, k, nsl],
                                 start=(k == 0), stop=(k == 3))
            mb = pln.tile([128, NH2], f32, tag="ln", bufs=3)
            nc.scalar.activation(mb[:], pst[:, 0, 0:NH2], AF.Copy)
            qq = pln.tile([128, NH2], f32, tag="ln", bufs=3)
            nc.gpsimd.tensor_tensor(qq[:], mb[:], mb[:], op=ALU.mult)
            vb = pln.tile([128, NH2], f32, tag="ln", bufs=3)
            nc.vector.scalar_tensor_tensor(out=vb[:], in0=pst[:, 1, 0:NH2],
                                           scalar=1.0, in1=qq[:],
                                           op0=ALU.mult, op1=ALU.subtract)
            sq = pln.tile([128, NH2], f32, tag="ln", bufs=3)
            nc.scalar.activation(sq[:], vb[:], AF.Ln, bias=eps_sb[:])
            rb = pln.tile([128, NH2], f32, tag="ln", bufs=3)
            nc.scalar.activation(rb[:], sq[:], AF.Exp, scale=-0.5)
            for k in range(4):
                eng = nc.gpsimd if (all_gpsimd or k >= 2) else nc.vector
                t = pln.tile([128, NH2], f32, tag="lt", bufs=2)
                eng.tensor_tensor(t[:], src_f32[:, k, nsl], mb[:],
                                  op=ALU.subtract)
                eng.tensor_tensor(dst_slice(k, nsl), t[:], rb[:],
                                  op=ALU.mult)

    def layernorm(src_f32, dst_slice, xsq, all_gpsimd=False):
        """src feature-major [128,4,NTOK] fp32 -> dst (via dst_slice(k, nsl)).
        Mean/E[x^2] via ones-matmul (broadcast across partitions); xsq
        precomputed (hoisted into the previous phase for overlap)."""
        for n0 in range(2):
            layernorm_n0(src_f32, dst_slice, xsq, n0, all_gpsimd)

    def emit_phase1():
        # Phase 1: routers + gates (emitted inside layer-0's attention
        # window so router matmuls fill the PE while ACT does softmax)
        rh = ph0.tile([128, 4, 3, BL * S], f32r, tag="hr", bufs=1)
        for m in range(3):
            wr1_sb = wr1_t[m]
            xm = xT[:].rearrange("p c (b mm s) -> p c b mm s", b=BL, mm=3)[:, :, :, m, :]
            for mo in range(4):
                pr = psA.tile([128, BL * S], f32, tag="a")
                for k in range(4):
                    nc.tensor.matmul(pr[:].rearrange("p (b s) -> p b s", b=BL),
                                     wr1_sb[:, k, ds(mo * 128, 128)], xm[:, k, :, :],
                                     start=(k == 0), stop=(k == 3))
                nc.scalar.activation(rh[:, mo, m, :], pr[:], AF.Gelu_apprx_tanh,
                                     bias=br1_r[:, m, mo:mo + 1])

        rlog = cst.tile([128, 6, E], f32)
        for m in range(3):
            for b in range(BL):
                q = b * 3 + m
                prl = psA.tile([128, E], f32, tag="a")
                for k in range(4):
                    nc.tensor.matmul(prl[:], rh[:, k, m, ds(b * 128, 128)],
                                     wr2_sb[:, m, k, :], start=(k == 0), stop=(k == 3))
                nc.vector.tensor_tensor(rlog[:, q, :], prl[:], br2_b[:, m, :], op=ALU.add)

        for q in range(6):
            r = rlog[:, q, :]
            v1 = pln.tile([128, 1], f32, tag="sc")
            nc.vector.tensor_reduce(v1[:], r, op=ALU.max, axis=AX.X)
            m1 = pln.tile([128, E], f32, tag="m6")
            nc.vector.tensor_scalar(out=m1[:], in0=r, scalar1=v1[:], scalar2=None,
                                    op0=ALU.is_equal)
            mk = pln.tile([128, E], f32, tag="m6")
            nc.vector.scalar_tensor_tensor(out=mk[:], in0=m1[:], scalar=-1e9,
                                           in1=r, op0=ALU.mult, op1=ALU.add)
            v2 = pln.tile([128, 1], f32, tag="sc")
            nc.vector.tensor_reduce(v2[:], mk[:], op=ALU.max, axis=AX.X)
            m2 = pln.tile([128, E], f32, tag="m6")
            nc.vector.tensor_scalar(out=m2[:], in0=mk[:], scalar1=v2[:], scalar2=None,
                                    op0=ALU.is_equal)
            dd = pln.tile([128, 1], f32, tag="sc")
            nc.vector.tensor_tensor(dd[:], v1[:], v2[:], op=ALU.subtract)
            g1 = pln.tile([128, 1], f32, tag="sc")
            nc.scalar.activation(g1[:], dd[:], AF.Sigmoid)
            g2 = pln.tile([128, 1], f32, tag="sc")
            nc.vector.tensor_scalar(out=g2[:], in0=g1[:], scalar1=-1.0, scalar2=1.0,
                                    op0=ALU.mult, op1=ALU.add)
            gm2 = pln.tile([128, E], f32, tag="m6")
            nc.vector.tensor_scalar(out=gm2[:], in0=m2[:], scalar1=g2[:], scalar2=None,
                                    op0=ALU.mult)
            gq = pln.tile([128, E], bf16, tag="m6b")
            nc.vector.scalar_tensor_tensor(out=gq[:], in0=m1[:], scalar=g1[:],
                                           in1=gm2[:], op0=ALU.mult, op1=ALU.add)
            pt = psA.tile([E, 128], bf16, tag="a", name="ptg")
            nc.tensor.transpose(pt[:], gq[:], ident[:])
            nc.scalar.activation(gaug[0:6, ds(q * 128, 128)], pt[:], AF.Copy)

        for e in range(E):
            for n0 in range(2):
                pg = psA.tile([128, NH2], f32, tag="a")
                nc.tensor.matmul(pg[:], sel_sb[:, ds(e * 128, 128)],
                                 gaug[0:6, ds(n0 * NH2, NH2)], start=True, stop=True)
                nc.scalar.activation(G_sb[:, e, ds(n0 * NH2, NH2)], pg[:], AF.Copy)

        ph0.release()

        wp1p = tc.alloc_tile_pool(name="wp1p", bufs=1, side="right")
        return wp1p

    for layer in range(DEPTH):
        if layer == 0:
            wq_t = wq0_t
        else:
            wq_t = []
            for k in range(4):
                wqk = wbig.tile([128, 3 * DE], bf16, tag="wq", bufs=4,
                                name=f"wq{k}")
                dma(wqk[:], d["wqkv"][layer, ds(k * 128, 128), :])
                wq_t.append(wqk)
        bqv = psml.tile([1, DE], bf16, tag="bqv", bufs=1)
        dma(bqv[:], d["bqkv_v"][layer])
        bmat_sb = psml.tile([7, DE], bf16, tag="bm", bufs=1)
        dma(bmat_sb[:], d["bmat"][layer])
        wo_sb = w512.tile([128, 4, DE], bf16, tag="w")
        dma(wo_sb[:], d["wo"][layer].rearrange("(c p) o -> p c o", p=128))
        ws1_sb = w512.tile([128, 4, DE], bf16, tag="w")
        dma(ws1_sb[:], d["ws1"][layer].rearrange("(c p) o -> p c o", p=128))
        ws2_sb = w512.tile([128, 4, DE], bf16, tag="w")
        dma(ws2_sb[:], d["ws2"][layer].rearrange("(c p) o -> p c o", p=128))
        if layer == 0:
            wr1_t = []
            for m in range(3):
                wr1_sb = ph0.tile([128, 4, DE], f32r, tag="wr", bufs=2,
                                  name="wr1_sb")
                dma(wr1_sb[:], d["wr1"][m].rearrange("(c p) o -> p c o", p=128))
                wr1_t.append(wr1_sb)

        # expert weights: resident for the whole layer, loaded once
        we_t = []
        for e in range(E):
            w1 = wexp.tile([128, 4, DE], bf16, tag="we", bufs=7, name=f"w1_{e}")
            dma(w1[:], d["we1"][layer, e].rearrange("(c p) o -> p c o", p=128))
            we_t.append(w1)
        if layer == DEPTH - 1:
            bp1_row = wp1p.tile([1, 3 * DE], bf16, tag="bp1")
            dma(bp1_row[:], d["bp1_row"][:])
            wp1_sb = []
            for g in range(4):
                wpg = wp1p.tile([128, 3, 3 * DE], bf16, tag="wp", bufs=3,
                                name=f"wp{g}")
                dma(wpg[:], d["wp1"].rearrange("(c p) o -> p c o", p=128)
                    [:, ds(g * 3, 3), :])
                wp1_sb.append(wpg)

        # ---- LN1 ----
        if layer == 0:
            xsq_next = make_xsq()
            emit_xsq(xsq_next, xT, 0)
            emit_xsq(xsq_next, xT, 1)
        hT = ph.tile([128, 4, NTOK], bf16, tag="h", bufs=2, name="hT")
        layernorm(xT, lambda k, nsl: hT[:, k, nsl], xsq_next)

        # ---- qkv: q,k feature-major ----
        qkT = pqk.tile([128, 8, NTOK], bf16)
        for mo in range(8):
            pq2 = [psA.tile([128, NH2], f32, tag="a", name=f"pq{i}") for i in range(2)]
            for k in range(4):
                for n0 in range(2):
                    nc.tensor.matmul(pq2[n0][:], wq_t[k][:, ds(mo * 128, 128)],
                                     hT[:, k, ds(n0 * NH2, NH2)],
                                     start=(k == 0), stop=(k == 3))
            for n0 in range(2):
                nc.vector.tensor_scalar(out=qkT[:, mo, ds(n0 * NH2, NH2)],
                                        in0=pq2[n0][:],
                                        scalar1=bqkv_qk[:, layer, mo:mo + 1],
                                        scalar2=None, op0=ALU.add)
        # ---- v token-major, bias via rank-1, into V_aug (pairs of tq) ----
        for tp in range(3):
            pv = psB.tile([128, 2, 512], f32, tag="b")
            for j in range(2):
                tq = tp * 2 + j
                for k in range(4):
                    nc.tensor.matmul(pv[:, j, :], hT[:, k, ds(tq * 128, 128)],
                                     wq_t[k][:, ds(2 * DE, DE)],
                                     start=(k == 0), stop=False)
                nc.tensor.matmul(pv[:, j, :], onesrow[0:1, 0:128], bqv[:],
                                 start=False, stop=True)
            for j in range(2):
                tq = tp * 2 + j
                dst = va[:, tq, :].rearrange("p (h w) -> p h w", h=NH)[:, :, 0:64]
                nc.scalar.activation(dst,
                                     pv[:, j, :].rearrange("p (h e) -> p h e", h=NH),
                                     AF.Copy)

        # ---- attention per (b, head): cross-pair software pipeline ----
        oT = ph.tile([128, 4, NTOK], bf16, tag="h", bufs=2, name="oT")
        prs = [(b, hp) for b in range(BL) for hp in range(NH // 2)]
        est = {}

        def qk_stage(pi):
            b, hp = prs[pi]
            pks, ess = [], []
            for h in (2 * hp, 2 * hp + 1):
                r0 = 64 * (h % 2)
                ck = 4 + h // 2
                qs = qkT[ds(r0, 64), h // 2, ds(b * 384, 384)]
                pk2 = psB.tile([128, 2, 512], f32, tag="b")
                for i in range(2):
                    nc.tensor.matmul(pk2[:, i, 0:NH2],
                                     qkT[ds(r0, 64), ck,
                                         ds(b * 384 + i * 128, 128)],
                                     qs, start=True, stop=True)
                pk1 = psA.tile([128, NH2], f32, tag="a")
                nc.tensor.matmul(pk1[:],
                                 qkT[ds(r0, 64), ck, ds(b * 384 + 256, 128)],
                                 qs, start=True, stop=True)
                pks.append((pk2, pk1))
            for j in range(2):
                pk2, pk1 = pks[j]
                es = pes.tile([128, 3, NH2], bf16, name=f"es{j}")
                nc.scalar.activation(es[:, 0:2, :], pk2[:, :, 0:NH2],
                                     AF.Exp, scale=0.125)
                nc.scalar.activation(es[:, 2, :], pk1[:], AF.Exp, scale=0.125)
                ess.append(es)
            est[pi] = ess

        def av_stage(pi):
            b, hp = prs[pi]
            ess = est[pi]
            spair = psm.tile([128, NH2], f32, tag="s")
            opair = psm.tile([128, NH2], f32, tag="o", bufs=1)
            for j, h in enumerate((2 * hp, 2 * hp + 1)):
                r0 = 64 * (h % 2)
                es = ess[j]
                po = psA.tile([128, NH2], f32, tag="a", name=f"po{h%2}")
                for i in range(3):
                    nc.tensor.matmul(po[:], va[:, b * 3 + i, ds(h * 128, 128)],
                                     es[:, i, :], start=(i == 0), stop=(i == 2))
                nc.vector.tensor_copy(spair[ds(r0, 64), :], po[ds(64, 64), :])
                nc.vector.tensor_copy(opair[ds(r0, 64), :], po[ds(0, 64), :])
            sln = psm.tile([128, NH2], f32, tag="s")
            nc.scalar.activation(sln[:], spair[:], AF.Ln)
            rcp = psm.tile([128, NH2], f32, tag="s")
            nc.scalar.activation(rcp[:], sln[:], AF.Exp, scale=-1.0)
            for h in (2 * hp, 2 * hp + 1):
                r0 = 64 * (h % 2)
                nc.gpsimd.tensor_tensor(oT[ds(r0, 64), h // 2, ds(b * 384, 384)],
                                        opair[ds(r0, 64), :], rcp[ds(r0, 64), :],
                                        op=ALU.mult)

        qk_stage(0)
        for pi in range(1, len(prs)):
            qk_stage(pi)
            av_stage(pi - 1)
        av_stage(len(prs) - 1)

        if layer == 0:
            wp1p = emit_phase1()

        # ---- attention out-projection + residual (n0-major so the n0=0
        # half of xT finalizes early and LN2 stats can start) ----
        xsq2 = make_xsq()
        for n0 in range(2):
            nsl = ds(n0 * NH2, NH2)
            for mo in range(4):
                pp = psA.tile([128, NH2], f32, tag="a", name="pp")
                for k in range(4):
                    nc.tensor.matmul(pp[:], wo_sb[:, k, ds(mo * 128, 128)],
                                     oT[:, k, nsl], start=(k == 0), stop=(k == 3))
                nc.vector.scalar_tensor_tensor(out=xT[:, mo, nsl], in0=pp[:],
                                               scalar=bo_r[:, layer, mo:mo + 1],
                                               in1=xT[:, mo, nsl],
                                               op0=ALU.add, op1=ALU.add)
            emit_xsq(xsq2, xT, n0)

        # ---- LN2 ----
        h2 = ph.tile([128, 4, NTOK], bf16, tag="h", bufs=2, name="h2")
        layernorm(xT, lambda k, nsl: h2[:, k, nsl], xsq2)

        # ---- MoE: shared expert + 6 gated experts, software-pipelined ----
        xsq_next = make_xsq()
        for n0 in range(2):
            nsl = ds(n0 * NH2, NH2)
            pd = [psA.tile([128, NH2], f32, tag="a", name=f"pd{i}") for i in range(4)]
            for mo in range(4):
                nc.tensor.matmul(pd[mo][:], bmat_sb[:, ds(mo * 128, 128)],
                                 gaug[:, nsl], start=True, stop=False)
            su = peh.tile([128, 4, NH2], bf16, tag="eh")
            for p in range(2):
                pu = psB.tile([128, 2, 512], f32, tag="b")
                for j in range(2):
                    mo = p * 2 + j
                    for k in range(4):
                        nc.tensor.matmul(pu[:, j, 0:NH2],
                                         ws1_sb[:, k, ds(mo * 128, 128)],
                                         h2[:, k, nsl], start=(k == 0), stop=(k == 3))
                    nc.scalar.activation(su[:, mo, :], pu[:, j, 0:NH2],
                                         AF.Gelu_apprx_tanh,
                                         bias=bs1_r[:, layer, mo:mo + 1])
            for mo in range(4):
                for k in range(4):
                    nc.tensor.matmul(pd[mo][:], ws2_sb[:, k, ds(mo * 128, 128)],
                                     su[:, k, :], start=False, stop=False)

            ehs = []

            def up_expert(e):
                w1 = we_t[e]
                eh = peh.tile([128, 4, NH2], bf16, tag="eh", name=f"eh{e%2}")
                for p in range(2):
                    pu = psB.tile([128, 2, 512], f32, tag="b")
                    for j in range(2):
                        mo = p * 2 + j
                        for k in range(4):
                            nc.tensor.matmul(pu[:, j, 0:NH2],
                                             w1[:, k, ds(mo * 128, 128)],
                                             h2[:, k, nsl],
                                             start=(k == 0), stop=(k == 3))
                        nc.scalar.activation(
                            eh[:, mo, :], pu[:, j, 0:NH2],
                            AF.Gelu_apprx_tanh,
                            bias=be1_r[:, layer, e * 4 + mo:e * 4 + mo + 1])
                    eng = (nc.vector if (p == 0 or
                           (layer == DEPTH - 1 and n0 == 1)) else nc.gpsimd)
                    for j in range(2):
                        mo = p * 2 + j
                        eng.tensor_tensor(eh[:, mo, :], eh[:, mo, :],
                                          G_sb[:, e, nsl], op=ALU.mult)
                ehs.append(eh)

            def down_expert(e):
                w2 = wexp.tile([128, 4, DE], bf16, tag="we2", bufs=3, name=f"w2_{e}")
                dma(w2[:], d["we2"][layer, e].rearrange("(c p) o -> p c o", p=128))
                eh = ehs[e]
                last = (e == E - 1)
                for mo in range(4):
                    for k in range(4):
                        nc.tensor.matmul(pd[mo][:], w2[:, k, ds(mo * 128, 128)],
                                         eh[:, k, :], start=False,
                                         stop=(last and k == 3))

            up_expert(0)
            for e in range(1, E):
                up_expert(e)
                down_expert(e - 1)
            down_expert(E - 1)

            for mo in range(4):
                nc.vector.tensor_tensor(xT[:, mo, nsl], pd[mo][:],
                                        xT[:, mo, nsl], op=ALU.add)
            emit_xsq(xsq_next, xT, n0)

    # =======================================================
    # final LN + mean-pool + head
    # =======================================================
    fT = ph.tile([128, 4, NTOK], bf16, tag="h", bufs=2, name="fT")
    layernorm(xT, lambda k, nsl: fT[:, k, nsl], xsq_next, all_gpsimd=True)

    pooled = wp1p.tile([128, 24], f32, tag="pool")
    pooledb = wp1p.tile([128, 24], bf16, tag="poolb")
    pview = pooled[:].rearrange("p (m k b) -> p b m k", m=3, k=4, b=BL)
    for k in range(4):
        for b in range(BL):
            nc.vector.tensor_reduce(
                pview[:, b, :, k],
                fT[:, k, ds(b * 384, 384)].rearrange("p (m s) -> p m s", m=3),
                op=ALU.add, axis=AX.X)
    nc.vector.tensor_scalar(out=pooledb[:], in0=pooled[:], scalar1=1.0 / S,
                            scalar2=None, op0=ALU.mult)

    # fused = relu(pooled @ Wp1 + bp1), token-major [BL, 1536]
    pfs = [psA.tile([BL, DE], f32, tag="a", name=f"pfs{i}") for i in range(3)]
    for kj in range(12):
        for ns in range(3):
            nc.tensor.matmul(pfs[ns][:], pooledb[:, ds(kj * 2, BL)],
                             wp1_sb[kj // 3][:, kj % 3, ds(ns * DE, DE)],
                             start=(kj == 0), stop=False)
    for ns in range(3):
        nc.tensor.matmul(pfs[ns][:], onesrow[0:1, 0:BL],
                         bp1_row[:, ds(ns * DE, DE)], start=False, stop=True)
    fused_sb = wp1p.tile([BL, 3 * DE], bf16, tag="fus")
    for ns in range(3):
        nc.scalar.activation(fused_sb[:, ds(ns * DE, DE)], pfs[ns][:], AF.Relu)

    fusedT = wp1p.tile([128, 12, BL], bf16, tag="fusT")
    for kj in range(12):
        pft = psB.tile([128, BL], bf16, tag="b", name="pft")
        nc.tensor.transpose(pft[:], fused_sb[:, ds(kj * 128, 128)], ident[0:BL, 0:BL])
        nc.scalar.activation(fusedT[:, kj, :], pft[:], AF.Copy)

    wh_sb = w512.tile([128, 12, NCLS], bf16, tag="w")
    dma(wh_sb[:], d["wh"].rearrange("(c p) o -> p c o", p=128))
    pout = psA.tile([NCLS, BL], f32, tag="a")
    for kj in range(12):
        nc.tensor.matmul(pout[:], wh_sb[:, kj, :], fusedT[:, kj, :],
                         start=(kj == 0), stop=(kj == 11))
    osb = wp1p.tile([NCLS, BL], f32, tag="osb")
    nc.scalar.activation(osb[:], pout[:], AF.Identity, bias=bh_sb[:, 0:1])
    dma(out_d[:], osb[:])

    for pool in [psB, psA, psml, peh, pln, psm, pes, pva, pqk, pxb, ph, px,
                 wexp, w512, wbig, wp1p, cst]:
        pool.release()


def _host_prep(inputs):
    p = {k: np.asarray(v) for k, v in inputs.items()}

    shared = {}
    shared["wa"] = _f32(p["Wa"])
    shared["wt"] = _f32(p["Wt"])
    shared["wv"] = _f32(p["Wv"])
    shared["bin_r"] = _f32(np.stack([p["ba"].reshape(4, 128),
                                     p["bt"].reshape(4, 128),
                                     p["bv"].reshape(4, 128)]))
    shared["wr1"] = _f32(p["Wr1"])
    shared["br1_r"] = _f32(np.asarray(p["br1"]).reshape(3, 4, 128))
    shared["wr2"] = _f32(p["Wr2"])
    shared["br2_b"] = _f32(np.broadcast_to(np.asarray(p["br2"])[:, None, :],
                                           (3, 128, E)))
    shared["wqkv"] = _bf16(p["Wqkv"])
    shared["bqkv_qk"] = _f32(np.asarray(p["bqkv"])[:, :1024].reshape(DEPTH, 8, 128))
    shared["bqkv_v"] = _bf16(np.asarray(p["bqkv"])[:, 1024:].reshape(DEPTH, 1, DE))
    shared["wo"] = _bf16(p["Wo"])
    shared["bo_r"] = _f32(np.asarray(p["bo"]).reshape(DEPTH, 4, 128))
    shared["ws1"] = _bf16(p["Ws1"])
    shared["bs1_r"] = _f32(np.asarray(p["bs1"]).reshape(DEPTH, 4, 128))
    shared["ws2"] = _bf16(p["Ws2"])
    shared["we1"] = _bf16(p["We1"])
    shared["be1_r"] = _f32(np.asarray(p["be1"]).reshape(DEPTH, 24, 128))
    shared["we2"] = _bf16(p["We2"])
    bmat = np.concatenate([np.asarray(p["be2"]),
                           np.asarray(p["bs2"])[:, None, :]], axis=1)
    shared["bmat"] = _bf16(bmat)
    shared["wp1"] = _bf16(p["Wp1"])
    shared["bp1_row"] = _bf16(np.asarray(p["bp1"]).reshape(1, 3 * DE))
    shared["wh"] = _bf16(p["Wh"])
    shared["bh_r"] = _f32(np.asarray(p["bh"]).reshape(NCLS, 1))
    shared["ones_s"] = _f32(np.full((128, 128), 1.0 / DE, np.float32))
    shared["ones_b"] = _bf16(np.full((128, 128), 1.0 / DE, np.float32))
    shared["onesrow"] = _bf16(np.ones((1, 128), np.float32))
    shared["ident"] = _bf16(np.eye(128, dtype=np.float32))
    sel = np.zeros((E, E * 128), np.float32)
    for e in range(E):
        sel[e, e * 128:(e + 1) * 128] = 1.0
    shared["sel"] = _bf16(sel)

    in_maps = []
    for c in range(NCORES):
        sl = slice(BL * c, BL * (c + 1))
        m = dict(shared)
        m["aT"] = _f32(np.asarray(p["audio"])[sl].transpose(2, 0, 1)
                       .reshape(ADIM, BL * S))
        m["tT"] = _f32(np.asarray(p["text"])[sl].transpose(2, 0, 1)
                       .reshape(TDIM, BL * S))
        m["vT"] = _f32(np.asarray(p["visual"])[sl].transpose(2, 0, 1)
                       .reshape(VDIM, BL * S))
        in_maps.append(m)
    return in_maps


def kernel(**inputs):
    from concourse.bass_utils import run_bass_kernel_spmd

    if "nc" not in _CACHE:
        _CACHE["nc"] = _build()
    nc, _ = _CACHE["nc"]

    in_maps = _host_prep(inputs)
    res = run_bass_kernel_spmd(nc, in_maps, core_ids=list(range(NCORES)))
    out = np.empty((B, NCLS), np.float32)
    for c in range(NCORES):
        out[BL * c: BL * (c + 1)] = res.results[c]["out"].T
    return out


# revision 45
# speedup vs baseline: 1.2366x; 1.0257x over previous
"""MoMKE (multimodal MoE transformer) forward on 8 trn2 NeuronCores.

Sharding: pure data-parallel over batch (B=16 -> 2 batch elements per core).
Each core runs the full model on its 2 sequences; no collectives.

On-device layout: activations are feature-major ("transposed", [feature,
token]) so weights in natural [in, out] layout serve directly as matmul
lhsT.  Tokens per core: 768 columns, ordered col = b*384 + m*128 + s for
local batch b in {0,1}, modality m in {a,t,v}, position s.

Scheduling: PE warm-up matmul stream during the initial input DMA;
expert loop software-pipelined (down-proj of expert e issued after
up-proj of e+1) with expert up-weights resident in SBUF per layer;
attention runs a cross-pair pipeline (QK matmuls of pair p+1 issued
before AV of pair p; adjacent heads at partition bases 0/64 execute
concurrently in separate PE row groups); router/gating phase is emitted
inside layer-0's attention window so its matmuls fill the PE while the
scalar engine does softmax; PSUM is organized as 4x single-bank + 2x
double-bank tiles; softmax reciprocal is Exp(-Ln(x)) on the scalar
engine (Ln/Exp share one ACT table); E[x^2] for each LayerNorm is
hoisted into the preceding phase; the residual stream is natively f32r
so stat matmuls need no dtype-copy; out-projection is n0-major so LN2
statistics start on the first token half early; elementwise work is
split across vector/gpsimd/scalar to keep the PE streaming.
"""

import numpy as np

B, S = 16, 128
ADIM, TDIM, VDIM = 512, 768, 1024
DE = 512
DEPTH = 4
NH = 8
HD = 64
E = 6
NCLS = 6
EPS = 1e-5
NCORES = 8
BL = B // NCORES          # local batch: 2
NTOK = BL * 3 * S         # 768 tokens/core
NH2 = 384                 # half of token columns (matmul N tile)

_CACHE = {}


def _f32(a):
    return np.ascontiguousarray(np.asarray(a, dtype=np.float32))


def _bf16(a):
    import ml_dtypes
    return np.ascontiguousarray(
        np.asarray(a, dtype=np.float32).astype(ml_dtypes.bfloat16))


def _fp8i(a):
    """[..., K=512, M] -> DoubleRow-interleaved fp8 [..., 128, 2, 2, M]."""
    import ml_dtypes
    a = np.asarray(a, dtype=np.float32)
    lead, (K, M) = a.shape[:-2], a.shape[-2:]
    assert K == 512
    b = a.reshape(*lead, 2, 2, 128, M)
    b = np.moveaxis(b, -2, -4)
    return np.ascontiguousarray(b.astype(ml_dtypes.float8_e4m3fn))


def _split_waits(nc, mybir):
    """This walrus build accepts at most one sync wait / one sync update per
    ISA instruction; Tile's sem assignment can attach several.  Spread the
    extras onto same-engine no-ops."""
    n = 0
    for bb in nc.main_func.blocks:
        insts = list(bb.instructions)
        out = []
        changed = False
        for ins in insts:
            si = ins.sync_info
            if si is None:
                out.append(ins)
                continue
            waits = list(si.on_wait or [])
            updates = list(si.on_update or [])
            post = []
            if len(waits) > 1 or len(updates) > 1:
                for w in waits[:-1]:
                    n += 1
                    nop = mybir.InstNoOp(name=f"xw-{n}", ins=[], outs=[])
                    nop.engine = ins.engine
                    nop.sync_info = mybir.SyncInfo(on_wait=[w], on_update=[])
                    out.append(nop)
                for u in updates[1:]:
                    n += 1
                    nop = mybir.InstNoOp(name=f"xu-{n}", ins=[], outs=[])
                    nop.engine = ins.engine
                    nop.sync_info = mybir.SyncInfo(on_wait=[], on_update=[u])
                    post.append(nop)
                ins.sync_info = mybir.SyncInfo(on_wait=waits[-1:],
                                               on_update=updates[:1])
                changed = True
            out.append(ins)
            out.extend(post)
        if changed:
            bb.instructions[:] = out
    return n


def _build():
    import concourse.bass as bass
    import concourse.mybir as mybir
    import concourse.tile as tile

    f32 = mybir.dt.float32
    fp8 = mybir.dt.float8e4
    f32r = mybir.dt.float32r
    bf16 = mybir.dt.bfloat16
    AF = mybir.ActivationFunctionType
    ALU = mybir.AluOpType
    AX = mybir.AxisListType

    nc = bass.Bass()

    d = {}

    def din(name, shape, dt):
        d[name] = nc.dram_tensor(name, shape, dt, kind="ExternalInput")

    din("ones_s", [128, 128], f32r)
    din("aT", [ADIM, BL * S], f32r)
    din("tT", [TDIM, BL * S], f32r)
    din("vT", [VDIM, BL * S], f32r)
    din("wa", [ADIM, DE], f32r)
    din("wt", [TDIM, DE], f32r)
    din("wv", [VDIM, DE], f32r)
    din("bin_r", [3, 4, 128], f32)
    din("wr1", [3, DE, DE], f32r)
    din("br1_r", [3, 4, 128], f32)
    din("wr2", [3, DE, E], f32r)
    din("br2_b", [3, 128, E], f32)
    din("wqkv", [DEPTH, DE, 3 * DE], bf16)
    din("bqkv_qk", [DEPTH, 8, 128], f32)
    din("bqkv_v", [DEPTH, 1, DE], bf16)
    din("wo", [DEPTH, DE, DE], bf16)
    din("bo_r", [DEPTH, 4, 128], f32)
    din("ws1", [DEPTH, DE, DE], bf16)
    din("bs1_r", [DEPTH, 4, 128], f32)
    din("ws2", [DEPTH, DE, DE], bf16)
    din("we1", [DEPTH, E, DE, DE], bf16)
    din("be1_r", [DEPTH, 24, 128], f32)
    din("we2", [DEPTH, E, DE, DE], bf16)
    din("bmat", [DEPTH, 7, DE], bf16)
    din("wp1", [3 * DE, 3 * DE], bf16)
    din("bp1_row", [1, 3 * DE], bf16)
    din("wh", [3 * DE, NCLS], bf16)
    din("bh_r", [NCLS, 1], f32)
    din("onesrow", [1, 128], bf16)
    din("ident", [128, 128], bf16)
    din("ones_b", [128, 128], bf16)
    din("sel", [E, E * 128], bf16)
    out_d = nc.dram_tensor("out", [NCLS, BL], f32, kind="ExternalOutput")

    with tile.TileContext(nc) as tc:
        _emit(nc, tc, d, out_d, f32, f32r, bf16, fp8, AF, ALU, AX)

    nfix = _split_waits(nc, mybir)
    return nc, nfix


def _emit(nc, tc, d, out_d, f32, f32r, bf16, fp8, AF, ALU, AX):
    from concourse.bass import ds

    import concourse.mybir as mybir
    DR = mybir.MatmulPerfMode.DoubleRow
    dma = nc.sync.dma_start

    def r32(ap):
        return ap.bitcast(f32r)

    cst = tc.alloc_tile_pool(name="cst", bufs=1)
    ph0 = tc.alloc_tile_pool(name="ph0", bufs=1, side="right")  # phase 0/1, released early
    wbig = tc.alloc_tile_pool(name="wbig", bufs=2)
    w512 = tc.alloc_tile_pool(name="w512", bufs=4)
    wexp = tc.alloc_tile_pool(name="wexp", bufs=2)
    px = tc.alloc_tile_pool(name="px", bufs=1)
    ph = tc.alloc_tile_pool(name="ph", bufs=2)
    pxb = tc.alloc_tile_pool(name="pxb", bufs=1)
    pqk = tc.alloc_tile_pool(name="pqk", bufs=1)
    pva = tc.alloc_tile_pool(name="pva", bufs=1)
    pes = tc.alloc_tile_pool(name="pes", bufs=2)
    psm = tc.alloc_tile_pool(name="psm", bufs=2)
    pln = tc.alloc_tile_pool(name="pln", bufs=4)
    peh = tc.alloc_tile_pool(name="peh", bufs=2)
    psml = tc.alloc_tile_pool(name="psml", bufs=4)
    psA = tc.alloc_tile_pool(name="psA", bufs=4, space="PSUM")
    psB = tc.alloc_tile_pool(name="psB", bufs=2, space="PSUM")

    # ---------------- PE warm-up: stream matmuls while input DMAs land ----
    ident = cst.tile([128, 128], bf16)
    dma(ident[:], d["ident"][:])
    bin_r = cst.tile([128, 3, 4], f32)
    dma(bin_r[:], d["bin_r"].rearrange("m c p -> p m c"))
    ones_b = cst.tile([128, 128], bf16)
    dma(ones_b[:], d["ones_b"][:])
    pwu = psA.tile([128, 128], f32, tag="a", name="warmup")
    for _ in range(64):
        nc.tensor.matmul(pwu[:], ident[:], ident[:], start=True, stop=True)

    eps_sb = cst.tile([128, 1], f32)
    nc.gpsimd.memset(eps_sb[:], EPS)
    G_sb = cst.tile([128, E, NTOK], bf16)        # per-expert gate rows, bcast
    gaug = cst.tile([7, NTOK], bf16)             # gates^T rows + ones row
    nc.gpsimd.memset(gaug[:], 1.0)  # rows 0..5 overwritten by gate evictions

    xT = px.tile([128, 4, NTOK], f32r)           # residual stream

    # =======================================================
    # Phase 0: input projections
    # =======================================================
    def in_proj(dname, wname, idim, m):
        kc = idim // 128
        pa = [psA.tile([128, BL * S], f32, tag="a", name=f"pa{i}") for i in range(4)]
        for k in range(kc):
            it = ph0.tile([128, BL * S], f32r, tag="it", bufs=3, name=f"it{m}{k}")
            dma(it[:], d[dname].rearrange("(c p) t -> p c t", p=128)[:, k, :])
            wi = ph0.tile([128, DE], f32r, tag="wi", bufs=3, name=f"wi{m}{k}")
            dma(wi[:], d[wname].rearrange("(c p) o -> p c o", p=128)[:, k, :])
            for mo in range(4):
                nc.tensor.matmul(pa[mo][:], wi[:, ds(mo * 128, 128)], it[:],
                                 start=(k == 0), stop=(k == kc - 1))
        for mo in range(4):
            dst = xT[:, mo, :].rearrange("p (b r) -> p b r", b=BL)[:, :, ds(m * 128, 128)]
            nc.vector.tensor_scalar(out=dst,
                                    in0=pa[mo][:].rearrange("p (b s) -> p b s", b=BL),
                                    scalar1=bin_r[:, m, mo:mo + 1], scalar2=None,
                                    op0=ALU.add)

    in_proj("aT", "wa", ADIM, 0)
    in_proj("tT", "wt", TDIM, 1)
    wq0_t = []
    for k in range(4):
        wqk = wbig.tile([128, 3 * DE], bf16, tag="wq", bufs=4, name=f"wq{k}")
        dma(wqk[:], d["wqkv"][0, ds(k * 128, 128), :])
        wq0_t.append(wqk)
    in_proj("vT", "wv", VDIM, 2)

    # ---------------- remaining constants (after the input DMAs) ----------
    ones_s = cst.tile([128, 128], f32r)          # 1/512 everywhere
    dma(ones_s[:], d["ones_s"][:])
    onesrow = cst.tile([1, 128], bf16)
    dma(onesrow[:], d["onesrow"][:])
    sel_sb = cst.tile([E, E * 128], bf16)
    dma(sel_sb[:], d["sel"][:])
    br1_r = cst.tile([128, 3, 4], f32)
    dma(br1_r[:], d["br1_r"].rearrange("m c p -> p m c"))
    wr2_sb = cst.tile([128, 3, 4, E], f32r)
    dma(wr2_sb[:], d["wr2"].rearrange("m (c p) e -> p m c e", p=128))
    br2_b = cst.tile([128, 3, E], f32)
    dma(br2_b[:], d["br2_b"].rearrange("m p e -> p m e"))
    bqkv_qk = cst.tile([128, DEPTH, 8], f32)
    dma(bqkv_qk[:], d["bqkv_qk"].rearrange("l c p -> p l c"))
    bo_r = cst.tile([128, DEPTH, 4], f32)
    dma(bo_r[:], d["bo_r"].rearrange("l c p -> p l c"))
    bs1_r = cst.tile([128, DEPTH, 4], f32)
    dma(bs1_r[:], d["bs1_r"].rearrange("l c p -> p l c"))
    be1_r = cst.tile([128, DEPTH, 24], f32)
    dma(be1_r[:], d["be1_r"].rearrange("l c p -> p l c"))
    bh_sb = cst.tile([NCLS, 1], f32)
    dma(bh_sb[:], d["bh_r"][:])

    va = pva.tile([128, 6, 8 * 128], bf16)
    nc.gpsimd.memset(
        va[:].rearrange("p t (h w) -> p t h w", h=NH)[:, :, :, 64:128], 1.0)

    # =======================================================
    # layers
    # =======================================================
    def make_xsq():
        return pxb.tile([128, 4, NTOK], bf16, tag="xsq", name="xsq")

    def emit_xsq(xsq, src, n0, eng=None):
        nsl = ds(n0 * NH2, NH2)
        (eng or nc.vector).tensor_tensor(xsq[:, :, nsl], src[:, :, nsl],
                                         src[:, :, nsl], op=ALU.mult)

    def layernorm_n0(src_f32, dst_slice, xsq, n0, all_gpsimd=False):
        if True:
            nsl = ds(n0 * NH2, NH2)
            pst = psB.tile([128, 2, 512], f32, tag="b")
            for k in range(4):
                nc.tensor.matmul(pst[:, 0, 0:NH2], ones_s[:], src_f32[:, k, nsl],
                                 start=(k == 0), stop=(k == 3))
            for k in range(4):
                nc.tensor.matmul(pst[:, 1, 0:NH2], ones_b[:], xsq[:, k, nsl],
                                 start=(k == 0), stop=(k == 3))
            mb = pln.tile([128, NH2], f32, tag="ln", bufs=3)
            nc.scalar.activation(mb[:], pst[:, 0, 0:NH2], AF.Copy)
            qq = pln.tile([128, NH2], f32, tag="ln", bufs=3)
            nc.gpsimd.tensor_tensor(qq[:], mb[:], mb[:], op=ALU.mult)
            vb = pln.tile([128, NH2], f32, tag="ln", bufs=3)
            nc.vector.scalar_tensor_tensor(out=vb[:], in0=pst[:, 1, 0:NH2],
                                           scalar=1.0, in1=qq[:],
                                           op0=ALU.mult, op1=ALU.subtract)
            sq = pln.tile([128, NH2], f32, tag="ln", bufs=3)
            nc.scalar.activation(sq[:], vb[:], AF.Ln, bias=eps_sb[:])
            rb = pln.tile([128, NH2], f32, tag="ln", bufs=3)
            nc.scalar.activation(rb[:], sq[:], AF.Exp, scale=-0.5)
            for k in range(4):
                eng = nc.gpsimd if (all_gpsimd or k >= 2) else nc.vector
                t = pln.tile([128, NH2], f32,
                             tag="lt" if eng is nc.vector else "ltg", bufs=2)
                eng.tensor_tensor(t[:], src_f32[:, k, nsl], mb[:],
                                  op=ALU.subtract)
                eng.tensor_tensor(dst_slice(k, nsl), t[:], rb[:],
                                  op=ALU.mult)

    def layernorm(src_f32, dst_slice, xsq, all_gpsimd=False):
        """src feature-major [128,4,NTOK] fp32 -> dst (via dst_slice(k, nsl)).
        Mean/E[x^2] via ones-matmul (broadcast across partitions); xsq
        precomputed (hoisted into the previous phase for overlap)."""
        for n0 in range(2):
            layernorm_n0(src_f32, dst_slice, xsq, n0, all_gpsimd)

    def emit_phase1():
        # Phase 1: routers + gates (emitted inside layer-0's attention
        # window so router matmuls fill the PE while ACT does softmax)
        rh = ph0.tile([128, 4, 3, BL * S], f32r, tag="hr", bufs=1)
        for m in range(3):
            wr1_sb = wr1_t[m]
            xm = xT[:].rearrange("p c (b mm s) -> p c b mm s", b=BL, mm=3)[:, :, :, m, :]
            for mo in range(4):
                pr = psA.tile([128, BL * S], f32, tag="a")
                for k in range(4):
                    nc.tensor.matmul(pr[:].rearrange("p (b s) -> p b s", b=BL),
                                     wr1_sb[:, k, ds(mo * 128, 128)], xm[:, k, :, :],
                                     start=(k == 0), stop=(k == 3))
                nc.scalar.activation(rh[:, mo, m, :], pr[:], AF.Gelu_apprx_tanh,
                                     bias=br1_r[:, m, mo:mo + 1])

        rlog = cst.tile([128, 6, E], f32)
        for m in range(3):
            for b in range(BL):
                q = b * 3 + m
                prl = psA.tile([128, E], f32, tag="a")
                for k in range(4):
                    nc.tensor.matmul(prl[:], rh[:, k, m, ds(b * 128, 128)],
                                     wr2_sb[:, m, k, :], start=(k == 0), stop=(k == 3))
                nc.vector.tensor_tensor(rlog[:, q, :], prl[:], br2_b[:, m, :], op=ALU.add)

        for q in range(6):
            r = rlog[:, q, :]
            v1 = pln.tile([128, 1], f32, tag="sc")
            nc.vector.tensor_reduce(v1[:], r, op=ALU.max, axis=AX.X)
            m1 = pln.tile([128, E], f32, tag="m6")
            nc.vector.tensor_scalar(out=m1[:], in0=r, scalar1=v1[:], scalar2=None,
                                    op0=ALU.is_equal)
            mk = pln.tile([128, E], f32, tag="m6")
            nc.vector.scalar_tensor_tensor(out=mk[:], in0=m1[:], scalar=-1e9,
                                           in1=r, op0=ALU.mult, op1=ALU.add)
            v2 = pln.tile([128, 1], f32, tag="sc")
            nc.vector.tensor_reduce(v2[:], mk[:], op=ALU.max, axis=AX.X)
            m2 = pln.tile([128, E], f32, tag="m6")
            nc.vector.tensor_scalar(out=m2[:], in0=mk[:], scalar1=v2[:], scalar2=None,
                                    op0=ALU.is_equal)
            dd = pln.tile([128, 1], f32, tag="sc")
            nc.vector.tensor_tensor(dd[:], v1[:], v2[:], op=ALU.subtract)
            g1 = pln.tile([128, 1], f32, tag="sc")
            nc.scalar.activation(g1[:], dd[:], AF.Sigmoid)
            g2 = pln.tile([128, 1], f32, tag="sc")
            nc.vector.tensor_scalar(out=g2[:], in0=g1[:], scalar1=-1.0, scalar2=1.0,
                                    op0=ALU.mult, op1=ALU.add)
            gm2 = pln.tile([128, E], f32, tag="m6")
            nc.vector.tensor_scalar(out=gm2[:], in0=m2[:], scalar1=g2[:], scalar2=None,
                                    op0=ALU.mult)
            gq = pln.tile([128, E], bf16, tag="m6b")
            nc.vector.scalar_tensor_tensor(out=gq[:], in0=m1[:], scalar=g1[:],
                                           in1=gm2[:], op0=ALU.mult, op1=ALU.add)
            pt = psA.tile([E, 128], bf16, tag="a", name="ptg")
            nc.tensor.transpose(pt[:], gq[:], ident[:])
            nc.scalar.activation(gaug[0:6, ds(q * 128, 128)], pt[:], AF.Copy)

        for e in range(E):
            for n0 in range(2):
                pg = psA.tile([128, NH2], f32, tag="a")
                nc.tensor.matmul(pg[:], sel_sb[:, ds(e * 128, 128)],
                                 gaug[0:6, ds(n0 * NH2, NH2)], start=True, stop=True)
                nc.scalar.activation(G_sb[:, e, ds(n0 * NH2, NH2)], pg[:], AF.Copy)

        ph0.release()

        wp1p = tc.alloc_tile_pool(name="wp1p", bufs=1, side="right")
        return wp1p

    for layer in range(DEPTH):
        if layer == 0:
            wq_t = wq0_t
        else:
            wq_t = []
            for k in range(4):
                wqk = wbig.tile([128, 3 * DE], bf16, tag="wq", bufs=4,
                                name=f"wq{k}")
                dma(wqk[:], d["wqkv"][layer, ds(k * 128, 128), :])
                wq_t.append(wqk)
        bqv = psml.tile([1, DE], bf16, tag="bqv", bufs=1)
        dma(bqv[:], d["bqkv_v"][layer])
        bmat_sb = psml.tile([7, DE], bf16, tag="bm", bufs=1)
        dma(bmat_sb[:], d["bmat"][layer])
        wo_sb = w512.tile([128, 4, DE], bf16, tag="w")
        dma(wo_sb[:], d["wo"][layer].rearrange("(c p) o -> p c o", p=128))
        ws1_sb = w512.tile([128, 4, DE], bf16, tag="w")
        dma(ws1_sb[:], d["ws1"][layer].rearrange("(c p) o -> p c o", p=128))
        ws2_sb = w512.tile([128, 4, DE], bf16, tag="w")
        dma(ws2_sb[:], d["ws2"][layer].rearrange("(c p) o -> p c o", p=128))
        if layer == 0:
            wr1_t = []
            for m in range(3):
                wr1_sb = ph0.tile([128, 4, DE], f32r, tag="wr", bufs=2,
                                  name="wr1_sb")
                dma(wr1_sb[:], d["wr1"][m].rearrange("(c p) o -> p c o", p=128))
                wr1_t.append(wr1_sb)

        # expert weights: resident for the whole layer, loaded once
        we_t = []
        for e in range(E):
            w1 = wexp.tile([128, 4, DE], bf16, tag="we", bufs=7, name=f"w1_{e}")
            dma(w1[:], d["we1"][layer, e].rearrange("(c p) o -> p c o", p=128))
            we_t.append(w1)
        if layer == DEPTH - 1:
            bp1_row = wp1p.tile([1, 3 * DE], bf16, tag="bp1")
            dma(bp1_row[:], d["bp1_row"][:])
            wp1_sb = []
            for g in range(4):
                wpg = wp1p.tile([128, 3, 3 * DE], bf16, tag="wp", bufs=3,
                                name=f"wp{g}")
                dma(wpg[:], d["wp1"].rearrange("(c p) o -> p c o", p=128)
                    [:, ds(g * 3, 3), :])
                wp1_sb.append(wpg)

        # ---- LN1 ----
        if layer == 0:
            xsq_next = make_xsq()
            emit_xsq(xsq_next, xT, 0, nc.gpsimd)
            emit_xsq(xsq_next, xT, 1, nc.gpsimd)
        hT = ph.tile([128, 4, NTOK], bf16, tag="h", bufs=2, name="hT")
        layernorm(xT, lambda k, nsl: hT[:, k, nsl], xsq_next)

        # ---- qkv: q,k feature-major ----
        qkT = pqk.tile([128, 8, NTOK], bf16)
        for mo in range(8):
            pq2 = [psA.tile([128, NH2], f32, tag="a", name=f"pq{i}") for i in range(2)]
            for k in range(4):
                for n0 in range(2):
                    nc.tensor.matmul(pq2[n0][:], wq_t[k][:, ds(mo * 128, 128)],
                                     hT[:, k, ds(n0 * NH2, NH2)],
                                     start=(k == 0), stop=(k == 3))
            for n0 in range(2):
                nc.vector.tensor_scalar(out=qkT[:, mo, ds(n0 * NH2, NH2)],
                                        in0=pq2[n0][:],
                                        scalar1=bqkv_qk[:, layer, mo:mo + 1],
                                        scalar2=None, op0=ALU.add)
        # ---- v token-major, bias via rank-1, into V_aug (pairs of tq) ----
        for tp in range(3):
            pv = psB.tile([128, 2, 512], f32, tag="b")
            for j in range(2):
                tq = tp * 2 + j
                for k in range(4):
                    nc.tensor.matmul(pv[:, j, :], hT[:, k, ds(tq * 128, 128)],
                                     wq_t[k][:, ds(2 * DE, DE)],
                                     start=(k == 0), stop=False)
                nc.tensor.matmul(pv[:, j, :], onesrow[0:1, 0:128], bqv[:],
                                 start=False, stop=True)
            for j in range(2):
                tq = tp * 2 + j
                dst = va[:, tq, :].rearrange("p (h w) -> p h w", h=NH)[:, :, 0:64]
                nc.scalar.activation(dst,
                                     pv[:, j, :].rearrange("p (h e) -> p h e", h=NH),
                                     AF.Copy)

        # ---- attention per (b, head): cross-pair software pipeline ----
        oT = ph.tile([128, 4, NTOK], bf16, tag="h", bufs=2, name="oT")
        prs = [(b, hp) for b in range(BL) for hp in range(NH // 2)]
        est = {}

        def qk_stage(pi):
            b, hp = prs[pi]
            pks, ess = [], []
            for h in (2 * hp, 2 * hp + 1):
                r0 = 64 * (h % 2)
                ck = 4 + h // 2
                qs = qkT[ds(r0, 64), h // 2, ds(b * 384, 384)]
                pk2 = psB.tile([128, 2, 512], f32, tag="b")
                for i in range(2):
                    nc.tensor.matmul(pk2[:, i, 0:NH2],
                                     qkT[ds(r0, 64), ck,
                                         ds(b * 384 + i * 128, 128)],
                                     qs, start=True, stop=True)
                pk1 = psA.tile([128, NH2], f32, tag="a")
                nc.tensor.matmul(pk1[:],
                                 qkT[ds(r0, 64), ck, ds(b * 384 + 256, 128)],
                                 qs, start=True, stop=True)
                pks.append((pk2, pk1))
            for j in range(2):
                pk2, pk1 = pks[j]
                es = pes.tile([128, 3, NH2], bf16, name=f"es{j}")
                nc.scalar.activation(es[:, 0:2, :], pk2[:, :, 0:NH2],
                                     AF.Exp, scale=0.125)
                nc.scalar.activation(es[:, 2, :], pk1[:], AF.Exp, scale=0.125)
                ess.append(es)
            est[pi] = ess

        def av_stage(pi):
            b, hp = prs[pi]
            ess = est[pi]
            spair = psm.tile([128, NH2], f32, tag="s")
            opair = psm.tile([128, NH2], f32, tag="o", bufs=1)
            for j, h in enumerate((2 * hp, 2 * hp + 1)):
                r0 = 64 * (h % 2)
                es = ess[j]
                po = psA.tile([128, NH2], f32, tag="a", name=f"po{h%2}")
                for i in range(3):
                    nc.tensor.matmul(po[:], va[:, b * 3 + i, ds(h * 128, 128)],
                                     es[:, i, :], start=(i == 0), stop=(i == 2))
                nc.vector.tensor_copy(spair[ds(r0, 64), :], po[ds(64, 64), :])
                nc.vector.tensor_copy(opair[ds(r0, 64), :], po[ds(0, 64), :])
            sln = psm.tile([128, NH2], f32, tag="s")
            nc.scalar.activation(sln[:], spair[:], AF.Ln)
            rcp = psm.tile([128, NH2], f32, tag="s")
            nc.scalar.activation(rcp[:], sln[:], AF.Exp, scale=-1.0)
            for h in (2 * hp, 2 * hp + 1):
                r0 = 64 * (h % 2)
                nc.gpsimd.tensor_tensor(oT[ds(r0, 64), h // 2, ds(b * 384, 384)],
                                        opair[ds(r0, 64), :], rcp[ds(r0, 64), :],
                                        op=ALU.mult)

        qk_stage(0)
        for pi in range(1, len(prs)):
            qk_stage(pi)
            av_stage(pi - 1)
        av_stage(len(prs) - 1)

        if layer == 0:
            wp1p = emit_phase1()

        # ---- attention out-projection + residual (n0-major so the n0=0
        # half of xT finalizes early and LN2 stats can start) ----
        xsq2 = make_xsq()
        for n0 in range(2):
            nsl = ds(n0 * NH2, NH2)
            for mo in range(4):
                pp = psA.tile([128, NH2], f32, tag="a", name="pp")
                for k in range(4):
                    nc.tensor.matmul(pp[:], wo_sb[:, k, ds(mo * 128, 128)],
                                     oT[:, k, nsl], start=(k == 0), stop=(k == 3))
                nc.vector.scalar_tensor_tensor(out=xT[:, mo, nsl], in0=pp[:],
                                               scalar=bo_r[:, layer, mo:mo + 1],
                                               in1=xT[:, mo, nsl],
                                               op0=ALU.add, op1=ALU.add)
            emit_xsq(xsq2, xT, n0)

        # ---- LN2 ----
        h2 = ph.tile([128, 4, NTOK], bf16, tag="h", bufs=2, name="h2")
        layernorm(xT, lambda k, nsl: h2[:, k, nsl], xsq2)

        # ---- MoE: shared expert + 6 gated experts, software-pipelined ----
        xsq_next = make_xsq()
        for n0 in range(2):
            nsl = ds(n0 * NH2, NH2)
            pd = [psA.tile([128, NH2], f32, tag="a", name=f"pd{i}") for i in range(4)]
            for mo in range(4):
                nc.tensor.matmul(pd[mo][:], bmat_sb[:, ds(mo * 128, 128)],
                                 gaug[:, nsl], start=True, stop=False)
            su = peh.tile([128, 4, NH2], bf16, tag="eh")
            for p in range(2):
                pu = psB.tile([128, 2, 512], f32, tag="b")
                for j in range(2):
                    mo = p * 2 + j
                    for k in range(4):
                        nc.tensor.matmul(pu[:, j, 0:NH2],
                                         ws1_sb[:, k, ds(mo * 128, 128)],
                                         h2[:, k, nsl], start=(k == 0), stop=(k == 3))
                    nc.scalar.activation(su[:, mo, :], pu[:, j, 0:NH2],
                                         AF.Gelu_apprx_tanh,
                                         bias=bs1_r[:, layer, mo:mo + 1])
            for mo in range(4):
                for k in range(4):
                    nc.tensor.matmul(pd[mo][:], ws2_sb[:, k, ds(mo * 128, 128)],
                                     su[:, k, :], start=False, stop=False)

            ehs = []

            def up_expert(e):
                w1 = we_t[e]
                eh = peh.tile([128, 4, NH2], bf16, tag="eh", name=f"eh{e%2}")
                for p in range(2):
                    pu = psB.tile([128, 2, 512], f32, tag="b")
                    for j in range(2):
                        mo = p * 2 + j
                        for k in range(4):
                            nc.tensor.matmul(pu[:, j, 0:NH2],
                                             w1[:, k, ds(mo * 128, 128)],
                                             h2[:, k, nsl],
                                             start=(k == 0), stop=(k == 3))
                        nc.scalar.activation(
                            eh[:, mo, :], pu[:, j, 0:NH2],
                            AF.Gelu_apprx_tanh,
                            bias=be1_r[:, layer, e * 4 + mo:e * 4 + mo + 1])
                    eng = (nc.vector if (p == 0 or
                           (layer == DEPTH - 1 and n0 == 1)) else nc.gpsimd)
                    for j in range(2):
                        mo = p * 2 + j
                        eng.tensor_tensor(eh[:, mo, :], eh[:, mo, :],
                                          G_sb[:, e, nsl], op=ALU.mult)
                ehs.append(eh)

            def down_expert(e):
                w2 = wexp.tile([128, 4, DE], bf16, tag="we2", bufs=3, name=f"w2_{e}")
                dma(w2[:], d["we2"][layer, e].rearrange("(c p) o -> p c o", p=128))
                eh = ehs[e]
                last = (e == E - 1)
                for mo in range(4):
                    for k in range(4):
                        nc.tensor.matmul(pd[mo][:], w2[:, k, ds(mo * 128, 128)],
                                         eh[:, k, :], start=False,
                                         stop=(last and k == 3))

            up_expert(0)
            for e in range(1, E):
                up_expert(e)
                down_expert(e - 1)
            down_expert(E - 1)

            for mo in range(4):
                nc.vector.tensor_tensor(xT[:, mo, nsl], pd[mo][:],
                                        xT[:, mo, nsl], op=ALU.add)
            emit_xsq(xsq_next, xT, n0)

    # =======================================================
    # final LN + mean-pool + head
    # =======================================================
    fT = ph.tile([128, 4, NTOK], bf16, tag="h", bufs=2, name="fT")
    layernorm(xT, lambda k, nsl: fT[:, k, nsl], xsq_next, all_gpsimd=True)

    pooled = wp1p.tile([128, 24], f32, tag="pool")
    pooledb = wp1p.tile([128, 24], bf16, tag="poolb")
    pview = pooled[:].rearrange("p (m k b) -> p b m k", m=3, k=4, b=BL)
    for k in range(4):
        for b in range(BL):
            nc.vector.tensor_reduce(
                pview[:, b, :, k],
                fT[:, k, ds(b * 384, 384)].rearrange("p (m s) -> p m s", m=3),
                op=ALU.add, axis=AX.X)
    nc.vector.tensor_scalar(out=pooledb[:], in0=pooled[:], scalar1=1.0 / S,
                            scalar2=None, op0=ALU.mult)

    # fused = relu(pooled @ Wp1 + bp1), token-major [BL, 1536]
    pfs = [psA.tile([BL, DE], f32, tag="a", name=f"pfs{i}") for i in range(3)]
    for kj in range(12):
        for ns in range(3):
            nc.tensor.matmul(pfs[ns][:], pooledb[:, ds(kj * 2, BL)],
                             wp1_sb[kj // 3][:, kj % 3, ds(ns * DE, DE)],
                             start=(kj == 0), stop=False)
    for ns in range(3):
        nc.tensor.matmul(pfs[ns][:], onesrow[0:1, 0:BL],
                         bp1_row[:, ds(ns * DE, DE)], start=False, stop=True)
    fused_sb = wp1p.tile([BL, 3 * DE], bf16, tag="fus")
    for ns in range(3):
        nc.scalar.activation(fused_sb[:, ds(ns * DE, DE)], pfs[ns][:], AF.Relu)

    fusedT = wp1p.tile([128, 12, BL], bf16, tag="fusT")
    for kj in range(12):
        pft = psB.tile([128, BL], bf16, tag="b", name="pft")
        nc.tensor.transpose(pft[:], fused_sb[:, ds(kj * 128, 128)], ident[0:BL, 0:BL])
        nc.scalar.activation(fusedT[:, kj, :], pft[:], AF.Copy)

    wh_sb = w512.tile([128, 12, NCLS], bf16, tag="w")
    dma(wh_sb[:], d["wh"].rearrange("(c p) o -> p c o", p=128))
    pout = psA.tile([NCLS, BL], f32, tag="a")
    for kj in range(12):
        nc.tensor.matmul(pout[:], wh_sb[:, kj, :], fusedT[:, kj, :],
                         start=(kj == 0), stop=(kj == 11))
    osb = wp1p.tile([NCLS, BL], f32, tag="osb")
    nc.scalar.activation(osb[:], pout[:], AF.Identity, bias=bh_sb[:, 0:1])
    dma(out_d[:], osb[:])

    for pool in [psB, psA, psml, peh, pln, psm, pes, pva, pqk, pxb, ph, px,
                 wexp, w512, wbig, wp1p, cst]:
        pool.release()


def _host_prep(inputs):
    p = {k: np.asarray(v) for k, v in inputs.items()}

    shared = {}
    shared["wa"] = _f32(p["Wa"])
    shared["wt"] = _f32(p["Wt"])
    shared["wv"] = _f32(p["Wv"])
    shared["bin_r"] = _f32(np.stack([p["ba"].reshape(4, 128),
                                     p["bt"].reshape(4, 128),
                                     p["bv"].reshape(4, 128)]))
    shared["wr1"] = _f32(p["Wr1"])
    shared["br1_r"] = _f32(np.asarray(p["br1"]).reshape(3, 4, 128))
    shared["wr2"] = _f32(p["Wr2"])
    shared["br2_b"] = _f32(np.broadcast_to(np.asarray(p["br2"])[:, None, :],
                                           (3, 128, E)))
    shared["wqkv"] = _bf16(p["Wqkv"])
    shared["bqkv_qk"] = _f32(np.asarray(p["bqkv"])[:, :1024].reshape(DEPTH, 8, 128))
    shared["bqkv_v"] = _bf16(np.asarray(p["bqkv"])[:, 1024:].reshape(DEPTH, 1, DE))
    shared["wo"] = _bf16(p["Wo"])
    shared["bo_r"] = _f32(np.asarray(p["bo"]).reshape(DEPTH, 4, 128))
    shared["ws1"] = _bf16(p["Ws1"])
    shared["bs1_r"] = _f32(np.asarray(p["bs1"]).reshape(DEPTH, 4, 128))
    shared["ws2"] = _bf16(p["Ws2"])
    shared["we1"] = _bf16(p["We1"])
    shared["be1_r"] = _f32(np.asarray(p["be1"]).reshape(DEPTH, 24, 128))
    shared["we2"] = _bf16(p["We2"])
    bmat = np.concatenate([np.asarray(p["be2"]),
                           np.asarray(p["bs2"])[:, None, :]], axis=1)
    shared["bmat"] = _bf16(bmat)
    shared["wp1"] = _bf16(p["Wp1"])
    shared["bp1_row"] = _bf16(np.asarray(p["bp1"]).reshape(1, 3 * DE))
    shared["wh"] = _bf16(p["Wh"])
    shared["bh_r"] = _f32(np.asarray(p["bh"]).reshape(NCLS, 1))
    shared["ones_s"] = _f32(np.full((128, 128), 1.0 / DE, np.float32))
    shared["ones_b"] = _bf16(np.full((128, 128), 1.0 / DE, np.float32))
    shared["onesrow"] = _bf16(np.ones((1, 128), np.float32))
    shared["ident"] = _bf16(np.eye(128, dtype=np.float32))
    sel = np.zeros((E, E * 128), np.float32)
    for e in range(E):
        sel[e, e * 128:(e + 1) * 128] = 1.0
    shared["sel"] = _bf16(sel)

    in_maps = []
    for c in range(NCORES):
        sl = slice(BL * c, BL * (c + 1))
        m = dict(shared)
        m["aT"] = _f32(np.asarray(p["audio"])[sl].transpose(2, 0, 1)
                       .reshape(ADIM, BL * S))
        m["tT"] = _f32(np.asarray(p["text"])[sl].transpose(2, 0, 1)
                       .reshape(TDIM, BL * S))
        m["vT"] = _f32(np.asarray(p["visual"])[sl].transpose(2, 0, 1)
                       .reshape(VDIM, BL * S))
        in_maps.append(m)
    return in_maps


def kernel(**inputs):
    from concourse.bass_utils import run_bass_kernel_spmd

    if "nc" not in _CACHE:
        _CACHE["nc"] = _build()
    nc, _ = _CACHE["nc"]

    in_maps = _host_prep(inputs)
    res = run_bass_kernel_spmd(nc, in_maps, core_ids=list(range(NCORES)))
    out = np.empty((B, NCLS), np.float32)
    for c in range(NCORES):
        out[BL * c: BL * (c + 1)] = res.results[c]["out"].T
    return out


# revision 47
# speedup vs baseline: 1.2510x; 1.0117x over previous
"""MoMKE (multimodal MoE transformer) forward on 8 trn2 NeuronCores.

Sharding: pure data-parallel over batch (B=16 -> 2 batch elements per core).
Each core runs the full model on its 2 sequences; no collectives.

On-device layout: activations are feature-major ("transposed", [feature,
token]) so weights in natural [in, out] layout serve directly as matmul
lhsT.  Tokens per core: 768 columns, ordered col = b*384 + m*128 + s for
local batch b in {0,1}, modality m in {a,t,v}, position s.

Scheduling: PE warm-up matmul stream during the initial input DMA;
expert loop software-pipelined (down-proj of expert e issued after
up-proj of e+1) with expert up-weights resident in SBUF per layer;
attention runs a cross-pair pipeline (QK matmuls of pair p+1 issued
before AV of pair p; adjacent heads at partition bases 0/64 execute
concurrently in separate PE row groups); router/gating phase is emitted
inside layer-0's attention window so its matmuls fill the PE while the
scalar engine does softmax; PSUM is organized as 4x single-bank + 2x
double-bank tiles; softmax reciprocal is Exp(-Ln(x)) on the scalar
engine (Ln/Exp share one ACT table); E[x^2] for each LayerNorm is
hoisted into the preceding phase; the residual stream is natively f32r
so stat matmuls need no dtype-copy; out-projection is n0-major so LN2
statistics start on the first token half early; elementwise work is
split across vector/gpsimd/scalar to keep the PE streaming.
"""

import numpy as np

B, S = 16, 128
ADIM, TDIM, VDIM = 512, 768, 1024
DE = 512
DEPTH = 4
NH = 8
HD = 64
E = 6
NCLS = 6
EPS = 1e-5
NCORES = 8
BL = B // NCORES          # local batch: 2
NTOK = BL * 3 * S         # 768 tokens/core
NH2 = 384                 # half of token columns (matmul N tile)

_CACHE = {}


def _f32(a):
    return np.ascontiguousarray(np.asarray(a, dtype=np.float32))


def _bf16(a):
    import ml_dtypes
    return np.ascontiguousarray(
        np.asarray(a, dtype=np.float32).astype(ml_dtypes.bfloat16))


def _fp8i(a):
    """[..., K=512, M] -> DoubleRow-interleaved fp8 [..., 128, 2, 2, M]."""
    import ml_dtypes
    a = np.asarray(a, dtype=np.float32)
    lead, (K, M) = a.shape[:-2], a.shape[-2:]
    assert K == 512
    b = a.reshape(*lead, 2, 2, 128, M)
    b = np.moveaxis(b, -2, -4)
    return np.ascontiguousarray(b.astype(ml_dtypes.float8_e4m3fn))


def _split_waits(nc, mybir):
    """This walrus build accepts at most one sync wait / one sync update per
    ISA instruction; Tile's sem assignment can attach several.  Spread the
    extras onto same-engine no-ops."""
    n = 0
    for bb in nc.main_func.blocks:
        insts = list(bb.instructions)
        out = []
        changed = False
        for ins in insts:
            si = ins.sync_info
            if si is None:
                out.append(ins)
                continue
            waits = list(si.on_wait or [])
            updates = list(si.on_update or [])
            post = []
            if len(waits) > 1 or len(updates) > 1:
                for w in waits[:-1]:
                    n += 1
                    nop = mybir.InstNoOp(name=f"xw-{n}", ins=[], outs=[])
                    nop.engine = ins.engine
                    nop.sync_info = mybir.SyncInfo(on_wait=[w], on_update=[])
                    out.append(nop)
                for u in updates[1:]:
                    n += 1
                    nop = mybir.InstNoOp(name=f"xu-{n}", ins=[], outs=[])
                    nop.engine = ins.engine
                    nop.sync_info = mybir.SyncInfo(on_wait=[], on_update=[u])
                    post.append(nop)
                ins.sync_info = mybir.SyncInfo(on_wait=waits[-1:],
                                               on_update=updates[:1])
                changed = True
            out.append(ins)
            out.extend(post)
        if changed:
            bb.instructions[:] = out
    return n


def _build():
    import concourse.bass as bass
    import concourse.mybir as mybir
    import concourse.tile as tile

    f32 = mybir.dt.float32
    fp8 = mybir.dt.float8e4
    f32r = mybir.dt.float32r
    bf16 = mybir.dt.bfloat16
    AF = mybir.ActivationFunctionType
    ALU = mybir.AluOpType
    AX = mybir.AxisListType

    nc = bass.Bass()

    d = {}

    def din(name, shape, dt):
        d[name] = nc.dram_tensor(name, shape, dt, kind="ExternalInput")

    din("ones_s", [128, 128], f32r)
    din("aT", [ADIM, BL * S], f32r)
    din("tT", [TDIM, BL * S], f32r)
    din("vT", [VDIM, BL * S], f32r)
    din("wa", [ADIM, DE], f32r)
    din("wt", [TDIM, DE], f32r)
    din("wv", [VDIM, DE], f32r)
    din("bin_r", [3, 4, 128], f32)
    din("wr1", [3, DE, DE], f32r)
    din("br1_r", [3, 4, 128], f32)
    din("wr2", [3, DE, E], f32r)
    din("br2_b", [3, 128, E], f32)
    din("wqkv", [DEPTH, DE, 3 * DE], bf16)
    din("bqkv_qk", [DEPTH, 8, 128], f32)
    din("bqkv_v", [DEPTH, 1, DE], bf16)
    din("wo", [DEPTH, DE, DE], bf16)
    din("bo_r", [DEPTH, 4, 128], f32)
    din("ws1", [DEPTH, DE, DE], bf16)
    din("bs1_r", [DEPTH, 4, 128], f32)
    din("ws2", [DEPTH, DE, DE], bf16)
    din("we1", [DEPTH, E, DE, DE], bf16)
    din("be1_r", [DEPTH, 24, 128], f32)
    din("we2", [DEPTH, E, DE, DE], bf16)
    din("bmat", [DEPTH, 7, DE], bf16)
    din("wp1", [3 * DE, 3 * DE], bf16)
    din("bp1_row", [1, 3 * DE], bf16)
    din("wh", [3 * DE, NCLS], bf16)
    din("bh_r", [NCLS, 1], f32)
    din("onesrow", [1, 128], bf16)
    din("ident", [128, 128], bf16)
    din("ones_b", [128, 128], bf16)
    din("sel", [E, E * 128], bf16)
    out_d = nc.dram_tensor("out", [NCLS, BL], f32, kind="ExternalOutput")

    with tile.TileContext(nc) as tc:
        _emit(nc, tc, d, out_d, f32, f32r, bf16, fp8, AF, ALU, AX)

    nfix = _split_waits(nc, mybir)
    return nc, nfix


def _emit(nc, tc, d, out_d, f32, f32r, bf16, fp8, AF, ALU, AX):
    from concourse.bass import ds

    import concourse.mybir as mybir
    DR = mybir.MatmulPerfMode.DoubleRow
    dma = nc.sync.dma_start

    def r32(ap):
        return ap.bitcast(f32r)

    cst = tc.alloc_tile_pool(name="cst", bufs=1)
    ph0 = tc.alloc_tile_pool(name="ph0", bufs=1, side="right")  # phase 0/1, released early
    wbig = tc.alloc_tile_pool(name="wbig", bufs=2)
    w512 = tc.alloc_tile_pool(name="w512", bufs=4)
    wexp = tc.alloc_tile_pool(name="wexp", bufs=2)
    px = tc.alloc_tile_pool(name="px", bufs=1)
    ph = tc.alloc_tile_pool(name="ph", bufs=2)
    pxb = tc.alloc_tile_pool(name="pxb", bufs=1)
    pqk = tc.alloc_tile_pool(name="pqk", bufs=1)
    pva = tc.alloc_tile_pool(name="pva", bufs=1)
    pes = tc.alloc_tile_pool(name="pes", bufs=2)
    psm = tc.alloc_tile_pool(name="psm", bufs=2)
    pln = tc.alloc_tile_pool(name="pln", bufs=4)
    peh = tc.alloc_tile_pool(name="peh", bufs=2)
    psml = tc.alloc_tile_pool(name="psml", bufs=4)
    psA = tc.alloc_tile_pool(name="psA", bufs=4, space="PSUM")
    psB = tc.alloc_tile_pool(name="psB", bufs=2, space="PSUM")

    # ---------------- PE warm-up: stream matmuls while input DMAs land ----
    ident = cst.tile([128, 128], bf16)
    dma(ident[:], d["ident"][:])
    bin_r = cst.tile([128, 3, 4], f32)
    dma(bin_r[:], d["bin_r"].rearrange("m c p -> p m c"))
    ones_b = cst.tile([128, 128], bf16)
    dma(ones_b[:], d["ones_b"][:])
    pwu = psA.tile([128, 128], f32, tag="a", name="warmup")
    for _ in range(64):
        nc.tensor.matmul(pwu[:], ident[:], ident[:], start=True, stop=True)

    eps_sb = cst.tile([128, 1], f32)
    nc.gpsimd.memset(eps_sb[:], EPS)
    G_sb = cst.tile([128, E, NTOK], bf16)        # per-expert gate rows, bcast
    gaug = cst.tile([7, NTOK], bf16)             # gates^T rows + ones row
    nc.gpsimd.memset(gaug[:], 1.0)  # rows 0..5 overwritten by gate evictions

    xT = px.tile([128, 4, NTOK], f32r)           # residual stream

    # =======================================================
    # Phase 0: input projections
    # =======================================================
    def in_proj(dname, wname, idim, m):
        kc = idim // 128
        pa = [psA.tile([128, BL * S], f32, tag="a", name=f"pa{i}") for i in range(4)]
        for k in range(kc):
            it = ph0.tile([128, BL * S], f32r, tag="it", bufs=3, name=f"it{m}{k}")
            dma(it[:], d[dname].rearrange("(c p) t -> p c t", p=128)[:, k, :])
            wi = ph0.tile([128, DE], f32r, tag="wi", bufs=3, name=f"wi{m}{k}")
            dma(wi[:], d[wname].rearrange("(c p) o -> p c o", p=128)[:, k, :])
            for mo in range(4):
                nc.tensor.matmul(pa[mo][:], wi[:, ds(mo * 128, 128)], it[:],
                                 start=(k == 0), stop=(k == kc - 1))
        for mo in range(4):
            dst = xT[:, mo, :].rearrange("p (b r) -> p b r", b=BL)[:, :, ds(m * 128, 128)]
            nc.vector.tensor_scalar(out=dst,
                                    in0=pa[mo][:].rearrange("p (b s) -> p b s", b=BL),
                                    scalar1=bin_r[:, m, mo:mo + 1], scalar2=None,
                                    op0=ALU.add)

    in_proj("aT", "wa", ADIM, 0)
    in_proj("tT", "wt", TDIM, 1)
    wq0_t = []
    for k in range(4):
        wqk = wbig.tile([128, 3 * DE], bf16, tag="wq", bufs=4, name=f"wq{k}")
        dma(wqk[:], d["wqkv"][0, ds(k * 128, 128), :])
        wq0_t.append(wqk)
    in_proj("vT", "wv", VDIM, 2)

    # ---------------- remaining constants (after the input DMAs) ----------
    ones_s = cst.tile([128, 128], f32r)          # 1/512 everywhere
    dma(ones_s[:], d["ones_s"][:])
    onesrow = cst.tile([1, 128], bf16)
    dma(onesrow[:], d["onesrow"][:])
    sel_sb = cst.tile([E, E * 128], bf16)
    dma(sel_sb[:], d["sel"][:])
    br1_r = cst.tile([128, 3, 4], f32)
    dma(br1_r[:], d["br1_r"].rearrange("m c p -> p m c"))
    wr2_sb = cst.tile([128, 3, 4, E], f32r)
    dma(wr2_sb[:], d["wr2"].rearrange("m (c p) e -> p m c e", p=128))
    br2_b = cst.tile([128, 3, E], f32)
    dma(br2_b[:], d["br2_b"].rearrange("m p e -> p m e"))
    bqkv_qk = cst.tile([128, DEPTH, 8], f32)
    dma(bqkv_qk[:], d["bqkv_qk"].rearrange("l c p -> p l c"))
    bo_r = cst.tile([128, DEPTH, 4], f32)
    dma(bo_r[:], d["bo_r"].rearrange("l c p -> p l c"))
    bs1_r = cst.tile([128, DEPTH, 4], f32)
    dma(bs1_r[:], d["bs1_r"].rearrange("l c p -> p l c"))
    be1_r = cst.tile([128, DEPTH, 24], f32)
    dma(be1_r[:], d["be1_r"].rearrange("l c p -> p l c"))
    bh_sb = cst.tile([NCLS, 1], f32)
    dma(bh_sb[:], d["bh_r"][:])

    va = pva.tile([128, 6, 8 * 128], bf16)
    nc.gpsimd.memset(
        va[:].rearrange("p t (h w) -> p t h w", h=NH)[:, :, :, 64:128], 1.0)

    # =======================================================
    # layers
    # =======================================================
    def make_xsq():
        return pxb.tile([128, 4, NTOK], bf16, tag="xsq", name="xsq")

    def emit_xsq(xsq, src, n0, eng=None):
        nsl = ds(n0 * NH2, NH2)
        (eng or nc.vector).tensor_tensor(xsq[:, :, nsl], src[:, :, nsl],
                                         src[:, :, nsl], op=ALU.mult)

    def layernorm_n0(src_f32, dst_slice, xsq, n0, all_gpsimd=False):
        if True:
            nsl = ds(n0 * NH2, NH2)
            pst = psB.tile([128, 2, 512], f32, tag="b")
            for k in range(4):
                nc.tensor.matmul(pst[:, 0, 0:NH2], ones_s[:], src_f32[:, k, nsl],
                                 start=(k == 0), stop=(k == 3))
            for k in range(4):
                nc.tensor.matmul(pst[:, 1, 0:NH2], ones_b[:], xsq[:, k, nsl],
                                 start=(k == 0), stop=(k == 3))
            mb = pln.tile([128, NH2], f32, tag="ln", bufs=3)
            nc.scalar.activation(mb[:], pst[:, 0, 0:NH2], AF.Copy)
            qq = pln.tile([128, NH2], f32, tag="ln", bufs=3)
            nc.gpsimd.tensor_tensor(qq[:], mb[:], mb[:], op=ALU.mult)
            vb = pln.tile([128, NH2], f32, tag="ln", bufs=3)
            nc.vector.scalar_tensor_tensor(out=vb[:], in0=pst[:, 1, 0:NH2],
                                           scalar=1.0, in1=qq[:],
                                           op0=ALU.mult, op1=ALU.subtract)
            sq = pln.tile([128, NH2], f32, tag="ln", bufs=3)
            nc.scalar.activation(sq[:], vb[:], AF.Ln, bias=eps_sb[:])
            rb = pln.tile([128, NH2], f32, tag="ln", bufs=3)
            nc.scalar.activation(rb[:], sq[:], AF.Exp, scale=-0.5)
            for k in range(4):
                eng = nc.gpsimd if (all_gpsimd or k >= 2) else nc.vector
                t = pln.tile([128, NH2], f32,
                             tag="lt" if eng is nc.vector else "ltg", bufs=2)
                eng.tensor_tensor(t[:], src_f32[:, k, nsl], mb[:],
                                  op=ALU.subtract)
                eng.tensor_tensor(dst_slice(k, nsl), t[:], rb[:],
                                  op=ALU.mult)

    def layernorm(src_f32, dst_slice, xsq, all_gpsimd=False):
        """src feature-major [128,4,NTOK] fp32 -> dst (via dst_slice(k, nsl)).
        Mean/E[x^2] via ones-matmul (broadcast across partitions); xsq
        precomputed (hoisted into the previous phase for overlap)."""
        for n0 in range(2):
            layernorm_n0(src_f32, dst_slice, xsq, n0, all_gpsimd)

    def emit_phase1():
        # Phase 1: routers + gates (emitted inside layer-0's attention
        # window so router matmuls fill the PE while ACT does softmax)
        rh = ph0.tile([128, 4, 3, BL * S], f32r, tag="hr", bufs=1)
        for m in range(3):
            wr1_sb = wr1_t[m]
            xm = xT[:].rearrange("p c (b mm s) -> p c b mm s", b=BL, mm=3)[:, :, :, m, :]
            for mo in range(4):
                pr = psA.tile([128, BL * S], f32, tag="a")
                for k in range(4):
                    nc.tensor.matmul(pr[:].rearrange("p (b s) -> p b s", b=BL),
                                     wr1_sb[:, k, ds(mo * 128, 128)], xm[:, k, :, :],
                                     start=(k == 0), stop=(k == 3))
                nc.scalar.activation(rh[:, mo, m, :], pr[:], AF.Gelu_apprx_tanh,
                                     bias=br1_r[:, m, mo:mo + 1])

        rlog = cst.tile([128, 6, E], f32)
        for m in range(3):
            for b in range(BL):
                q = b * 3 + m
                prl = psA.tile([128, E], f32, tag="a")
                for k in range(4):
                    nc.tensor.matmul(prl[:], rh[:, k, m, ds(b * 128, 128)],
                                     wr2_sb[:, m, k, :], start=(k == 0), stop=(k == 3))
                nc.vector.tensor_tensor(rlog[:, q, :], prl[:], br2_b[:, m, :], op=ALU.add)

        for q in range(6):
            r = rlog[:, q, :]
            v1 = pln.tile([128, 1], f32, tag="sc")
            nc.vector.tensor_reduce(v1[:], r, op=ALU.max, axis=AX.X)
            m1 = pln.tile([128, E], f32, tag="m6")
            nc.vector.tensor_scalar(out=m1[:], in0=r, scalar1=v1[:], scalar2=None,
                                    op0=ALU.is_equal)
            mk = pln.tile([128, E], f32, tag="m6")
            nc.vector.scalar_tensor_tensor(out=mk[:], in0=m1[:], scalar=-1e9,
                                           in1=r, op0=ALU.mult, op1=ALU.add)
            v2 = pln.tile([128, 1], f32, tag="sc")
            nc.vector.tensor_reduce(v2[:], mk[:], op=ALU.max, axis=AX.X)
            m2 = pln.tile([128, E], f32, tag="m6")
            nc.vector.tensor_scalar(out=m2[:], in0=mk[:], scalar1=v2[:], scalar2=None,
                                    op0=ALU.is_equal)
            dd = pln.tile([128, 1], f32, tag="sc")
            nc.vector.tensor_tensor(dd[:], v1[:], v2[:], op=ALU.subtract)
            g1 = pln.tile([128, 1], f32, tag="sc")
            nc.scalar.activation(g1[:], dd[:], AF.Sigmoid)
            g2 = pln.tile([128, 1], f32, tag="sc")
            nc.vector.tensor_scalar(out=g2[:], in0=g1[:], scalar1=-1.0, scalar2=1.0,
                                    op0=ALU.mult, op1=ALU.add)
            gm2 = pln.tile([128, E], f32, tag="m6")
            nc.vector.tensor_scalar(out=gm2[:], in0=m2[:], scalar1=g2[:], scalar2=None,
                                    op0=ALU.mult)
            gq = pln.tile([128, E], bf16, tag="m6b")
            nc.vector.scalar_tensor_tensor(out=gq[:], in0=m1[:], scalar=g1[:],
                                           in1=gm2[:], op0=ALU.mult, op1=ALU.add)
            pt = psA.tile([E, 128], bf16, tag="a", name="ptg")
            nc.tensor.transpose(pt[:], gq[:], ident[:])
            nc.scalar.activation(gaug[0:6, ds(q * 128, 128)], pt[:], AF.Copy)

        for e in range(E):
            for n0 in range(2):
                pg = psA.tile([128, NH2], f32, tag="a")
                nc.tensor.matmul(pg[:], sel_sb[:, ds(e * 128, 128)],
                                 gaug[0:6, ds(n0 * NH2, NH2)], start=True, stop=True)
                nc.scalar.activation(G_sb[:, e, ds(n0 * NH2, NH2)], pg[:], AF.Copy)

        ph0.release()

        wp1p = tc.alloc_tile_pool(name="wp1p", bufs=1, side="right")
        return wp1p

    for layer in range(DEPTH):
        if layer == 0:
            wq_t = wq0_t
        else:
            wq_t = []
            for k in range(4):
                wqk = wbig.tile([128, 3 * DE], bf16, tag="wq", bufs=4,
                                name=f"wq{k}")
                dma(wqk[:], d["wqkv"][layer, ds(k * 128, 128), :])
                wq_t.append(wqk)
        bqv = psml.tile([1, DE], bf16, tag="bqv", bufs=1)
        dma(bqv[:], d["bqkv_v"][layer])
        bmat_sb = psml.tile([7, DE], bf16, tag="bm", bufs=1)
        dma(bmat_sb[:], d["bmat"][layer])
        wo_sb = w512.tile([128, 4, DE], bf16, tag="w")
        dma(wo_sb[:], d["wo"][layer].rearrange("(c p) o -> p c o", p=128))
        ws1_sb = w512.tile([128, 4, DE], bf16, tag="w")
        dma(ws1_sb[:], d["ws1"][layer].rearrange("(c p) o -> p c o", p=128))
        ws2_sb = w512.tile([128, 4, DE], bf16, tag="w")
        dma(ws2_sb[:], d["ws2"][layer].rearrange("(c p) o -> p c o", p=128))
        if layer == 0:
            wr1_t = []
            for m in range(3):
                wr1_sb = ph0.tile([128, 4, DE], f32r, tag="wr", bufs=2,
                                  name="wr1_sb")
                dma(wr1_sb[:], d["wr1"][m].rearrange("(c p) o -> p c o", p=128))
                wr1_t.append(wr1_sb)

        # expert weights: resident for the whole layer, loaded once
        we_t = []
        for e in range(E):
            w1 = wexp.tile([128, 4, DE], bf16, tag="we", bufs=7, name=f"w1_{e}")
            dma(w1[:], d["we1"][layer, e].rearrange("(c p) o -> p c o", p=128))
            we_t.append(w1)
        if layer == DEPTH - 1:
            bp1_row = wp1p.tile([1, 3 * DE], bf16, tag="bp1")
            dma(bp1_row[:], d["bp1_row"][:])
            wp1_sb = []
            for g in range(4):
                wpg = wp1p.tile([128, 3, 3 * DE], bf16, tag="wp", bufs=3,
                                name=f"wp{g}")
                dma(wpg[:], d["wp1"].rearrange("(c p) o -> p c o", p=128)
                    [:, ds(g * 3, 3), :])
                wp1_sb.append(wpg)

        # ---- LN1 ----
        if layer == 0:
            xsq_next = make_xsq()
            emit_xsq(xsq_next, xT, 0, nc.gpsimd)
            emit_xsq(xsq_next, xT, 1, nc.gpsimd)
        hT = ph.tile([128, 4, NTOK], bf16, tag="h", bufs=2, name="hT")
        layernorm(xT, lambda k, nsl: hT[:, k, nsl], xsq_next)

        # ---- qkv: q,k feature-major ----
        qkT = pqk.tile([128, 8, NTOK], bf16)
        for mo in range(8):
            pq2 = [psA.tile([128, NH2], f32, tag="a", name=f"pq{i}") for i in range(2)]
            for k in range(4):
                for n0 in range(2):
                    nc.tensor.matmul(pq2[n0][:], wq_t[k][:, ds(mo * 128, 128)],
                                     hT[:, k, ds(n0 * NH2, NH2)],
                                     start=(k == 0), stop=(k == 3))
            for n0 in range(2):
                nc.vector.tensor_scalar(out=qkT[:, mo, ds(n0 * NH2, NH2)],
                                        in0=pq2[n0][:],
                                        scalar1=bqkv_qk[:, layer, mo:mo + 1],
                                        scalar2=None, op0=ALU.add)
        # ---- v token-major, bias via rank-1, into V_aug (pairs of tq) ----
        for tp in range(3):
            pv = psB.tile([128, 2, 512], f32, tag="b")
            for j in range(2):
                tq = tp * 2 + j
                for k in range(4):
                    nc.tensor.matmul(pv[:, j, :], hT[:, k, ds(tq * 128, 128)],
                                     wq_t[k][:, ds(2 * DE, DE)],
                                     start=(k == 0), stop=False)
                nc.tensor.matmul(pv[:, j, :], onesrow[0:1, 0:128], bqv[:],
                                 start=False, stop=True)
            for j in range(2):
                tq = tp * 2 + j
                dst = va[:, tq, :].rearrange("p (h w) -> p h w", h=NH)[:, :, 0:64]
                nc.scalar.activation(dst,
                                     pv[:, j, :].rearrange("p (h e) -> p h e", h=NH),
                                     AF.Copy)

        # ---- attention per (b, head): cross-pair software pipeline ----
        oT = ph.tile([128, 4, NTOK], bf16, tag="h", bufs=2, name="oT")
        prs = [(b, hp) for b in range(BL) for hp in range(NH // 2)]
        est = {}

        def qk_stage(pi):
            b, hp = prs[pi]
            pks, ess = [], []
            for h in (2 * hp, 2 * hp + 1):
                r0 = 64 * (h % 2)
                ck = 4 + h // 2
                qs = qkT[ds(r0, 64), h // 2, ds(b * 384, 384)]
                pk2 = psB.tile([128, 2, 512], f32, tag="b")
                for i in range(2):
                    nc.tensor.matmul(pk2[:, i, 0:NH2],
                                     qkT[ds(r0, 64), ck,
                                         ds(b * 384 + i * 128, 128)],
                                     qs, start=True, stop=True)
                pk1 = psA.tile([128, NH2], f32, tag="a")
                nc.tensor.matmul(pk1[:],
                                 qkT[ds(r0, 64), ck, ds(b * 384 + 256, 128)],
                                 qs, start=True, stop=True)
                pks.append((pk2, pk1))
            for j in range(2):
                pk2, pk1 = pks[j]
                es = pes.tile([128, 3, NH2], bf16, name=f"es{j}")
                nc.scalar.activation(es[:, 0:2, :], pk2[:, :, 0:NH2],
                                     AF.Exp, scale=0.125)
                nc.scalar.activation(es[:, 2, :], pk1[:], AF.Exp, scale=0.125)
                ess.append(es)
            est[pi] = ess

        def av_stage(pi):
            b, hp = prs[pi]
            ess = est[pi]
            spair = psm.tile([128, NH2], f32, tag="s", bufs=3)
            opair = psm.tile([128, NH2], f32, tag="o", bufs=1)
            for j, h in enumerate((2 * hp, 2 * hp + 1)):
                r0 = 64 * (h % 2)
                es = ess[j]
                po = psA.tile([128, NH2], f32, tag="a", name=f"po{h%2}")
                for i in range(3):
                    nc.tensor.matmul(po[:], va[:, b * 3 + i, ds(h * 128, 128)],
                                     es[:, i, :], start=(i == 0), stop=(i == 2))
                nc.vector.tensor_copy(spair[ds(r0, 64), :], po[ds(64, 64), :])
                nc.vector.tensor_copy(opair[ds(r0, 64), :], po[ds(0, 64), :])
            sln = psm.tile([128, NH2], f32, tag="s", bufs=3)
            nc.scalar.activation(sln[:], spair[:], AF.Ln)
            rcp = psm.tile([128, NH2], f32, tag="s", bufs=3)
            nc.scalar.activation(rcp[:], sln[:], AF.Exp, scale=-1.0)
            for h in (2 * hp, 2 * hp + 1):
                r0 = 64 * (h % 2)
                nc.gpsimd.tensor_tensor(oT[ds(r0, 64), h // 2, ds(b * 384, 384)],
                                        opair[ds(r0, 64), :], rcp[ds(r0, 64), :],
                                        op=ALU.mult)

        qk_stage(0)
        for pi in range(1, len(prs)):
            qk_stage(pi)
            av_stage(pi - 1)
        av_stage(len(prs) - 1)

        if layer == 0:
            wp1p = emit_phase1()

        # ---- attention out-projection + residual (n0-major so the n0=0
        # half of xT finalizes early and LN2 stats can start) ----
        xsq2 = make_xsq()
        for n0 in range(2):
            nsl = ds(n0 * NH2, NH2)
            for mo in range(4):
                pp = psA.tile([128, NH2], f32, tag="a", name="pp")
                for k in range(4):
                    nc.tensor.matmul(pp[:], wo_sb[:, k, ds(mo * 128, 128)],
                                     oT[:, k, nsl], start=(k == 0), stop=(k == 3))
                nc.vector.scalar_tensor_tensor(out=xT[:, mo, nsl], in0=pp[:],
                                               scalar=bo_r[:, layer, mo:mo + 1],
                                               in1=xT[:, mo, nsl],
                                               op0=ALU.add, op1=ALU.add)
            emit_xsq(xsq2, xT, n0)

        # ---- LN2 ----
        h2 = ph.tile([128, 4, NTOK], bf16, tag="h", bufs=2, name="h2")
        layernorm(xT, lambda k, nsl: h2[:, k, nsl], xsq2)

        # ---- MoE: shared expert + 6 gated experts, software-pipelined ----
        xsq_next = make_xsq()
        for n0 in range(2):
            nsl = ds(n0 * NH2, NH2)
            pd = [psA.tile([128, NH2], f32, tag="a", name=f"pd{i}") for i in range(4)]
            for mo in range(4):
                nc.tensor.matmul(pd[mo][:], bmat_sb[:, ds(mo * 128, 128)],
                                 gaug[:, nsl], start=True, stop=False)
            su = peh.tile([128, 4, NH2], bf16, tag="eh")
            for p in range(2):
                pu = psB.tile([128, 2, 512], f32, tag="b")
                for j in range(2):
                    mo = p * 2 + j
                    for k in range(4):
                        nc.tensor.matmul(pu[:, j, 0:NH2],
                                         ws1_sb[:, k, ds(mo * 128, 128)],
                                         h2[:, k, nsl], start=(k == 0), stop=(k == 3))
                    nc.scalar.activation(su[:, mo, :], pu[:, j, 0:NH2],
                                         AF.Gelu_apprx_tanh,
                                         bias=bs1_r[:, layer, mo:mo + 1])
            for mo in range(4):
                for k in range(4):
                    nc.tensor.matmul(pd[mo][:], ws2_sb[:, k, ds(mo * 128, 128)],
                                     su[:, k, :], start=False, stop=False)

            ehs = []

            def up_expert(e):
                w1 = we_t[e]
                eh = peh.tile([128, 4, NH2], bf16, tag="eh", name=f"eh{e%2}")
                for p in range(2):
                    pu = psB.tile([128, 2, 512], f32, tag="b")
                    for j in range(2):
                        mo = p * 2 + j
                        for k in range(4):
                            nc.tensor.matmul(pu[:, j, 0:NH2],
                                             w1[:, k, ds(mo * 128, 128)],
                                             h2[:, k, nsl],
                                             start=(k == 0), stop=(k == 3))
                        nc.scalar.activation(
                            eh[:, mo, :], pu[:, j, 0:NH2],
                            AF.Gelu_apprx_tanh,
                            bias=be1_r[:, layer, e * 4 + mo:e * 4 + mo + 1])
                    eng = (nc.vector if (p == 0 or
                           (layer == DEPTH - 1 and n0 == 1)) else nc.gpsimd)
                    for j in range(2):
                        mo = p * 2 + j
                        eng.tensor_tensor(eh[:, mo, :], eh[:, mo, :],
                                          G_sb[:, e, nsl], op=ALU.mult)
                ehs.append(eh)

            def down_expert(e):
                w2 = wexp.tile([128, 4, DE], bf16, tag="we2", bufs=3, name=f"w2_{e}")
                dma(w2[:], d["we2"][layer, e].rearrange("(c p) o -> p c o", p=128))
                eh = ehs[e]
                last = (e == E - 1)
                for mo in range(4):
                    for k in range(4):
                        nc.tensor.matmul(pd[mo][:], w2[:, k, ds(mo * 128, 128)],
                                         eh[:, k, :], start=False,
                                         stop=(last and k == 3))

            up_expert(0)
            for e in range(1, E):
                up_expert(e)
                down_expert(e - 1)
            down_expert(E - 1)

            for mo in range(4):
                nc.vector.tensor_tensor(xT[:, mo, nsl], pd[mo][:],
                                        xT[:, mo, nsl], op=ALU.add)
            emit_xsq(xsq_next, xT, n0)

    # =======================================================
    # final LN + mean-pool + head
    # =======================================================
    fT = ph.tile([128, 4, NTOK], bf16, tag="h", bufs=2, name="fT")
    layernorm(xT, lambda k, nsl: fT[:, k, nsl], xsq_next, all_gpsimd=True)

    pooled = wp1p.tile([128, 24], f32, tag="pool")
    pooledb = wp1p.tile([128, 24], bf16, tag="poolb")
    pview = pooled[:].rearrange("p (m k b) -> p b m k", m=3, k=4, b=BL)
    for k in range(4):
        for b in range(BL):
            nc.vector.tensor_reduce(
                pview[:, b, :, k],
                fT[:, k, ds(b * 384, 384)].rearrange("p (m s) -> p m s", m=3),
                op=ALU.add, axis=AX.X)
    nc.vector.tensor_scalar(out=pooledb[:], in0=pooled[:], scalar1=1.0 / S,
                            scalar2=None, op0=ALU.mult)

    # fused = relu(pooled @ Wp1 + bp1), token-major [BL, 1536]
    pfs = [psA.tile([BL, DE], f32, tag="a", name=f"pfs{i}") for i in range(3)]
    for kj in range(12):
        for ns in range(3):
            nc.tensor.matmul(pfs[ns][:], pooledb[:, ds(kj * 2, BL)],
                             wp1_sb[kj // 3][:, kj % 3, ds(ns * DE, DE)],
                             start=(kj == 0), stop=False)
    for ns in range(3):
        nc.tensor.matmul(pfs[ns][:], onesrow[0:1, 0:BL],
                         bp1_row[:, ds(ns * DE, DE)], start=False, stop=True)
    fused_sb = wp1p.tile([BL, 3 * DE], bf16, tag="fus")
    for ns in range(3):
        nc.scalar.activation(fused_sb[:, ds(ns * DE, DE)], pfs[ns][:], AF.Relu)

    fusedT = wp1p.tile([128, 12, BL], bf16, tag="fusT")
    for kj in range(12):
        pft = psB.tile([128, BL], bf16, tag="b", name="pft")
        nc.tensor.transpose(pft[:], fused_sb[:, ds(kj * 128, 128)], ident[0:BL, 0:BL])
        nc.scalar.activation(fusedT[:, kj, :], pft[:], AF.Copy)

    wh_sb = w512.tile([128, 12, NCLS], bf16, tag="w")
    dma(wh_sb[:], d["wh"].rearrange("(c p) o -> p c o", p=128))
    pout = psA.tile([NCLS, BL], f32, tag="a")
    for kj in range(12):
        nc.tensor.matmul(pout[:], wh_sb[:, kj, :], fusedT[:, kj, :],
                         start=(kj == 0), stop=(kj == 11))
    osb = wp1p.tile([NCLS, BL], f32, tag="osb")
    nc.scalar.activation(osb[:], pout[:], AF.Identity, bias=bh_sb[:, 0:1])
    dma(out_d[:], osb[:])

    for pool in [psB, psA, psml, peh, pln, psm, pes, pva, pqk, pxb, ph, px,
                 wexp, w512, wbig, wp1p, cst]:
        pool.release()


def _host_prep(inputs):
    p = {k: np.asarray(v) for k, v in inputs.items()}

    shared = {}
    shared["wa"] = _f32(p["Wa"])
    shared["wt"] = _f32(p["Wt"])
    shared["wv"] = _f32(p["Wv"])
    shared["bin_r"] = _f32(np.stack([p["ba"].reshape(4, 128),
                                     p["bt"].reshape(4, 128),
                                     p["bv"].reshape(4, 128)]))
    shared["wr1"] = _f32(p["Wr1"])
    shared["br1_r"] = _f32(np.asarray(p["br1"]).reshape(3, 4, 128))
    shared["wr2"] = _f32(p["Wr2"])
    shared["br2_b"] = _f32(np.broadcast_to(np.asarray(p["br2"])[:, None, :],
                                           (3, 128, E)))
    shared["wqkv"] = _bf16(p["Wqkv"])
    shared["bqkv_qk"] = _f32(np.asarray(p["bqkv"])[:, :1024].reshape(DEPTH, 8, 128))
    shared["bqkv_v"] = _bf16(np.asarray(p["bqkv"])[:, 1024:].reshape(DEPTH, 1, DE))
    shared["wo"] = _bf16(p["Wo"])
    shared["bo_r"] = _f32(np.asarray(p["bo"]).reshape(DEPTH, 4, 128))
    shared["ws1"] = _bf16(p["Ws1"])
    shared["bs1_r"] = _f32(np.asarray(p["bs1"]).reshape(DEPTH, 4, 128))
    shared["ws2"] = _bf16(p["Ws2"])
    shared["we1"] = _bf16(p["We1"])
    shared["be1_r"] = _f32(np.asarray(p["be1"]).reshape(DEPTH, 24, 128))
    shared["we2"] = _bf16(p["We2"])
    bmat = np.concatenate([np.asarray(p["be2"]),
                           np.asarray(p["bs2"])[:, None, :]], axis=1)
    shared["bmat"] = _bf16(bmat)
    shared["wp1"] = _bf16(p["Wp1"])
    shared["bp1_row"] = _bf16(np.asarray(p["bp1"]).reshape(1, 3 * DE))
    shared["wh"] = _bf16(p["Wh"])
    shared["bh_r"] = _f32(np.asarray(p["bh"]).reshape(NCLS, 1))
    shared["ones_s"] = _f32(np.full((128, 128), 1.0 / DE, np.float32))
    shared["ones_b"] = _bf16(np.full((128, 128), 1.0 / DE, np.float32))
    shared["onesrow"] = _bf16(np.ones((1, 128), np.float32))
    shared["ident"] = _bf16(np.eye(128, dtype=np.float32))
    sel = np.zeros((E, E * 128), np.float32)
    for e in range(E):
        sel[e, e * 128:(e + 1) * 128] = 1.0
    shared["sel"] = _bf16(sel)

    in_maps = []
    for c in range(NCORES):
        sl = slice(BL * c, BL * (c + 1))
        m = dict(shared)
        m["aT"] = _f32(np.asarray(p["audio"])[sl].transpose(2, 0, 1)
                       .reshape(ADIM, BL * S))
        m["tT"] = _f32(np.asarray(p["text"])[sl].transpose(2, 0, 1)
                       .reshape(TDIM, BL * S))
        m["vT"] = _f32(np.asarray(p["visual"])[sl].transpose(2, 0, 1)
                       .reshape(VDIM, BL * S))
        in_maps.append(m)
    return in_maps


def kernel(**inputs):
    from concourse.bass_utils import run_bass_kernel_spmd

    if "nc" not in _CACHE:
        _CACHE["nc"] = _build()
    nc, _ = _CACHE["nc"]

    in_maps = _host_prep(inputs)
    res = run_bass_kernel_spmd(nc, in_maps, core_ids=list(range(NCORES)))
    out = np.empty((B, NCLS), np.float32)
    for c in range(NCORES):
        out[BL * c: BL * (c + 1)] = res.results[c]["out"].T
    return out


# revision 48
# speedup vs baseline: 1.2530x; 1.0016x over previous
"""MoMKE (multimodal MoE transformer) forward on 8 trn2 NeuronCores.

Sharding: pure data-parallel over batch (B=16 -> 2 batch elements per core).
Each core runs the full model on its 2 sequences; no collectives.

On-device layout: activations are feature-major ("transposed", [feature,
token]) so weights in natural [in, out] layout serve directly as matmul
lhsT.  Tokens per core: 768 columns, ordered col = b*384 + m*128 + s for
local batch b in {0,1}, modality m in {a,t,v}, position s.

Scheduling: PE warm-up matmul stream during the initial input DMA;
expert loop software-pipelined (down-proj of expert e issued after
up-proj of e+1) with expert up-weights resident in SBUF per layer;
attention runs a cross-pair pipeline (QK matmuls of pair p+1 issued
before AV of pair p; adjacent heads at partition bases 0/64 execute
concurrently in separate PE row groups); router/gating phase is emitted
inside layer-0's attention window so its matmuls fill the PE while the
scalar engine does softmax; PSUM is organized as 4x single-bank + 2x
double-bank tiles; softmax reciprocal is Exp(-Ln(x)) on the scalar
engine (Ln/Exp share one ACT table); E[x^2] for each LayerNorm is
hoisted into the preceding phase; the residual stream is natively f32r
so stat matmuls need no dtype-copy; out-projection is n0-major so LN2
statistics start on the first token half early; elementwise work is
split across vector/gpsimd/scalar to keep the PE streaming.
"""

import numpy as np

B, S = 16, 128
ADIM, TDIM, VDIM = 512, 768, 1024
DE = 512
DEPTH = 4
NH = 8
HD = 64
E = 6
NCLS = 6
EPS = 1e-5
NCORES = 8
BL = B // NCORES          # local batch: 2
NTOK = BL * 3 * S         # 768 tokens/core
NH2 = 384                 # half of token columns (matmul N tile)

_CACHE = {}


def _f32(a):
    return np.ascontiguousarray(np.asarray(a, dtype=np.float32))


def _bf16(a):
    import ml_dtypes
    return np.ascontiguousarray(
        np.asarray(a, dtype=np.float32).astype(ml_dtypes.bfloat16))


def _fp8i(a):
    """[..., K=512, M] -> DoubleRow-interleaved fp8 [..., 128, 2, 2, M]."""
    import ml_dtypes
    a = np.asarray(a, dtype=np.float32)
    lead, (K, M) = a.shape[:-2], a.shape[-2:]
    assert K == 512
    b = a.reshape(*lead, 2, 2, 128, M)
    b = np.moveaxis(b, -2, -4)
    return np.ascontiguousarray(b.astype(ml_dtypes.float8_e4m3fn))


def _split_waits(nc, mybir):
    """This walrus build accepts at most one sync wait / one sync update per
    ISA instruction; Tile's sem assignment can attach several.  Spread the
    extras onto same-engine no-ops."""
    n = 0
    for bb in nc.main_func.blocks:
        insts = list(bb.instructions)
        out = []
        changed = False
        for ins in insts:
            si = ins.sync_info
            if si is None:
                out.append(ins)
                continue
            waits = list(si.on_wait or [])
            updates = list(si.on_update or [])
            post = []
            if len(waits) > 1 or len(updates) > 1:
                for w in waits[:-1]:
                    n += 1
                    nop = mybir.InstNoOp(name=f"xw-{n}", ins=[], outs=[])
                    nop.engine = ins.engine
                    nop.sync_info = mybir.SyncInfo(on_wait=[w], on_update=[])
                    out.append(nop)
                for u in updates[1:]:
                    n += 1
                    nop = mybir.InstNoOp(name=f"xu-{n}", ins=[], outs=[])
                    nop.engine = ins.engine
                    nop.sync_info = mybir.SyncInfo(on_wait=[], on_update=[u])
                    post.append(nop)
                ins.sync_info = mybir.SyncInfo(on_wait=waits[-1:],
                                               on_update=updates[:1])
                changed = True
            out.append(ins)
            out.extend(post)
        if changed:
            bb.instructions[:] = out
    return n


def _build():
    import concourse.bass as bass
    import concourse.mybir as mybir
    import concourse.tile as tile

    f32 = mybir.dt.float32
    fp8 = mybir.dt.float8e4
    f32r = mybir.dt.float32r
    bf16 = mybir.dt.bfloat16
    AF = mybir.ActivationFunctionType
    ALU = mybir.AluOpType
    AX = mybir.AxisListType

    nc = bass.Bass()

    d = {}

    def din(name, shape, dt):
        d[name] = nc.dram_tensor(name, shape, dt, kind="ExternalInput")

    din("ones_s", [128, 128], f32r)
    din("aT", [ADIM, BL * S], f32r)
    din("tT", [TDIM, BL * S], f32r)
    din("vT", [VDIM, BL * S], f32r)
    din("wa", [ADIM, DE], f32r)
    din("wt", [TDIM, DE], f32r)
    din("wv", [VDIM, DE], f32r)
    din("bin_r", [3, 4, 128], f32)
    din("wr1", [3, DE, DE], f32r)
    din("br1_r", [3, 4, 128], f32)
    din("wr2", [3, DE, E], f32r)
    din("br2_b", [3, 128, E], f32)
    din("wqkv", [DEPTH, DE, 3 * DE], bf16)
    din("bqkv_qk", [DEPTH, 8, 128], f32)
    din("bqkv_v", [DEPTH, 1, DE], bf16)
    din("wo", [DEPTH, DE, DE], bf16)
    din("bo_r", [DEPTH, 4, 128], f32)
    din("ws1", [DEPTH, DE, DE], bf16)
    din("bs1_r", [DEPTH, 4, 128], f32)
    din("ws2", [DEPTH, DE, DE], bf16)
    din("we1", [DEPTH, E, DE, DE], bf16)
    din("be1_r", [DEPTH, 24, 128], f32)
    din("we2", [DEPTH, E, DE, DE], bf16)
    din("bmat", [DEPTH, 7, DE], bf16)
    din("wp1", [3 * DE, 3 * DE], bf16)
    din("bp1_row", [1, 3 * DE], bf16)
    din("wh", [3 * DE, NCLS], bf16)
    din("bh_r", [NCLS, 1], f32)
    din("onesrow", [1, 128], bf16)
    din("ident", [128, 128], bf16)
    din("ones_b", [128, 128], bf16)
    din("sel", [E, E * 128], bf16)
    out_d = nc.dram_tensor("out", [NCLS, BL], f32, kind="ExternalOutput")

    with tile.TileContext(nc) as tc:
        _emit(nc, tc, d, out_d, f32, f32r, bf16, fp8, AF, ALU, AX)

    nfix = _split_waits(nc, mybir)
    return nc, nfix


def _emit(nc, tc, d, out_d, f32, f32r, bf16, fp8, AF, ALU, AX):
    from concourse.bass import ds

    import concourse.mybir as mybir
    DR = mybir.MatmulPerfMode.DoubleRow
    dma = nc.sync.dma_start

    def r32(ap):
        return ap.bitcast(f32r)

    cst = tc.alloc_tile_pool(name="cst", bufs=1)
    ph0 = tc.alloc_tile_pool(name="ph0", bufs=1, side="right")  # phase 0/1, released early
    wbig = tc.alloc_tile_pool(name="wbig", bufs=2)
    w512 = tc.alloc_tile_pool(name="w512", bufs=4)
    wexp = tc.alloc_tile_pool(name="wexp", bufs=2)
    px = tc.alloc_tile_pool(name="px", bufs=1)
    ph = tc.alloc_tile_pool(name="ph", bufs=2)
    pxb = tc.alloc_tile_pool(name="pxb", bufs=1)
    pqk = tc.alloc_tile_pool(name="pqk", bufs=1)
    pva = tc.alloc_tile_pool(name="pva", bufs=1)
    pes = tc.alloc_tile_pool(name="pes", bufs=2)
    psm = tc.alloc_tile_pool(name="psm", bufs=2)
    pln = tc.alloc_tile_pool(name="pln", bufs=4)
    peh = tc.alloc_tile_pool(name="peh", bufs=2)
    psml = tc.alloc_tile_pool(name="psml", bufs=4)
    psA = tc.alloc_tile_pool(name="psA", bufs=4, space="PSUM")
    psB = tc.alloc_tile_pool(name="psB", bufs=2, space="PSUM")

    # ---------------- PE warm-up: stream matmuls while input DMAs land ----
    ident = cst.tile([128, 128], bf16)
    dma(ident[:], d["ident"][:])
    bin_r = cst.tile([128, 3, 4], f32)
    dma(bin_r[:], d["bin_r"].rearrange("m c p -> p m c"))
    ones_b = cst.tile([128, 128], bf16)
    dma(ones_b[:], d["ones_b"][:])
    pwu = psA.tile([128, 128], f32, tag="a", name="warmup")
    for _ in range(64):
        nc.tensor.matmul(pwu[:], ident[:], ident[:], start=True, stop=True)

    eps_sb = cst.tile([128, 1], f32)
    nc.gpsimd.memset(eps_sb[:], EPS)
    G_sb = cst.tile([128, E, NTOK], bf16)        # per-expert gate rows, bcast
    gaug = cst.tile([7, NTOK], bf16)             # gates^T rows + ones row
    nc.gpsimd.memset(gaug[:], 1.0)  # rows 0..5 overwritten by gate evictions

    xT = px.tile([128, 4, NTOK], f32r)           # residual stream

    # =======================================================
    # Phase 0: input projections
    # =======================================================
    def in_proj(dname, wname, idim, m):
        kc = idim // 128
        pa = [psA.tile([128, BL * S], f32, tag="a", name=f"pa{i}") for i in range(4)]
        for k in range(kc):
            it = ph0.tile([128, BL * S], f32r, tag="it", bufs=3, name=f"it{m}{k}")
            dma(it[:], d[dname].rearrange("(c p) t -> p c t", p=128)[:, k, :])
            wi = ph0.tile([128, DE], f32r, tag="wi", bufs=3, name=f"wi{m}{k}")
            dma(wi[:], d[wname].rearrange("(c p) o -> p c o", p=128)[:, k, :])
            for mo in range(4):
                nc.tensor.matmul(pa[mo][:], wi[:, ds(mo * 128, 128)], it[:],
                                 start=(k == 0), stop=(k == kc - 1))
        for mo in range(4):
            dst = xT[:, mo, :].rearrange("p (b r) -> p b r", b=BL)[:, :, ds(m * 128, 128)]
            nc.vector.tensor_scalar(out=dst,
                                    in0=pa[mo][:].rearrange("p (b s) -> p b s", b=BL),
                                    scalar1=bin_r[:, m, mo:mo + 1], scalar2=None,
                                    op0=ALU.add)

    in_proj("aT", "wa", ADIM, 0)
    in_proj("tT", "wt", TDIM, 1)
    wq0_t = []
    for k in range(4):
        wqk = wbig.tile([128, 3 * DE], bf16, tag="wq", bufs=4, name=f"wq{k}")
        dma(wqk[:], d["wqkv"][0, ds(k * 128, 128), :])
        wq0_t.append(wqk)
    in_proj("vT", "wv", VDIM, 2)

    # ---------------- remaining constants (after the input DMAs) ----------
    ones_s = cst.tile([128, 128], f32r)          # 1/512 everywhere
    dma(ones_s[:], d["ones_s"][:])
    onesrow = cst.tile([1, 128], bf16)
    dma(onesrow[:], d["onesrow"][:])
    sel_sb = cst.tile([E, E * 128], bf16)
    dma(sel_sb[:], d["sel"][:])
    br1_r = cst.tile([128, 3, 4], f32)
    dma(br1_r[:], d["br1_r"].rearrange("m c p -> p m c"))
    wr2_sb = cst.tile([128, 3, 4, E], f32r)
    dma(wr2_sb[:], d["wr2"].rearrange("m (c p) e -> p m c e", p=128))
    br2_b = cst.tile([128, 3, E], f32)
    dma(br2_b[:], d["br2_b"].rearrange("m p e -> p m e"))
    bqkv_qk = cst.tile([128, DEPTH, 8], f32)
    dma(bqkv_qk[:], d["bqkv_qk"].rearrange("l c p -> p l c"))
    bo_r = cst.tile([128, DEPTH, 4], f32)
    dma(bo_r[:], d["bo_r"].rearrange("l c p -> p l c"))
    bs1_r = cst.tile([128, DEPTH, 4], f32)
    dma(bs1_r[:], d["bs1_r"].rearrange("l c p -> p l c"))
    be1_r = cst.tile([128, DEPTH, 24], f32)
    dma(be1_r[:], d["be1_r"].rearrange("l c p -> p l c"))
    bh_sb = cst.tile([NCLS, 1], f32)
    dma(bh_sb[:], d["bh_r"][:])

    va = pva.tile([128, 6, 8 * 128], bf16)
    nc.gpsimd.memset(
        va[:].rearrange("p t (h w) -> p t h w", h=NH)[:, :, :, 64:128], 1.0)

    # =======================================================
    # layers
    # =======================================================
    def make_xsq():
        return pxb.tile([128, 4, NTOK], bf16, tag="xsq", name="xsq")

    def emit_xsq(xsq, src, n0, eng=None):
        nsl = ds(n0 * NH2, NH2)
        (eng or nc.vector).tensor_tensor(xsq[:, :, nsl], src[:, :, nsl],
                                         src[:, :, nsl], op=ALU.mult)

    def layernorm_n0(src_f32, dst_slice, xsq, n0, all_gpsimd=False):
        if True:
            nsl = ds(n0 * NH2, NH2)
            pst = psB.tile([128, 2, 512], f32, tag="b")
            for k in range(4):
                nc.tensor.matmul(pst[:, 0, 0:NH2], ones_s[:], src_f32[:, k, nsl],
                                 start=(k == 0), stop=(k == 3))
            for k in range(4):
                nc.tensor.matmul(pst[:, 1, 0:NH2], ones_b[:], xsq[:, k, nsl],
                                 start=(k == 0), stop=(k == 3))
            mb = pln.tile([128, NH2], f32, tag="ln", bufs=3)
            nc.scalar.activation(mb[:], pst[:, 0, 0:NH2], AF.Copy)
            qq = pln.tile([128, NH2], f32, tag="ln", bufs=3)
            nc.gpsimd.tensor_tensor(qq[:], mb[:], mb[:], op=ALU.mult)
            vb = pln.tile([128, NH2], f32, tag="ln", bufs=3)
            nc.vector.scalar_tensor_tensor(out=vb[:], in0=pst[:, 1, 0:NH2],
                                           scalar=1.0, in1=qq[:],
                                           op0=ALU.mult, op1=ALU.subtract)
            sq = pln.tile([128, NH2], f32, tag="ln", bufs=3)
            nc.scalar.activation(sq[:], vb[:], AF.Ln, bias=eps_sb[:])
            rb = pln.tile([128, NH2], f32, tag="ln", bufs=3)
            nc.scalar.activation(rb[:], sq[:], AF.Exp, scale=-0.5)
            for k in range(4):
                eng = nc.gpsimd if (all_gpsimd or k >= 2) else nc.vector
                t = pln.tile([128, NH2], f32,
                             tag="lt" if eng is nc.vector else "ltg", bufs=2)
                eng.tensor_tensor(t[:], src_f32[:, k, nsl], mb[:],
                                  op=ALU.subtract)
                eng.tensor_tensor(dst_slice(k, nsl), t[:], rb[:],
                                  op=ALU.mult)

    def layernorm(src_f32, dst_slice, xsq, all_gpsimd=False):
        """src feature-major [128,4,NTOK] fp32 -> dst (via dst_slice(k, nsl)).
        Mean/E[x^2] via ones-matmul (broadcast across partitions); xsq
        precomputed (hoisted into the previous phase for overlap)."""
        for n0 in range(2):
            layernorm_n0(src_f32, dst_slice, xsq, n0, all_gpsimd)

    def emit_phase1():
        # Phase 1: routers + gates (emitted inside layer-0's attention
        # window so router matmuls fill the PE while ACT does softmax)
        rh = ph0.tile([128, 4, 3, BL * S], f32r, tag="hr", bufs=1)
        for m in range(3):
            wr1_sb = wr1_t[m]
            xm = xT[:].rearrange("p c (b mm s) -> p c b mm s", b=BL, mm=3)[:, :, :, m, :]
            for mo in range(4):
                pr = psA.tile([128, BL * S], f32, tag="a")
                for k in range(4):
                    nc.tensor.matmul(pr[:].rearrange("p (b s) -> p b s", b=BL),
                                     wr1_sb[:, k, ds(mo * 128, 128)], xm[:, k, :, :],
                                     start=(k == 0), stop=(k == 3))
                nc.scalar.activation(rh[:, mo, m, :], pr[:], AF.Gelu_apprx_tanh,
                                     bias=br1_r[:, m, mo:mo + 1])

        rlog = cst.tile([128, 6, E], f32)
        for m in range(3):
            for b in range(BL):
                q = b * 3 + m
                prl = psA.tile([128, E], f32, tag="a")
                for k in range(4):
                    nc.tensor.matmul(prl[:], rh[:, k, m, ds(b * 128, 128)],
                                     wr2_sb[:, m, k, :], start=(k == 0), stop=(k == 3))
                nc.vector.tensor_tensor(rlog[:, q, :], prl[:], br2_b[:, m, :], op=ALU.add)

        for q in range(6):
            r = rlog[:, q, :]
            v1 = pln.tile([128, 1], f32, tag="sc")
            nc.vector.tensor_reduce(v1[:], r, op=ALU.max, axis=AX.X)
            m1 = pln.tile([128, E], f32, tag="m6")
            nc.vector.tensor_scalar(out=m1[:], in0=r, scalar1=v1[:], scalar2=None,
                                    op0=ALU.is_equal)
            mk = pln.tile([128, E], f32, tag="m6")
            nc.vector.scalar_tensor_tensor(out=mk[:], in0=m1[:], scalar=-1e9,
                                           in1=r, op0=ALU.mult, op1=ALU.add)
            v2 = pln.tile([128, 1], f32, tag="sc")
            nc.vector.tensor_reduce(v2[:], mk[:], op=ALU.max, axis=AX.X)
            m2 = pln.tile([128, E], f32, tag="m6")
            nc.vector.tensor_scalar(out=m2[:], in0=mk[:], scalar1=v2[:], scalar2=None,
                                    op0=ALU.is_equal)
            dd = pln.tile([128, 1], f32, tag="sc")
            nc.vector.tensor_tensor(dd[:], v1[:], v2[:], op=ALU.subtract)
            g1 = pln.tile([128, 1], f32, tag="sc")
            nc.scalar.activation(g1[:], dd[:], AF.Sigmoid)
            g2 = pln.tile([128, 1], f32, tag="sc")
            nc.vector.tensor_scalar(out=g2[:], in0=g1[:], scalar1=-1.0, scalar2=1.0,
                                    op0=ALU.mult, op1=ALU.add)
            gm2 = pln.tile([128, E], f32, tag="m6")
            nc.vector.tensor_scalar(out=gm2[:], in0=m2[:], scalar1=g2[:], scalar2=None,
                                    op0=ALU.mult)
            gq = pln.tile([128, E], bf16, tag="m6b")
            nc.vector.scalar_tensor_tensor(out=gq[:], in0=m1[:], scalar=g1[:],
                                           in1=gm2[:], op0=ALU.mult, op1=ALU.add)
            pt = psA.tile([E, 128], bf16, tag="a", name="ptg")
            nc.tensor.transpose(pt[:], gq[:], ident[:])
            nc.scalar.activation(gaug[0:6, ds(q * 128, 128)], pt[:], AF.Copy)

        for e in range(E):
            for n0 in range(2):
                pg = psA.tile([128, NH2], f32, tag="a")
                nc.tensor.matmul(pg[:], sel_sb[:, ds(e * 128, 128)],
                                 gaug[0:6, ds(n0 * NH2, NH2)], start=True, stop=True)
                nc.scalar.activation(G_sb[:, e, ds(n0 * NH2, NH2)], pg[:], AF.Copy)

        ph0.release()

        wp1p = tc.alloc_tile_pool(name="wp1p", bufs=1, side="right")
        return wp1p

    for layer in range(DEPTH):
        if layer == 0:
            wq_t = wq0_t
        else:
            wq_t = []
            for k in range(4):
                wqk = wbig.tile([128, 3 * DE], bf16, tag="wq", bufs=4,
                                name=f"wq{k}")
                dma(wqk[:], d["wqkv"][layer, ds(k * 128, 128), :])
                wq_t.append(wqk)
        bqv = psml.tile([1, DE], bf16, tag="bqv", bufs=1)
        dma(bqv[:], d["bqkv_v"][layer])
        bmat_sb = psml.tile([7, DE], bf16, tag="bm", bufs=1)
        dma(bmat_sb[:], d["bmat"][layer])
        wo_sb = w512.tile([128, 4, DE], bf16, tag="w")
        dma(wo_sb[:], d["wo"][layer].rearrange("(c p) o -> p c o", p=128))
        ws1_sb = w512.tile([128, 4, DE], bf16, tag="w")
        dma(ws1_sb[:], d["ws1"][layer].rearrange("(c p) o -> p c o", p=128))
        ws2_sb = w512.tile([128, 4, DE], bf16, tag="w")
        dma(ws2_sb[:], d["ws2"][layer].rearrange("(c p) o -> p c o", p=128))
        if layer == 0:
            wr1_t = []
            for m in range(3):
                wr1_sb = ph0.tile([128, 4, DE], f32r, tag="wr", bufs=2,
                                  name="wr1_sb")
                dma(wr1_sb[:], d["wr1"][m].rearrange("(c p) o -> p c o", p=128))
                wr1_t.append(wr1_sb)

        # expert weights: resident for the whole layer, loaded once
        we_t = []
        for e in range(E):
            w1 = wexp.tile([128, 4, DE], bf16, tag="we", bufs=7, name=f"w1_{e}")
            dma(w1[:], d["we1"][layer, e].rearrange("(c p) o -> p c o", p=128))
            we_t.append(w1)
        if layer == DEPTH - 1:
            bp1_row = wp1p.tile([1, 3 * DE], bf16, tag="bp1")
            dma(bp1_row[:], d["bp1_row"][:])
            wp1_sb = []
            for g in range(4):
                wpg = wp1p.tile([128, 3, 3 * DE], bf16, tag="wp", bufs=3,
                                name=f"wp{g}")
                dma(wpg[:], d["wp1"].rearrange("(c p) o -> p c o", p=128)
                    [:, ds(g * 3, 3), :])
                wp1_sb.append(wpg)

        # ---- LN1 ----
        if layer == 0:
            xsq_next = make_xsq()
            emit_xsq(xsq_next, xT, 0, nc.gpsimd)
            emit_xsq(xsq_next, xT, 1, nc.gpsimd)
        hT = ph.tile([128, 4, NTOK], bf16, tag="h", bufs=2, name="hT")
        layernorm(xT, lambda k, nsl: hT[:, k, nsl], xsq_next)

        # ---- qkv: q,k feature-major ----
        qkT = pqk.tile([128, 8, NTOK], bf16)
        for mo in range(8):
            pq2 = [psA.tile([128, NH2], f32, tag="a", name=f"pq{i}") for i in range(2)]
            for k in range(4):
                for n0 in range(2):
                    nc.tensor.matmul(pq2[n0][:], wq_t[k][:, ds(mo * 128, 128)],
                                     hT[:, k, ds(n0 * NH2, NH2)],
                                     start=(k == 0), stop=(k == 3))
            for n0 in range(2):
                nc.vector.tensor_scalar(out=qkT[:, mo, ds(n0 * NH2, NH2)],
                                        in0=pq2[n0][:],
                                        scalar1=bqkv_qk[:, layer, mo:mo + 1],
                                        scalar2=None, op0=ALU.add)
        # ---- v token-major, bias via rank-1, into V_aug (pairs of tq) ----
        for tp in range(3):
            pv = psB.tile([128, 2, 512], f32, tag="b")
            for j in range(2):
                tq = tp * 2 + j
                for k in range(4):
                    nc.tensor.matmul(pv[:, j, :], hT[:, k, ds(tq * 128, 128)],
                                     wq_t[k][:, ds(2 * DE, DE)],
                                     start=(k == 0), stop=False)
                nc.tensor.matmul(pv[:, j, :], onesrow[0:1, 0:128], bqv[:],
                                 start=False, stop=True)
            for j in range(2):
                tq = tp * 2 + j
                dst = va[:, tq, :].rearrange("p (h w) -> p h w", h=NH)[:, :, 0:64]
                nc.scalar.activation(dst,
                                     pv[:, j, :].rearrange("p (h e) -> p h e", h=NH),
                                     AF.Copy)

        # ---- attention per (b, head): cross-pair software pipeline ----
        oT = ph.tile([128, 4, NTOK], bf16, tag="h", bufs=2, name="oT")
        prs = [(b, hp) for b in range(BL) for hp in range(NH // 2)]
        est = {}

        def qk_stage(pi):
            b, hp = prs[pi]
            pks, ess = [], []
            for h in (2 * hp, 2 * hp + 1):
                r0 = 64 * (h % 2)
                ck = 4 + h // 2
                qs = qkT[ds(r0, 64), h // 2, ds(b * 384, 384)]
                pk2 = psB.tile([128, 2, 512], f32, tag="b")
                for i in range(2):
                    nc.tensor.matmul(pk2[:, i, 0:NH2],
                                     qkT[ds(r0, 64), ck,
                                         ds(b * 384 + i * 128, 128)],
                                     qs, start=True, stop=True)
                pk1 = psA.tile([128, NH2], f32, tag="a")
                nc.tensor.matmul(pk1[:],
                                 qkT[ds(r0, 64), ck, ds(b * 384 + 256, 128)],
                                 qs, start=True, stop=True)
                pks.append((pk2, pk1))
            for j in range(2):
                pk2, pk1 = pks[j]
                es = pes.tile([128, 3, NH2], bf16, name=f"es{j}")
                nc.scalar.activation(es[:, 0:2, :], pk2[:, :, 0:NH2],
                                     AF.Exp, scale=0.125)
                nc.scalar.activation(es[:, 2, :], pk1[:], AF.Exp, scale=0.125)
                ess.append(es)
            est[pi] = ess

        def av_stage(pi):
            b, hp = prs[pi]
            ess = est[pi]
            spair = psm.tile([128, NH2], f32, tag="s", bufs=3)
            opair = psm.tile([128, NH2], f32, tag="o", bufs=1)
            pos = []
            for j, h in enumerate((2 * hp, 2 * hp + 1)):
                r0 = 64 * (h % 2)
                es = ess[j]
                po = psA.tile([128, NH2], f32, tag="a", name=f"po{h%2}")
                for i in range(3):
                    nc.tensor.matmul(po[:], va[:, b * 3 + i, ds(h * 128, 128)],
                                     es[:, i, :], start=(i == 0), stop=(i == 2))
                nc.vector.tensor_copy(spair[ds(r0, 64), :], po[ds(64, 64), :])
                pos.append(po)
            for j, h in enumerate((2 * hp, 2 * hp + 1)):
                r0 = 64 * (h % 2)
                nc.vector.tensor_copy(opair[ds(r0, 64), :], pos[j][ds(0, 64), :])
            sln = psm.tile([128, NH2], f32, tag="s", bufs=3)
            nc.scalar.activation(sln[:], spair[:], AF.Ln)
            rcp = psm.tile([128, NH2], f32, tag="s", bufs=3)
            nc.scalar.activation(rcp[:], sln[:], AF.Exp, scale=-1.0)
            for h in (2 * hp, 2 * hp + 1):
                r0 = 64 * (h % 2)
                nc.gpsimd.tensor_tensor(oT[ds(r0, 64), h // 2, ds(b * 384, 384)],
                                        opair[ds(r0, 64), :], rcp[ds(r0, 64), :],
                                        op=ALU.mult)

        qk_stage(0)
        for pi in range(1, len(prs)):
            qk_stage(pi)
            av_stage(pi - 1)
        av_stage(len(prs) - 1)

        if layer == 0:
            wp1p = emit_phase1()

        # ---- attention out-projection + residual (n0-major so the n0=0
        # half of xT finalizes early and LN2 stats can start) ----
        xsq2 = make_xsq()
        for n0 in range(2):
            nsl = ds(n0 * NH2, NH2)
            for mo in range(4):
                pp = psA.tile([128, NH2], f32, tag="a", name="pp")
                for k in range(4):
                    nc.tensor.matmul(pp[:], wo_sb[:, k, ds(mo * 128, 128)],
                                     oT[:, k, nsl], start=(k == 0), stop=(k == 3))
                nc.vector.scalar_tensor_tensor(out=xT[:, mo, nsl], in0=pp[:],
                                               scalar=bo_r[:, layer, mo:mo + 1],
                                               in1=xT[:, mo, nsl],
                                               op0=ALU.add, op1=ALU.add)
            emit_xsq(xsq2, xT, n0)

        # ---- LN2 ----
        h2 = ph.tile([128, 4, NTOK], bf16, tag="h", bufs=2, name="h2")
        layernorm(xT, lambda k, nsl: h2[:, k, nsl], xsq2)

        # ---- MoE: shared expert + 6 gated experts, software-pipelined ----
        xsq_next = make_xsq()
        for n0 in range(2):
            nsl = ds(n0 * NH2, NH2)
            pd = [psA.tile([128, NH2], f32, tag="a", name=f"pd{i}") for i in range(4)]
            for mo in range(4):
                nc.tensor.matmul(pd[mo][:], bmat_sb[:, ds(mo * 128, 128)],
                                 gaug[:, nsl], start=True, stop=False)
            su = peh.tile([128, 4, NH2], bf16, tag="eh")
            for p in range(2):
                pu = psB.tile([128, 2, 512], f32, tag="b")
                for j in range(2):
                    mo = p * 2 + j
                    for k in range(4):
                        nc.tensor.matmul(pu[:, j, 0:NH2],
                                         ws1_sb[:, k, ds(mo * 128, 128)],
                                         h2[:, k, nsl], start=(k == 0), stop=(k == 3))
                    nc.scalar.activation(su[:, mo, :], pu[:, j, 0:NH2],
                                         AF.Gelu_apprx_tanh,
                                         bias=bs1_r[:, layer, mo:mo + 1])
            for mo in range(4):
                for k in range(4):
                    nc.tensor.matmul(pd[mo][:], ws2_sb[:, k, ds(mo * 128, 128)],
                                     su[:, k, :], start=False, stop=False)

            ehs = []

            def up_expert(e):
                w1 = we_t[e]
                eh = peh.tile([128, 4, NH2], bf16, tag="eh", name=f"eh{e%2}")
                for p in range(2):
                    pu = psB.tile([128, 2, 512], f32, tag="b")
                    for j in range(2):
                        mo = p * 2 + j
                        for k in range(4):
                            nc.tensor.matmul(pu[:, j, 0:NH2],
                                             w1[:, k, ds(mo * 128, 128)],
                                             h2[:, k, nsl],
                                             start=(k == 0), stop=(k == 3))
                        nc.scalar.activation(
                            eh[:, mo, :], pu[:, j, 0:NH2],
                            AF.Gelu_apprx_tanh,
                            bias=be1_r[:, layer, e * 4 + mo:e * 4 + mo + 1])
                    eng = (nc.vector if (p == 0 or
                           (layer == DEPTH - 1 and n0 == 1)) else nc.gpsimd)
                    for j in range(2):
                        mo = p * 2 + j
                        eng.tensor_tensor(eh[:, mo, :], eh[:, mo, :],
                                          G_sb[:, e, nsl], op=ALU.mult)
                ehs.append(eh)

            def down_expert(e):
                w2 = wexp.tile([128, 4, DE], bf16, tag="we2", bufs=3, name=f"w2_{e}")
                dma(w2[:], d["we2"][layer, e].rearrange("(c p) o -> p c o", p=128))
                eh = ehs[e]
                last = (e == E - 1)
                for mo in range(4):
                    for k in range(4):
                        nc.tensor.matmul(pd[mo][:], w2[:, k, ds(mo * 128, 128)],
                                         eh[:, k, :], start=False,
                                         stop=(last and k == 3))

            up_expert(0)
            for e in range(1, E):
                up_expert(e)
                down_expert(e - 1)
            down_expert(E - 1)

            for mo in range(4):
                nc.vector.tensor_tensor(xT[:, mo, nsl], pd[mo][:],
                                        xT[:, mo, nsl], op=ALU.add)
            emit_xsq(xsq_next, xT, n0)

    # =======================================================
    # final LN + mean-pool + head
    # =======================================================
    fT = ph.tile([128, 4, NTOK], bf16, tag="h", bufs=2, name="fT")
    layernorm(xT, lambda k, nsl: fT[:, k, nsl], xsq_next, all_gpsimd=True)

    pooled = wp1p.tile([128, 24], f32, tag="pool")
    pooledb = wp1p.tile([128, 24], bf16, tag="poolb")
    pview = pooled[:].rearrange("p (m k b) -> p b m k", m=3, k=4, b=BL)
    for k in range(4):
        for b in range(BL):
            nc.vector.tensor_reduce(
                pview[:, b, :, k],
                fT[:, k, ds(b * 384, 384)].rearrange("p (m s) -> p m s", m=3),
                op=ALU.add, axis=AX.X)
    nc.vector.tensor_scalar(out=pooledb[:], in0=pooled[:], scalar1=1.0 / S,
                            scalar2=None, op0=ALU.mult)

    # fused = relu(pooled @ Wp1 + bp1), token-major [BL, 1536]
    pfs = [psA.tile([BL, DE], f32, tag="a", name=f"pfs{i}") for i in range(3)]
    for kj in range(12):
        for ns in range(3):
            nc.tensor.matmul(pfs[ns][:], pooledb[:, ds(kj * 2, BL)],
                             wp1_sb[kj // 3][:, kj % 3, ds(ns * DE, DE)],
                             start=(kj == 0), stop=False)
    for ns in range(3):
        nc.tensor.matmul(pfs[ns][:], onesrow[0:1, 0:BL],
                         bp1_row[:, ds(ns * DE, DE)], start=False, stop=True)
    fused_sb = wp1p.tile([BL, 3 * DE], bf16, tag="fus")
    for ns in range(3):
        nc.scalar.activation(fused_sb[:, ds(ns * DE, DE)], pfs[ns][:], AF.Relu)

    fusedT = wp1p.tile([128, 12, BL], bf16, tag="fusT")
    for kj in range(12):
        pft = psB.tile([128, BL], bf16, tag="b", name="pft")
        nc.tensor.transpose(pft[:], fused_sb[:, ds(kj * 128, 128)], ident[0:BL, 0:BL])
        nc.scalar.activation(fusedT[:, kj, :], pft[:], AF.Copy)

    wh_sb = w512.tile([128, 12, NCLS], bf16, tag="w")
    dma(wh_sb[:], d["wh"].rearrange("(c p) o -> p c o", p=128))
    pout = psA.tile([NCLS, BL], f32, tag="a")
    for kj in range(12):
        nc.tensor.matmul(pout[:], wh_sb[:, kj, :], fusedT[:, kj, :],
                         start=(kj == 0), stop=(kj == 11))
    osb = wp1p.tile([NCLS, BL], f32, tag="osb")
    nc.scalar.activation(osb[:], pout[:], AF.Identity, bias=bh_sb[:, 0:1])
    dma(out_d[:], osb[:])

    for pool in [psB, psA, psml, peh, pln, psm, pes, pva, pqk, pxb, ph, px,
                 wexp, w512, wbig, wp1p, cst]:
        pool.release()


def _host_prep(inputs):
    p = {k: np.asarray(v) for k, v in inputs.items()}

    shared = {}
    shared["wa"] = _f32(p["Wa"])
    shared["wt"] = _f32(p["Wt"])
    shared["wv"] = _f32(p["Wv"])
    shared["bin_r"] = _f32(np.stack([p["ba"].reshape(4, 128),
                                     p["bt"].reshape(4, 128),
                                     p["bv"].reshape(4, 128)]))
    shared["wr1"] = _f32(p["Wr1"])
    shared["br1_r"] = _f32(np.asarray(p["br1"]).reshape(3, 4, 128))
    shared["wr2"] = _f32(p["Wr2"])
    shared["br2_b"] = _f32(np.broadcast_to(np.asarray(p["br2"])[:, None, :],
                                           (3, 128, E)))
    shared["wqkv"] = _bf16(p["Wqkv"])
    shared["bqkv_qk"] = _f32(np.asarray(p["bqkv"])[:, :1024].reshape(DEPTH, 8, 128))
    shared["bqkv_v"] = _bf16(np.asarray(p["bqkv"])[:, 1024:].reshape(DEPTH, 1, DE))
    shared["wo"] = _bf16(p["Wo"])
    shared["bo_r"] = _f32(np.asarray(p["bo"]).reshape(DEPTH, 4, 128))
    shared["ws1"] = _bf16(p["Ws1"])
    shared["bs1_r"] = _f32(np.asarray(p["bs1"]).reshape(DEPTH, 4, 128))
    shared["ws2"] = _bf16(p["Ws2"])
    shared["we1"] = _bf16(p["We1"])
    shared["be1_r"] = _f32(np.asarray(p["be1"]).reshape(DEPTH, 24, 128))
    shared["we2"] = _bf16(p["We2"])
    bmat = np.concatenate([np.asarray(p["be2"]),
                           np.asarray(p["bs2"])[:, None, :]], axis=1)
    shared["bmat"] = _bf16(bmat)
    shared["wp1"] = _bf16(p["Wp1"])
    shared["bp1_row"] = _bf16(np.asarray(p["bp1"]).reshape(1, 3 * DE))
    shared["wh"] = _bf16(p["Wh"])
    shared["bh_r"] = _f32(np.asarray(p["bh"]).reshape(NCLS, 1))
    shared["ones_s"] = _f32(np.full((128, 128), 1.0 / DE, np.float32))
    shared["ones_b"] = _bf16(np.full((128, 128), 1.0 / DE, np.float32))
    shared["onesrow"] = _bf16(np.ones((1, 128), np.float32))
    shared["ident"] = _bf16(np.eye(128, dtype=np.float32))
    sel = np.zeros((E, E * 128), np.float32)
    for e in range(E):
        sel[e, e * 128:(e + 1) * 128] = 1.0
    shared["sel"] = _bf16(sel)

    in_maps = []
    for c in range(NCORES):
        sl = slice(BL * c, BL * (c + 1))
        m = dict(shared)
        m["aT"] = _f32(np.asarray(p["audio"])[sl].transpose(2, 0, 1)
                       .reshape(ADIM, BL * S))
        m["tT"] = _f32(np.asarray(p["text"])[sl].transpose(2, 0, 1)
                       .reshape(TDIM, BL * S))
        m["vT"] = _f32(np.asarray(p["visual"])[sl].transpose(2, 0, 1)
                       .reshape(VDIM, BL * S))
        in_maps.append(m)
    return in_maps


def kernel(**inputs):
    from concourse.bass_utils import run_bass_kernel_spmd

    if "nc" not in _CACHE:
        _CACHE["nc"] = _build()
    nc, _ = _CACHE["nc"]

    in_maps = _host_prep(inputs)
    res = run_bass_kernel_spmd(nc, in_maps, core_ids=list(range(NCORES)))
    out = np.empty((B, NCLS), np.float32)
    for c in range(NCORES):
        out[BL * c: BL * (c + 1)] = res.results[c]["out"].T
    return out
